# revision 22
# baseline (speedup 1.0000x reference)
"""Trainium2 8-core Bass kernel for nn_AttAggFME.

Sharding: core c = (batch b=c//2, half h=c%2). Every core runs IDENTICAL
code with "top half" geometry; h=1 cores receive vertically flipped data
(host flips rows) and dy-tap-flipped conv weights, and their outputs are
flipped back on the host.

Per core: 2048 attention queries (own rows), motion encoder on own rows +
16 halo rows (halo comes free from host-sliced inputs), conv1..4 with halo
recompute. Two intra-pair AllGathers: V^T (full 4096 values for attn@V) and
16 halo rows of motion_fea_global before conv1. Cross-core row-order and
rank-offset asymmetries are handled with reverse-on-send plus host-supplied
0/1 selector masks.

All matmuls bf16 with f32 PSUM accumulation; softmax logits stay f32 in
PSUM, exp on ScalarE; denominators via ones-matmul (scores are computed
transposed [keys, queries] so attn@V needs no transposes).
"""

import numpy as np
import ml_dtypes

import concourse.bass as bass
import concourse.tile as tile
from concourse import mybir, bacc
from concourse.bass_utils import run_bass_kernel_spmd

F32 = mybir.dt.float32
BF16 = mybir.dt.bfloat16
AF = mybir.ActivationFunctionType

N_CORES = 8
H = W = 64
HW = H * W
D = 128
NET = 128
GROUPS = [[0, 1], [2, 3], [4, 5], [6, 7]]

# Row ranges (own-frame; every core is "top half": own rows [0:32))
ROWS = {
    "own":   (0, 32),
    "mf":    (0, 48),   # motion_fea (conv1 needs 16 halo rows)
    "mfg":   (0, 48),   # motion_fea_global (own 0:32 + partner halo 32:48)
    "c1o":   (0, 47),   # conv1 out
    "c2o":   (0, 43),   # conv2 out
    "c3o":   (0, 35),   # conv3 out
    "x":     (0, 34),   # conv4 out
    "cor":   (0, 49),   # convc2 out
    "c1out": (0, 50),   # convc1 out
    "f1out": (0, 50),   # convf1 out
    "flo":   (0, 49),   # convf2 out
    "corr":  (0, 50),   # corr input rows shipped
    "flow":  (0, 53),   # flow input rows shipped
    "net":   (0, 48),   # net / upfeat input rows shipped
}
RHO = (0, 33)       # deconv x-row parity range
UP = (0, 66)        # up rows stored
UP_OUT = (0, 64)    # up rows output / trade out rows

TAPS3 = [(ty - 1, tx - 1) for ty in range(3) for tx in range(3)]


def _nrows(key):
    lo, hi = ROWS[key]
    return hi - lo


PARAM_SPECS = [
    # activations (f32, own-frame)
    ("inp",  [128, HW], F32),
    ("net",  [128, _nrows("net") * W], F32),
    ("upf",  [128, _nrows("net") * W], F32),
    ("corr", [81, _nrows("corr") * W], F32),
    ("f1stk", [98, _nrows("f1out") * W], BF16),
    # weights (bf16) / biases (f32) / selectors
    ("w_qk", [128, 256], BF16),
    ("w_c1", [81, 256], BF16), ("b_c1a", [128, 1], F32), ("b_c1b", [128, 1], F32),
    ("w_c2", [128, 9 * 2 * 192], BF16), ("b_c2a", [128, 1], F32), ("b_c2b", [64, 1], F32),
    ("w_f1", [98, 128], BF16), ("b_f1", [128, 1], F32),
    ("w_f2", [128, 9 * 64], BF16), ("b_f2p", [128, 1], F32),
    ("w_cm", [128, 9 * 2 * 126], BF16), ("b_cm", [126, 1], F32),
    ("w_v", [128, 128], BF16),
    ("w_1", [128, 9 * 4 * 128], BF16), ("b_1", [128, 1], F32),
    ("w_2", [128, 9 * 96], BF16), ("b_2", [96, 1], F32),
    ("w_3", [96, 9 * 64], BF16), ("b_3", [64, 1], F32),
    ("w_4", [64, 9 * 32], BF16), ("b_4", [32, 1], F32),
    ("w_dc", [96, 3 * 64], BF16), ("b_dc", [64, 1], F32),
    ("w_tr_a", [128, 64], BF16), ("w_tr_b", [16, 64], BF16), ("b_tr", [64, 1], F32),
    ("w_fm", [96, 3 * 3], BF16), ("b_fm", [3, 1], F32),
    ("sel0", [128, 1], F32), ("sel1", [128, 1], F32),
]


# ---------------------------------------------------------------------------
# Host-side weight prep


def prep_weights(inp, flip):
    """All conv weights packed for the kernel; flip=True mirrors dy taps."""
    def bf(a):
        return np.ascontiguousarray(a, dtype=np.float32).astype(ml_dtypes.bfloat16)

    def col(a, n=None, off=0):
        a = np.asarray(a, np.float32).reshape(-1)
        n = n or a.shape[0]
        out = np.zeros((n, 1), np.float32)
        out[off:off + a.shape[0], 0] = a
        return out

    def fl(wt):  # [Cout, Cin, kh, kw] -> dy mirrored
        return wt[:, :, ::-1, :] if flip else wt

    w = {}
    qk = inp["att_to_qk_w"][:, :, 0, 0].astype(np.float64).T.copy()  # [128, 256]
    qk[:, :D] *= D ** -0.5
    w["w_qk"] = bf(qk)

    w["w_c1"] = bf(inp["convc1_w"][:, :, 0, 0].T)
    w["b_c1a"] = col(inp["convc1_b"][:128])
    w["b_c1b"] = col(inp["convc1_b"][128:])

    def conv3x3(wt):
        wt = fl(wt)
        Cout, Cin = wt.shape[:2]
        nkt = (Cin + 127) // 128
        K = 128 if nkt > 1 else Cin
        blocks = []
        for (dy, dx) in TAPS3:
            for kt in range(nkt):
                cs, ce = kt * 128, min(Cin, (kt + 1) * 128)
                blk = np.zeros((K, Cout), np.float64)
                blk[: ce - cs] = wt[:, cs:ce, dy + 1, dx + 1].T
                blocks.append(blk)
        return bf(np.concatenate(blocks, axis=1))

    w["w_c2"] = conv3x3(inp["convc2_w"])
    w["b_c2a"] = col(inp["convc2_b"][:128])
    w["b_c2b"] = col(inp["convc2_b"][128:])

    f1 = fl(inp["convf1_w"])
    lf1 = np.zeros((98, 128), np.float64)
    for ty in range(7):
        for tx in range(7):
            for ci in range(2):
                lf1[2 * (ty * 7 + tx) + ci] = f1[:, ci, ty, tx]
    w["w_f1"] = bf(lf1)
    w["b_f1"] = col(inp["convf1_b"])

    w["w_f2"] = conv3x3(inp["convf2_w"])
    w["b_f2p"] = col(inp["convf2_b"], n=128, off=64)

    w["w_cm"] = conv3x3(inp["conv_motion_w"])
    w["b_cm"] = col(inp["conv_motion_b"])

    gv = float(np.asarray(inp["gamma"]).reshape(-1)[0]) * inp["agg_to_v_w"][:, :, 0, 0]
    w["w_v"] = bf(gv.T)

    w["w_1"] = conv3x3(inp["conv1_w"]); w["b_1"] = col(inp["conv1_b"])
    w["w_2"] = conv3x3(inp["conv2_w"]); w["b_2"] = col(inp["conv2_b"])
    w["w_3"] = conv3x3(inp["conv3_w"]); w["b_3"] = col(inp["conv3_b"])
    w["w_4"] = conv3x3(inp["conv4_w"]); w["b_4"] = col(inp["conv4_b"])

    # deconv: ConvTranspose2d(k4,s2,p1) w [32ci,16co,4,4]; flip: kh -> 3-kh
    dw = inp["upfeat_w"].astype(np.float64)
    if flip:
        dw = dw[:, :, ::-1, :]
    ldc = np.zeros((3, 96, 64), np.float64)
    for dyi, dy in enumerate((-1, 0, 1)):
        for dxi, dx in enumerate((-1, 0, 1)):
            for a in range(2):
                kh = a + 1 - 2 * dy
                if not (0 <= kh < 4) or (a == 0 and dy not in (0, -1)) or (a == 1 and dy not in (0, 1)):
                    continue
                for b in range(2):
                    kw = b + 1 - 2 * dx
                    if not (0 <= kw < 4) or (b == 0 and dx not in (0, -1)) or (b == 1 and dx not in (0, 1)):
                        continue
                    ldc[dyi, dxi * 32:dxi * 32 + 32, (a * 2 + b) * 16:(a * 2 + b) * 16 + 16] = dw[:, :, kh, kw]
    w["w_dc"] = bf(ldc.transpose(1, 0, 2).reshape(96, 3 * 64))
    w["b_dc"] = col(np.tile(np.asarray(inp["upfeat_b"], np.float64), 4))

    tw = fl(inp["trade_w"]).astype(np.float64)
    la = np.zeros((128, 64), np.float64)
    lb = np.zeros((16, 64), np.float64)
    for ty in range(3):
        for tx in range(3):
            t = ty * 3 + tx
            blk = tw[:, :, ty, tx].T
            if t < 8:
                la[t * 16:(t + 1) * 16] = blk
            else:
                lb[:] = blk
    w["w_tr_a"] = bf(la)
    w["w_tr_b"] = bf(lb)
    w["b_tr"] = col(inp["trade_b"])

    fw = fl(inp["flow_w"]).astype(np.float64)
    mw = fl(inp["mask_w"]).astype(np.float64)
    lfm = np.zeros((3, 96, 3), np.float64)
    for dyi, dy in enumerate((-1, 0, 1)):
        for dxi, dx in enumerate((-1, 0, 1)):
            lfm[dyi, dxi * 32:dxi * 32 + 32, 0:2] = fw[:, :, dy + 1, dx + 1].T
            lfm[dyi, dxi * 32:dxi * 32 + 32, 2] = mw[0, :, dy + 1, dx + 1]
    w["w_fm"] = bf(lfm.transpose(1, 0, 2).reshape(96, 3 * 3))
    w["b_fm"] = col(np.concatenate([np.asarray(inp["flow_b"]), np.asarray(inp["mask_b"])]))
    return w


# ---------------------------------------------------------------------------
# Padded spatial SBUF tensors


class PadT:
    """SBUF tile [C, rtot, stride] with `pad` zeroed margin rows/cols; g0 =
    own-frame row of the first real row."""

    def __init__(self, nc, pool, name, C, key_or_range, pad, dtype=BF16, tag=None):
        g0, g1 = ROWS[key_or_range] if isinstance(key_or_range, str) else key_or_range
        self.nc, self.C, self.g0, self.rows, self.pad = nc, C, g0, g1 - g0, pad
        self.stride = W + 2 * pad
        self.rtot = self.rows + 2 * pad
        self.t = pool.tile([C, self.rtot, self.stride], dtype, name=name,
                           tag=tag or name)

    def zero_margins(self):
        nc, p = self.nc, self.pad
        if p == 0:
            return
        nc.gpsimd.memset(self.t[:, 0:p, :], 0.0)
        nc.gpsimd.memset(self.t[:, self.rtot - p:self.rtot, :], 0.0)
        nc.gpsimd.memset(self.t[:, p:p + self.rows, 0:p], 0.0)
        nc.gpsimd.memset(self.t[:, p:p + self.rows, self.stride - p:self.stride], 0.0)

    def ap(self, r_lo, r_hi, dy=0, dx=0, c_lo=0, c_hi=None):
        c_hi = self.C if c_hi is None else c_hi
        a = r_lo - self.g0 + self.pad + dy
        b = r_hi - self.g0 + self.pad + dy
        assert 0 <= a and b <= self.rtot, (r_lo, r_hi, dy, self.g0, self.rows)
        assert 0 <= self.pad + dx and dx <= self.pad
        return self.t[c_lo:c_hi, a:b, self.pad + dx:self.pad + dx + W]


def chunks(lo, hi, step):
    r = lo
    while r < hi:
        yield r, min(hi, r + step)
        r += step


# ---------------------------------------------------------------------------
# Graph build


def build_nc():
    nc = bacc.Bacc()
    P = {}
    for name, shape, dt in PARAM_SPECS:
        P[name] = nc.declare_dram_parameter(name, shape, dt, isOutput=False)
    P["tradeoff_out"] = nc.declare_dram_parameter("tradeoff_out", [64, 64 * 128], F32, isOutput=True)
    P["up_out"] = nc.declare_dram_parameter("up_out", [16, 64 * 128], F32, isOutput=True)
    P["flow_out"] = nc.declare_dram_parameter("flow_out", [2, 32 * W], F32, isOutput=True)
    P["mask_out"] = nc.declare_dram_parameter("mask_out", [1, 32 * W], F32, isOutput=True)

    with tile.TileContext(nc) as tc:
        _emit(nc, tc, P)
    nc.finalize()
    return nc


def _emit(nc, tc, P):
    from contextlib import ExitStack
    ctx = ExitStack()
    pool = ctx.enter_context(tc.tile_pool(name="main", bufs=1))
    stage = ctx.enter_context(tc.tile_pool(name="stage", bufs=3))
    small = ctx.enter_context(tc.tile_pool(name="small", bufs=2))
    dram = ctx.enter_context(tc.tile_pool(name="dram", bufs=1, space="DRAM"))
    ps_conv = ctx.enter_context(tc.tile_pool(name="ps_conv", bufs=2, space="PSUM"))
    ps_scores = ctx.enter_context(tc.tile_pool(name="ps_scores", bufs=2, space="PSUM"))
    ps_agg = ctx.enter_context(tc.tile_pool(name="ps_agg", bufs=2, space="PSUM"))

    scope = nc.named_scope

    # ---- weights, emitted in order of first use --------------------------
    WS = {}

    def loadw(*names):
        for name in names:
            spec = next(s for s in PARAM_SPECS if s[0] == name)
            t = pool.tile(spec[1], spec[2], name=f"sb_{name}")
            nc.gpsimd.dma_start(t[:], P[name][:])
            WS[name] = t

    def wslice(name, tap, kt, nkt, m_lo, m_hi, Cout, K=128):
        base = (tap * nkt + kt) * Cout
        return WS[name][0:K, base + m_lo:base + m_hi]

    # ---- input casts -----------------------------------------------------
    def load_cast_flat(dst, dram_p, C, total):
        for lo, hi in chunks(0, total, 1024):
            st = stage.tile([C, hi - lo], F32, tag="f32stage")
            nc.sync.dma_start(st[0:C, 0:hi - lo], dram_p[0:C, lo:hi])
            nc.vector.tensor_copy(dst[0:C, lo:hi], st[0:C, 0:hi - lo])

    def load_cast_padt(dst, dram_p, C, key):
        lo, hi = ROWS[key]
        for r0, r1 in chunks(lo, hi, 16):
            st = stage.tile([C, (r1 - r0) * W], F32, tag="f32stage")
            nc.sync.dma_start(st[0:C, 0:(r1 - r0) * W],
                              dram_p[0:C, (r0 - lo) * W:(r1 - lo) * W])
            nc.vector.tensor_copy(
                dst.ap(r0, r1, c_lo=0, c_hi=C),
                st[0:C, 0:(r1 - r0) * W].rearrange("c (r w) -> c r w", w=W))

    loadw("w_c1", "b_c1a", "b_c1b")
    crows = _nrows("corr")
    corr_bf = pool.tile([81, crows * W], BF16, name="corr_bf", tag="T_corr_dsum_xs")
    load_cast_flat(corr_bf, P["corr"], 81, crows * W)
    f1rows0 = _nrows("f1out")
    f1stack = pool.tile([98, f1rows0 * W], BF16, name="f1stack", tag="T_f1s_upl")
    nc.sync.dma_start(f1stack[:, :], P["f1stk"][:, :])

    loadw("w_f1", "b_f1", "w_c2", "b_c2a", "b_c2b", "w_f2", "b_f2p",
          "w_cm", "b_cm", "w_v", "w_qk", "sel0", "sel1")

    inp_bf = pool.tile([128, HW], BF16, name="inp_bf", tag="T_inp_xs2")
    load_cast_flat(inp_bf, P["inp"], 128, HW)

    loadw("w_1", "b_1", "w_2", "b_2", "w_3", "b_3", "w_4", "b_4",
          "w_dc", "b_dc", "w_tr_a", "w_tr_b", "b_tr", "w_fm", "b_fm")

    net_bf = PadT(nc, pool, "net_bf", 128, "net", 1)
    upf_bf = PadT(nc, pool, "upf_bf", 128, "net", 1)
    net_bf.zero_margins(); upf_bf.zero_margins()
    load_cast_padt(net_bf, P["net"], 128, "net")
    load_cast_padt(upf_bf, P["upf"], 128, "net")

    ones_col = pool.tile([128, 1], BF16, name="ones_col")
    nc.gpsimd.memset(ones_col[:, :], 1.0)
    ones_row = pool.tile([1, 128], BF16, name="ones_row")
    nc.gpsimd.memset(ones_row[:, :], 1.0)

    # ---- generic 3x3 conv ------------------------------------------------
    def conv3x3(wname, srcs, parts, out_lo, out_hi, dil, Cout, act="lrelu"):
        nkt = len(srcs)
        for r0, r1 in chunks(out_lo, out_hi, 8):
            for m_lo, m_hi, dstt, poff, bias_ap in parts:
                mn = m_hi - m_lo
                ps = ps_conv.tile([poff + mn, (r1 - r0) * W], F32, tag="ps_conv")
                first = True
                for ti in range(9):
                    dy, dx = TAPS3[ti]
                    for kt, (src, K) in enumerate(srcs):
                        nc.tensor.matmul(
                            ps[poff:poff + mn, :],
                            lhsT=wslice(wname, ti, kt, nkt, m_lo, m_hi, Cout, K),
                            rhs=src.ap(r0, r1, dy * dil, dx * dil, 0, K),
                            start=first, stop=(ti == 8 and kt == nkt - 1),
                            tile_position=(0, poff) if poff else None)
                        first = False
                o = dstt.ap(r0, r1, 0, 0, poff, poff + mn)
                if act == "lrelu":
                    nc.scalar.activation(o, ps[poff:poff + mn, :], AF.Prelu,
                                         bias=bias_ap, alpha=0.1)
                else:
                    nc.scalar.activation(o, ps[poff:poff + mn, :], AF.Identity,
                                         bias=bias_ap)

    # ---- motion encoder (phase A: rows feeding own V^T; phase B: halo) ---
    _sc = scope("motencA"); _sc.__enter__()
    c1out_a = PadT(nc, pool, "c1out_a", 128, "c1out", 1, tag="T_c1a_upt")
    c1out_b = PadT(nc, pool, "c1out_b", 128, "c1out", 1, tag="T_c1b_upsa")
    c1out_a.zero_margins(); c1out_b.zero_margins()

    def convc1(lo, hi):
        for r0, r1 in chunks(lo, hi, 8):
            off = (r0 - ROWS["corr"][0]) * W
            nn_ = (r1 - r0) * W
            for dstt, m_lo, bias in ((c1out_a, 0, "b_c1a"), (c1out_b, 128, "b_c1b")):
                ps = ps_conv.tile([128, nn_], F32, tag="ps_conv")
                nc.tensor.matmul(ps[:, :], lhsT=WS["w_c1"][:, m_lo:m_lo + 128],
                                 rhs=corr_bf[:, off:off + nn_], start=True, stop=True)
                nc.scalar.activation(dstt.ap(r0, r1), ps[:, :], AF.Prelu,
                                     bias=WS[bias][:, 0:1], alpha=0.1)

    f1_lo = ROWS["f1out"][0]
    f1s3d = f1stack[:, :].rearrange("c (r w) -> c r w", w=W)

    f1out = PadT(nc, pool, "f1out", 128, "f1out", 1, tag="T_f1o_c3o")
    f1out.zero_margins()

    def convf1(lo, hi):
        for r0, r1 in chunks(lo, hi, 8):
            ps = ps_conv.tile([128, (r1 - r0) * W], F32, tag="ps_conv")
            o0 = (r0 - ROWS["f1out"][0]) * W
            nc.tensor.matmul(ps[:, :], lhsT=WS["w_f1"][:, :],
                             rhs=f1stack[:, o0:o0 + (r1 - r0) * W],
                             start=True, stop=True)
            nc.scalar.activation(f1out.ap(r0, r1), ps[:, :], AF.Prelu,
                                 bias=WS["b_f1"][:, 0:1], alpha=0.1)

    cf0 = PadT(nc, pool, "cf0", 128, "cor", 1, tag="T_cf0_c1o")
    cf1 = PadT(nc, pool, "cf1", 128, "cor", 1, tag="T_cf1_c2o")
    cf0.zero_margins(); cf1.zero_margins()

    def convc2(lo, hi):
        conv3x3("w_c2", [(c1out_a, 128), (c1out_b, 128)],
                [(0, 128, cf0, 0, WS["b_c2a"][:, 0:1]),
                 (128, 192, cf1, 0, WS["b_c2b"][0:64, 0:1])],
                lo, hi, 1, 192)

    def convf2(lo, hi):
        for r0, r1 in chunks(lo, hi, 8):
            ps = ps_conv.tile([128, (r1 - r0) * W], F32, tag="ps_conv")
            for ti in range(9):
                dy, dx = TAPS3[ti]
                nc.tensor.matmul(ps[64:128, :],
                                 lhsT=wslice("w_f2", ti, 0, 1, 0, 64, 64),
                                 rhs=f1out.ap(r0, r1, dy, dx),
                                 start=(ti == 0), stop=(ti == 8),
                                 tile_position=(0, 64))
            nc.scalar.activation(cf1.ap(r0, r1, 0, 0, 64, 128), ps[64:128, :],
                                 AF.Prelu, bias=WS["b_f2p"][64:128, 0:1], alpha=0.1)

    mf = PadT(nc, pool, "mf", 128, "mf", 1)

    def convcm(lo, hi):
        conv3x3("w_cm", [(cf0, 128), (cf1, 128)],
                [(0, 126, mf, 0, WS["b_cm"][:, 0:1])], lo, hi, 1, 126)

    def mf_concat(lo, hi):
        nc.sync.dma_start(mf.ap(lo, hi, c_lo=126, c_hi=128),
                          f1s3d[48:50, lo - f1_lo:hi - f1_lo, 0:W])

    # phase A: everything needed for own-rows V^T
    convc1(0, 34)
    convf1(0, 34)
    convc2(0, 33)
    convf2(0, 33)
    mf.zero_margins()
    convcm(0, 32)
    mf_concat(0, 32)
    _sc.__exit__(None, None, None)

    # ---- V^T + collective #1 (reverse-on-send) ---------------------------
    _sc = scope("vt_coll"); _sc.__enter__()
    vt_all = pool.tile([128, 32 * 128], BF16, name="vt_all")
    vt_in = dram.tile([2048, 128], BF16, name="vt_in")
    vt_out = dram.tile([4096, 128], BF16, name="vt_out")
    mf_flat = pool.tile([128, 2048], BF16, name="mf_flat", tag="T_mff_vtmp_xo")
    nc.vector.tensor_copy(mf_flat[:, :].rearrange("c (r w) -> c r w", w=W),
                          mf.ap(0, 32))
    for jt in range(16):
        ps = ps_conv.tile([128, 128], F32, tag="ps_conv")
        nc.tensor.matmul(ps[:, :], lhsT=mf_flat[:, 128 * jt:128 * (jt + 1)],
                         rhs=WS["w_v"][:, :], start=True, stop=True)
        nc.vector.tensor_copy(vt_all[:, 128 * jt:128 * (jt + 1)], ps[:, :])
        for rr in range(2):
            r = 2 * jt + rr
            nc.sync.dma_start(vt_in[(31 - r) * 64:(31 - r) * 64 + 64, :],
                              vt_all[64 * rr:64 * (rr + 1), 128 * jt:128 * (jt + 1)])
    nc.gpsimd.collective_compute("AllGather", mybir.AluOpType.bypass,
                                 replica_groups=GROUPS,
                                 ins=[vt_in.opt()], outs=[vt_out.opt()])
    _sc.__exit__(None, None, None)

    # ---- q, k (fills the collective wait) --------------------------------
    _sc = scope("qk"); _sc.__enter__()
    k_bf = pool.tile([128, HW], BF16, name="k_bf", tag="T_k_upq")
    q_bf = pool.tile([128, 2048], BF16, name="q_bf", tag="T_q_hblk1")
    for lo, hi in chunks(0, HW, 512):
        ps = ps_conv.tile([128, 512], F32, tag="ps_conv")
        nc.tensor.matmul(ps[:, :], lhsT=WS["w_qk"][:, 128:256],
                         rhs=inp_bf[:, lo:hi], start=True, stop=True)
        nc.vector.tensor_copy(k_bf[:, lo:hi], ps[:, :])
    for lo, hi in chunks(0, 2048, 512):
        ps = ps_conv.tile([128, 512], F32, tag="ps_conv")
        nc.tensor.matmul(ps[:, :], lhsT=WS["w_qk"][:, 0:128],
                         rhs=inp_bf[:, lo:hi], start=True, stop=True)
        nc.vector.tensor_copy(q_bf[:, lo:hi], ps[:, :])
    _sc.__exit__(None, None, None)

    # ---- motion encoder phase B (halo rows; also fills collective wait) --
    _sc = scope("motencB"); _sc.__enter__()
    convc1(34, 50)
    convf1(34, 50)
    convc2(33, 49)
    convf2(33, 49)
    convcm(32, 48)
    mf_concat(32, 48)
    _sc.__exit__(None, None, None)

    # ---- collective #1 receive (mask-select, partner rows ascend mine) ---
    _sc = scope("vt_recv"); _sc.__enter__()
    selp1 = pool.tile([128, 16 * 128], BF16, name="selp1", tag="T_selp1")
    nc.sync.dma_start(
        vt_all[:, 2048:4096].rearrange("p (j d) -> p j d", d=128),
        vt_out[0:2048, :].rearrange("(j p) d -> p j d", p=128))
    nc.sync.dma_start(
        selp1[:, :].rearrange("p (j d) -> p j d", d=128),
        vt_out[2048:4096, :].rearrange("(j p) d -> p j d", p=128))
    nc.vector.tensor_scalar_mul(vt_all[:, 2048:4096], vt_all[:, 2048:4096],
                                WS["sel0"][:, 0:1])
    nc.vector.tensor_scalar_mul(selp1[:, :], selp1[:, :], WS["sel1"][:, 0:1])
    nc.vector.tensor_add(vt_all[:, 2048:4096], vt_all[:, 2048:4096], selp1[:, :])
    _sc.__exit__(None, None, None)

    # ---- attention (i-chunks in order 2,3,0,1; pipelined normalize) ------
    _sc = scope("attn"); _sc.__enter__()
    mfg = PadT(nc, pool, "mfg", 128, "mfg", 1, tag="T_cf1_c2o")
    mfg.zero_margins()

    aggs = {}
    dsbs = {}

    def jloop(ic):
        i0 = ic * 512
        agg_ps = ps_agg.tile([128, 512], F32, tag="ps_agg")
        aggs[ic] = agg_ps
        dsum = pool.tile([128, 1024], BF16, name=f"dsum{ic}", tag="T_corr_dsum_xs")
        for jg in range(16):
            sc_ps = ps_scores.tile([128, 1024], F32, tag="ps_sc")
            for u in range(2):
                j = 2 * jg + u
                nc.tensor.matmul(sc_ps[:, 512 * u:512 * (u + 1)],
                                 lhsT=k_bf[:, 128 * j:128 * (j + 1)],
                                 rhs=q_bf[:, i0:i0 + 512], start=True, stop=True)
            pb = small.tile([128, 1024], BF16, tag="probs")
            nc.scalar.activation(pb[:, :], sc_ps[:, :], AF.Exp)
            if jg == 0:
                nc.vector.tensor_copy(dsum[:, :], pb[:, :])
            else:
                nc.vector.tensor_add(dsum[:, :], dsum[:, :], pb[:, :])
            for u in range(2):
                j = 2 * jg + u
                nc.tensor.matmul(agg_ps[:, :],
                                 lhsT=vt_all[:, 128 * j:128 * (j + 1)],
                                 rhs=pb[:, 512 * u:512 * (u + 1)],
                                 start=(j == 0), stop=(j == 31),
                                 skip_group_check=True)
        dsb = small.tile([128, 512], BF16, tag="dsb")
        nc.vector.tensor_add(dsb[:, :], dsum[:, 0:512], dsum[:, 512:1024])
        dsbs[ic] = dsb

    def norm_rest(ic):
        agg_ps = aggs.pop(ic)
        dsb = dsbs.pop(ic)
        sums_ps = ps_conv.tile([1, 512], F32, tag="ps_conv")
        nc.tensor.matmul(sums_ps[0:1, :], lhsT=ones_col[:, :], rhs=dsb[:, :],
                         start=True, stop=True)
        recip = small.tile([1, 512], F32, tag="recip")
        nc.vector.reciprocal(recip[0:1, :], sums_ps[0:1, :])
        recip_bf = small.tile([1, 512], BF16, tag="recipbf")
        nc.vector.tensor_copy(recip_bf[0:1, :], recip[0:1, :])
        bc_ps = ps_conv.tile([128, 512], F32, tag="ps_conv")
        nc.tensor.matmul(bc_ps[:, :], lhsT=ones_row[0:1, :], rhs=recip_bf[0:1, :],
                         start=True, stop=True)
        bc_sb = small.tile([128, 512], F32, tag="nrm_f32")
        nc.scalar.copy(bc_sb[:, :], bc_ps[:, :])
        agg_sb = small.tile([128, 512], F32, tag="nrm_f32")
        nc.vector.tensor_mul(agg_sb[:, :], agg_ps[:, :], bc_sb[:, :])
        r0 = ic * 8
        nc.vector.tensor_add(mfg.ap(r0, r0 + 8),
                             agg_sb[:, :].rearrange("c (r w) -> c r w", w=W),
                             mf.ap(r0, r0 + 8))

    mfg_in = dram.tile([128, 16 * W], BF16, name="mfg_in")
    mfg_out = dram.tile([256, 16 * W], BF16, name="mfg_out")

    jloop(2)
    jloop(3)
    norm_rest(2)
    jloop(0)
    norm_rest(3)
    # ---- collective #2 launch (edge rows 16..31 ready) -------------------
    for r in range(16, 32):
        nc.sync.dma_start(mfg_in[:, (31 - r) * W:(31 - r) * W + W], mfg.ap(r, r + 1))
    nc.gpsimd.collective_compute("AllGather", mybir.AluOpType.bypass,
                                 replica_groups=GROUPS,
                                 ins=[mfg_in.opt()], outs=[mfg_out.opt()])
    jloop(1)
    norm_rest(0)
    norm_rest(1)
    _sc.__exit__(None, None, None)

    # ---- collective #2 receive -------------------------------------------
    _sc = scope("halo"); _sc.__enter__()
    hblk1 = pool.tile([128, 16 * W], BF16, name="hblk1", tag="T_selp1")
    nc.sync.dma_start(mfg.ap(32, 48), mfg_out[0:128, :].rearrange("c (r w) -> c r w", w=W))
    nc.sync.dma_start(hblk1[:, :], mfg_out[128:256, :])
    nc.vector.tensor_scalar_mul(mfg.ap(32, 48), mfg.ap(32, 48), WS["sel0"][:, 0:1])
    nc.vector.tensor_scalar_mul(hblk1[:, :], hblk1[:, :], WS["sel1"][:, 0:1])
    nc.vector.tensor_add(mfg.ap(32, 48), mfg.ap(32, 48),
                         hblk1[:, :].rearrange("c (r w) -> c r w", w=W))
    _sc.__exit__(None, None, None)

    # ---- conv stack ------------------------------------------------------
    _sc = scope("convs"); _sc.__enter__()
    c1o = PadT(nc, pool, "c1o", 128, "c1o", 4)
    c1o.zero_margins()
    conv3x3("w_1", [(net_bf, 128), (upf_bf, 128), (mf, 128), (mfg, 128)],
            [(0, 128, c1o, 0, WS["b_1"][:, 0:1])], *ROWS["c1o"], 1, 128)
    c2o = PadT(nc, pool, "c2o", 96, "c2o", 8)
    c2o.zero_margins()
    conv3x3("w_2", [(c1o, 128)], [(0, 96, c2o, 0, WS["b_2"][:, 0:1])],
            *ROWS["c2o"], 4, 96)
    c3o = PadT(nc, pool, "c3o", 64, "c3o", 1)
    c3o.zero_margins()
    conv3x3("w_3", [(c2o, 96)], [(0, 64, c3o, 0, WS["b_3"][:, 0:1])],
            *ROWS["c3o"], 8, 64)
    xo = PadT(nc, pool, "xo", 32, "x", 1, tag="T_mff_vtmp_xo")
    xo.zero_margins()
    conv3x3("w_4", [(c3o, 64)], [(0, 32, xo, 0, WS["b_4"][:, 0:1])],
            *ROWS["x"], 1, 32)
    _sc.__exit__(None, None, None)

    # ---- tail: single x-stack, plane deconv, up, trade, flow/mask --------
    _sc = scope("tail"); _sc.__enter__()
    rho_lo, rho_hi = RHO
    nrho = rho_hi - rho_lo
    # xs_all[ (dx+1)*32+ci, r, c ] = x[ci, rho_lo-1+r, c+dx]; rows [-1, 34)
    xs_all = pool.tile([96, (nrho + 2) * W], BF16, name="xs_all",
                       tag="T_corr_dsum_xs")
    for dxi, dx in enumerate((-1, 0, 1)):
        nc.sync.dma_start(
            xs_all[32 * dxi:32 * (dxi + 1), :].rearrange("c (r w) -> c r w", w=W),
            xo.ap(rho_lo - 1, rho_hi + 1, 0, dx))

    def xs_rhs(r0, r1, dy):
        o = (r0 + dy + 1 - rho_lo) * W
        return xs_all[:, o:o + (r1 - r0) * W]

    # deconv -> parity-major planes up_pl [ (a*2+b)*16+co, rho*W ]
    up_pl = pool.tile([64, nrho * W], BF16, name="up_pl", tag="T_f1s_upl")
    for r0, r1 in chunks(rho_lo, rho_hi, 8):
        o0 = (r0 - rho_lo) * W
        nn_ = (r1 - r0) * W
        ps = ps_conv.tile([64, nn_], F32, tag="ps_conv")
        for dyi, dy in enumerate((-1, 0, 1)):
            nc.tensor.matmul(ps[0:64, :nn_],
                             lhsT=WS["w_dc"][:, 64 * dyi:64 * (dyi + 1)],
                             rhs=xs_rhs(r0, r1, dy),
                             start=(dyi == 0), stop=(dyi == 2))
        nc.scalar.activation(up_pl[0:64, o0:o0 + nn_], ps[0:64, :nn_], AF.Prelu,
                             bias=WS["b_dc"][:, 0:1], alpha=0.1)

    # up_t (interleaved + 1-px margins) via plane-extract DMA + DVE interleave
    up_rows = UP[1] - UP[0]
    up_t = pool.tile([16, up_rows + 2, 130], BF16, name="up_t", tag="T_c1a_upt")
    nc.vector.memset(up_t[:, 0:1, :], 0.0)
    nc.vector.memset(up_t[:, up_rows + 1:up_rows + 2, :], 0.0)
    nc.vector.memset(up_t[:, 1:up_rows + 1, 0:1], 0.0)
    nc.vector.memset(up_t[:, 1:up_rows + 1, 129:130], 0.0)
    nh = (nrho + 1) // 2
    for half in range(2):
        q0 = rho_lo + nh * half
        q1 = min(rho_hi, q0 + nh)
        nq = (q1 - q0) * W
        up_q = pool.tile([16, 4 * nh * W], BF16, name=f"up_q{half}", tag="T_k_upq")
        for par in range(4):
            nc.gpsimd.dma_start(
                up_q[0:16, par * nq:(par + 1) * nq],
                up_pl[16 * par:16 * (par + 1),
                      (q0 - rho_lo) * W:(q0 - rho_lo) * W + nq])
        for a in range(2):
            for b in range(2):
                par = a * 2 + b
                lr = 1 + 2 * q0 + a - UP[0]
                nr = sum(1 for rho in range(q0, q1)
                         if 0 <= 2 * rho + a - UP[0] < up_rows)
                nc.vector.tensor_copy(
                    up_t[0:16, lr:lr + 2 * nr:2, 1 + b:1 + b + 128:2],
                    up_q[0:16, par * nq:par * nq + nr * W].rearrange(
                        "c (r w) -> c r w", w=W))

    tr_lo, tr_hi = UP_OUT
    for half in range(2):
        h0 = tr_lo + 32 * half
        h1 = h0 + 32
        upsa = pool.tile([128, 32 * 128], BF16, name=f"upsa{half}", tag="T_c1b_upsa")
        for ty in range(3):
            for tx in range(3):
                t = ty * 3 + tx
                if t == 8:
                    continue
                nc.gpsimd.dma_start(
                    upsa[16 * t:16 * (t + 1), :].rearrange("c (r w) -> c r w", w=128),
                    up_t[:, h0 - UP[0] + ty:h0 - UP[0] + ty + 32, tx:tx + 128])
        for r0, r1 in chunks(h0, h1, 4):
            o0 = (r0 - h0) * 128
            nn_ = (r1 - r0) * 128
            ps = ps_conv.tile([64, nn_], F32, tag="ps_conv")
            nc.tensor.matmul(ps[0:64, :nn_], lhsT=WS["w_tr_a"][:, :],
                             rhs=upsa[:, o0:o0 + nn_], start=True, stop=False)
            nc.tensor.matmul(ps[0:64, :nn_], lhsT=WS["w_tr_b"][:, :],
                             rhs=up_t[0:16, r0 - UP[0] + 2:r1 - UP[0] + 2, 2:130],
                             start=False, stop=True)
            st = stage.tile([64, nn_], F32, tag="trstage")
            nc.scalar.activation(st[0:64, :nn_], ps[0:64, :nn_], AF.Identity,
                                 bias=WS["b_tr"][:, 0:1])
            nc.sync.dma_start(
                P["tradeoff_out"][:, (r0 - tr_lo) * 128:(r0 - tr_lo) * 128 + nn_],
                st[0:64, :nn_])

    for r0, r1 in chunks(*UP_OUT, 4):
        st = stage.tile([16, (r1 - r0) * 128], F32, tag="upstage")
        nc.vector.tensor_copy(
            st[0:16, :].rearrange("c (r w) -> c r w", w=128),
            up_t[:, 1 + r0 - UP[0]:1 + r1 - UP[0], 1:129])
        nc.sync.dma_start(
            P["up_out"][:, (r0 - UP_OUT[0]) * 128:(r1 - UP_OUT[0]) * 128],
            st[0:16, :])

    for r0, r1 in chunks(0, 32, 8):
        nn_ = (r1 - r0) * W
        ps = ps_conv.tile([3, nn_], F32, tag="ps_conv")
        for dyi, dy in enumerate((-1, 0, 1)):
            nc.tensor.matmul(ps[0:3, :nn_],
                             lhsT=WS["w_fm"][:, 3 * dyi:3 * (dyi + 1)],
                             rhs=xs_rhs(r0, r1, dy),
                             start=(dyi == 0), stop=(dyi == 2))
        st = stage.tile([3, nn_], F32, tag="trstage")
        nc.scalar.activation(st[0:3, :nn_], ps[0:3, :nn_], AF.Identity,
                             bias=WS["b_fm"][:, 0:1])
        nc.sync.dma_start(P["flow_out"][:, r0 * W:r1 * W], st[0:2, :nn_])
        nc.sync.dma_start(P["mask_out"][:, r0 * W:r1 * W], st[2:3, :nn_])
    _sc.__exit__(None, None, None)

    ctx.close()


# ---------------------------------------------------------------------------
# Host-side sharding / gather


_NC_CACHE = None
_W_CACHE = {}


def _get_nc():
    global _NC_CACHE
    if _NC_CACHE is None:
        _NC_CACHE = build_nc()
    return _NC_CACHE


def build_f1stack(flow_b, flip):
    """Tap-shifted flow copies for the K-stacked convf1: [98, f1rows*W]."""
    v = np.asarray(flow_b, np.float32)
    if flip:
        v = v[:, ::-1, :]
    lo, hi = ROWS["f1out"]
    out = np.zeros((98, hi - lo, W), np.float32)
    for ty in range(7):
        for tx in range(7):
            t = ty * 7 + tx
            r0 = max(lo, -(ty - 3))
            r1 = min(hi, 64 - (ty - 3))
            c0 = max(0, -(tx - 3))
            c1 = min(W, W - (tx - 3))
            if r0 >= r1 or c0 >= c1:
                continue
            out[2 * t:2 * t + 2, r0 - lo:r1 - lo, c0:c1] = \
                v[:, r0 + ty - 3:r1 + ty - 3, c0 + tx - 3:c1 + tx - 3]
    return np.ascontiguousarray(out.reshape(98, -1)).astype(ml_dtypes.bfloat16)


def prepare_in_maps(inputs):
    w0 = prep_weights(inputs, flip=False)
    w1 = prep_weights(inputs, flip=True)
    sel = {
        0: (np.zeros((128, 1), np.float32), np.ones((128, 1), np.float32)),
        1: (np.ones((128, 1), np.float32), np.zeros((128, 1), np.float32)),
    }
    in_maps = []
    for c in range(N_CORES):
        b, h = c // 2, c % 2
        flip = h == 1

        def rows(a, key):
            lo, hi = ROWS[key]
            v = a[:, ::-1, :] if flip else a
            return np.ascontiguousarray(v[:, lo:hi, :], dtype=np.float32).reshape(a.shape[0], -1)

        x1 = inputs["x1"][b]
        v = x1[NET:]
        v = v[:, ::-1, :] if flip else v
        m = {"inp": np.ascontiguousarray(v, dtype=np.float32).reshape(128, HW)}
        m["net"] = rows(x1[:NET], "net")
        m["upf"] = rows(inputs["upfeat"][b], "net")
        m["corr"] = rows(inputs["corr"][b], "corr")
        m["f1stk"] = build_f1stack(inputs["flow"][b], flip)
        wd = w1 if flip else w0
        for k, v2 in wd.items():
            m[k] = v2
        m["sel0"], m["sel1"] = sel[h]
        in_maps.append(m)
    return in_maps


def assemble_outputs(results):
    B = 4
    tradeoff = np.zeros((B, 64, 128, 128), np.float32)
    up = np.zeros((B, 16, 128, 128), np.float32)
    flow_out = np.zeros((B, 2, 64, 64), np.float32)
    mask = np.zeros((B, 1, 64, 64), np.float32)
    for c in range(N_CORES):
        b, h = c // 2, c % 2
        r = results[c]
        tr = r["tradeoff_out"].reshape(64, 64, 128)
        u = r["up_out"].reshape(16, 64, 128)
        fo = r["flow_out"].reshape(2, 32, 64)
        mo = r["mask_out"].reshape(1, 32, 64)
        if h == 1:
            tr, u, fo, mo = tr[:, ::-1], u[:, ::-1], fo[:, ::-1], mo[:, ::-1]
            tradeoff[b, :, 64:128] = tr
            up[b, :, 64:128] = u
            flow_out[b, :, 32:64] = fo
            mask[b, :, 32:64] = mo
        else:
            tradeoff[b, :, 0:64] = tr
            up[b, :, 0:64] = u
            flow_out[b, :, 0:32] = fo
            mask[b, :, 0:32] = mo
    return (tradeoff, up, flow_out, mask)


def run(inputs, trace=False, **kw):
    nc = _get_nc()
    in_maps = prepare_in_maps(inputs)
    res = run_bass_kernel_spmd(nc, in_maps, core_ids=list(range(N_CORES)),
                               trace=trace, **kw)
    return assemble_outputs(res.results), res


def kernel(**inputs):
    outs, _ = run(inputs)
    return outs


# revision 23
# speedup vs baseline: 1.0882x; 1.0882x over previous
"""Trainium2 8-core Bass kernel for nn_AttAggFME.

Sharding: core c = (batch b=c//2, half h=c%2). Every core runs IDENTICAL
code with "top half" geometry; h=1 cores receive vertically flipped data
(host flips rows) and dy-tap-flipped conv weights, and their outputs are
flipped back on the host.

Per core: 2048 attention queries (own rows), motion encoder on own rows +
16 halo rows (halo comes free from host-sliced inputs), conv1..4 with halo
recompute. Two intra-pair AllGathers: V^T (full 4096 values for attn@V) and
16 halo rows of motion_fea_global before conv1. Cross-core row-order and
rank-offset asymmetries are handled with reverse-on-send plus host-supplied
0/1 selector masks.

All matmuls bf16 with f32 PSUM accumulation; softmax logits stay f32 in
PSUM, exp on ScalarE; denominators via ones-matmul (scores are computed
transposed [keys, queries] so attn@V needs no transposes).
"""

import numpy as np
import ml_dtypes

import concourse.bass as bass
import concourse.tile as tile
from concourse import mybir, bacc
from concourse.bass_utils import run_bass_kernel_spmd

F32 = mybir.dt.float32
BF16 = mybir.dt.bfloat16
AF = mybir.ActivationFunctionType

N_CORES = 8
H = W = 64
HW = H * W
D = 128
NET = 128
GROUPS = [[0, 1], [2, 3], [4, 5], [6, 7]]

# Row ranges (own-frame; every core is "top half": own rows [0:32))
ROWS = {
    "own":   (0, 32),
    "mf":    (0, 48),   # motion_fea (conv1 needs 16 halo rows)
    "mfg":   (0, 48),   # motion_fea_global (own 0:32 + partner halo 32:48)
    "c1o":   (0, 47),   # conv1 out
    "c2o":   (0, 43),   # conv2 out
    "c3o":   (0, 35),   # conv3 out
    "x":     (0, 34),   # conv4 out
    "cor":   (0, 49),   # convc2 out
    "c1out": (0, 50),   # convc1 out
    "f1out": (0, 50),   # convf1 out
    "flo":   (0, 49),   # convf2 out
    "corr":  (0, 50),   # corr input rows shipped
    "flow":  (0, 53),   # flow input rows shipped
    "net":   (0, 48),   # net / upfeat input rows shipped
}
RHO = (0, 33)       # deconv x-row parity range
UP = (0, 66)        # up rows stored
UP_OUT = (0, 64)    # up rows output / trade out rows

TAPS3 = [(ty - 1, tx - 1) for ty in range(3) for tx in range(3)]


def _nrows(key):
    lo, hi = ROWS[key]
    return hi - lo


PARAM_SPECS = [
    # activations (f32, own-frame)
    ("inp",  [128, HW], F32),
    ("net",  [128, _nrows("net") * W], F32),
    ("upf",  [128, _nrows("net") * W], F32),
    ("corr", [81, _nrows("corr") * W], F32),
    ("f1stk", [98, _nrows("f1out") * W], BF16),
    # weights (bf16) / biases (f32) / selectors
    ("w_qk", [128, 256], BF16),
    ("w_c1", [81, 256], BF16), ("b_c1a", [128, 1], F32), ("b_c1b", [128, 1], F32),
    ("w_c2", [128, 9 * 2 * 192], BF16), ("b_c2a", [128, 1], F32), ("b_c2b", [64, 1], F32),
    ("w_f1", [98, 128], BF16), ("b_f1", [128, 1], F32),
    ("w_f2", [128, 9 * 64], BF16), ("b_f2p", [128, 1], F32),
    ("w_cm", [128, 9 * 2 * 126], BF16), ("b_cm", [126, 1], F32),
    ("w_v", [128, 128], BF16),
    ("w_1", [128, 9 * 4 * 128], BF16), ("b_1", [128, 1], F32),
    ("w_2", [128, 9 * 96], BF16), ("b_2", [96, 1], F32),
    ("w_3", [96, 9 * 64], BF16), ("b_3", [64, 1], F32),
    ("w_4", [64, 9 * 32], BF16), ("b_4", [32, 1], F32),
    ("w_dc", [96, 3 * 64], BF16), ("b_dc", [64, 1], F32),
    ("w_tr_a", [128, 64], BF16), ("w_tr_b", [16, 64], BF16), ("b_tr", [64, 1], F32),
    ("w_fm", [96, 3 * 3], BF16), ("b_fm", [3, 1], F32),
    ("sel0", [128, 1], F32), ("sel1", [128, 1], F32),
]


# ---------------------------------------------------------------------------
# Host-side weight prep


def prep_weights(inp, flip):
    """All conv weights packed for the kernel; flip=True mirrors dy taps."""
    def bf(a):
        return np.ascontiguousarray(a, dtype=np.float32).astype(ml_dtypes.bfloat16)

    def col(a, n=None, off=0):
        a = np.asarray(a, np.float32).reshape(-1)
        n = n or a.shape[0]
        out = np.zeros((n, 1), np.float32)
        out[off:off + a.shape[0], 0] = a
        return out

    def fl(wt):  # [Cout, Cin, kh, kw] -> dy mirrored
        return wt[:, :, ::-1, :] if flip else wt

    w = {}
    qk = inp["att_to_qk_w"][:, :, 0, 0].astype(np.float64).T.copy()  # [128, 256]
    qk[:, :D] *= D ** -0.5
    w["w_qk"] = bf(qk)

    w["w_c1"] = bf(inp["convc1_w"][:, :, 0, 0].T)
    w["b_c1a"] = col(inp["convc1_b"][:128])
    w["b_c1b"] = col(inp["convc1_b"][128:])

    def conv3x3(wt):
        wt = fl(wt)
        Cout, Cin = wt.shape[:2]
        nkt = (Cin + 127) // 128
        K = 128 if nkt > 1 else Cin
        blocks = []
        for (dy, dx) in TAPS3:
            for kt in range(nkt):
                cs, ce = kt * 128, min(Cin, (kt + 1) * 128)
                blk = np.zeros((K, Cout), np.float64)
                blk[: ce - cs] = wt[:, cs:ce, dy + 1, dx + 1].T
                blocks.append(blk)
        return bf(np.concatenate(blocks, axis=1))

    w["w_c2"] = conv3x3(inp["convc2_w"])
    w["b_c2a"] = col(inp["convc2_b"][:128])
    w["b_c2b"] = col(inp["convc2_b"][128:])

    f1 = fl(inp["convf1_w"])
    lf1 = np.zeros((98, 128), np.float64)
    for ty in range(7):
        for tx in range(7):
            for ci in range(2):
                lf1[2 * (ty * 7 + tx) + ci] = f1[:, ci, ty, tx]
    w["w_f1"] = bf(lf1)
    w["b_f1"] = col(inp["convf1_b"])

    w["w_f2"] = conv3x3(inp["convf2_w"])
    w["b_f2p"] = col(inp["convf2_b"], n=128, off=64)

    w["w_cm"] = conv3x3(inp["conv_motion_w"])
    w["b_cm"] = col(inp["conv_motion_b"])

    gv = float(np.asarray(inp["gamma"]).reshape(-1)[0]) * inp["agg_to_v_w"][:, :, 0, 0]
    w["w_v"] = bf(gv.T)

    w["w_1"] = conv3x3(inp["conv1_w"]); w["b_1"] = col(inp["conv1_b"])
    w["w_2"] = conv3x3(inp["conv2_w"]); w["b_2"] = col(inp["conv2_b"])
    w["w_3"] = conv3x3(inp["conv3_w"]); w["b_3"] = col(inp["conv3_b"])
    w["w_4"] = conv3x3(inp["conv4_w"]); w["b_4"] = col(inp["conv4_b"])

    # deconv: ConvTranspose2d(k4,s2,p1) w [32ci,16co,4,4]; flip: kh -> 3-kh
    dw = inp["upfeat_w"].astype(np.float64)
    if flip:
        dw = dw[:, :, ::-1, :]
    ldc = np.zeros((3, 96, 64), np.float64)
    for dyi, dy in enumerate((-1, 0, 1)):
        for dxi, dx in enumerate((-1, 0, 1)):
            for a in range(2):
                kh = a + 1 - 2 * dy
                if not (0 <= kh < 4) or (a == 0 and dy not in (0, -1)) or (a == 1 and dy not in (0, 1)):
                    continue
                for b in range(2):
                    kw = b + 1 - 2 * dx
                    if not (0 <= kw < 4) or (b == 0 and dx not in (0, -1)) or (b == 1 and dx not in (0, 1)):
                        continue
                    ldc[dyi, dxi * 32:dxi * 32 + 32, (a * 2 + b) * 16:(a * 2 + b) * 16 + 16] = dw[:, :, kh, kw]
    w["w_dc"] = bf(ldc.transpose(1, 0, 2).reshape(96, 3 * 64))
    w["b_dc"] = col(np.tile(np.asarray(inp["upfeat_b"], np.float64), 4))

    tw = fl(inp["trade_w"]).astype(np.float64)
    la = np.zeros((128, 64), np.float64)
    lb = np.zeros((16, 64), np.float64)
    for ty in range(3):
        for tx in range(3):
            t = ty * 3 + tx
            blk = tw[:, :, ty, tx].T
            if t < 8:
                la[t * 16:(t + 1) * 16] = blk
            else:
                lb[:] = blk
    w["w_tr_a"] = bf(la)
    w["w_tr_b"] = bf(lb)
    w["b_tr"] = col(inp["trade_b"])

    fw = fl(inp["flow_w"]).astype(np.float64)
    mw = fl(inp["mask_w"]).astype(np.float64)
    lfm = np.zeros((3, 96, 3), np.float64)
    for dyi, dy in enumerate((-1, 0, 1)):
        for dxi, dx in enumerate((-1, 0, 1)):
            lfm[dyi, dxi * 32:dxi * 32 + 32, 0:2] = fw[:, :, dy + 1, dx + 1].T
            lfm[dyi, dxi * 32:dxi * 32 + 32, 2] = mw[0, :, dy + 1, dx + 1]
    w["w_fm"] = bf(lfm.transpose(1, 0, 2).reshape(96, 3 * 3))
    w["b_fm"] = col(np.concatenate([np.asarray(inp["flow_b"]), np.asarray(inp["mask_b"])]))
    return w


# ---------------------------------------------------------------------------
# Padded spatial SBUF tensors


class PadT:
    """SBUF tile [C, rtot, stride] with `pad` zeroed margin rows/cols; g0 =
    own-frame row of the first real row."""

    def __init__(self, nc, pool, name, C, key_or_range, pad, dtype=BF16, tag=None):
        g0, g1 = ROWS[key_or_range] if isinstance(key_or_range, str) else key_or_range
        self.nc, self.C, self.g0, self.rows, self.pad = nc, C, g0, g1 - g0, pad
        self.stride = W + 2 * pad
        self.rtot = self.rows + 2 * pad
        self.t = pool.tile([C, self.rtot, self.stride], dtype, name=name,
                           tag=tag or name)

    def zero_margins(self):
        nc, p = self.nc, self.pad
        if p == 0:
            return
        nc.gpsimd.memset(self.t[:, 0:p, :], 0.0)
        nc.gpsimd.memset(self.t[:, self.rtot - p:self.rtot, :], 0.0)
        nc.gpsimd.memset(self.t[:, p:p + self.rows, 0:p], 0.0)
        nc.gpsimd.memset(self.t[:, p:p + self.rows, self.stride - p:self.stride], 0.0)

    def ap(self, r_lo, r_hi, dy=0, dx=0, c_lo=0, c_hi=None):
        c_hi = self.C if c_hi is None else c_hi
        a = r_lo - self.g0 + self.pad + dy
        b = r_hi - self.g0 + self.pad + dy
        assert 0 <= a and b <= self.rtot, (r_lo, r_hi, dy, self.g0, self.rows)
        assert 0 <= self.pad + dx and dx <= self.pad
        return self.t[c_lo:c_hi, a:b, self.pad + dx:self.pad + dx + W]


def chunks(lo, hi, step):
    r = lo
    while r < hi:
        yield r, min(hi, r + step)
        r += step


# ---------------------------------------------------------------------------
# Graph build


def build_nc():
    nc = bacc.Bacc()
    P = {}
    for name, shape, dt in PARAM_SPECS:
        P[name] = nc.declare_dram_parameter(name, shape, dt, isOutput=False)
    P["tradeoff_out"] = nc.declare_dram_parameter("tradeoff_out", [64, 64 * 128], F32, isOutput=True)
    P["up_out"] = nc.declare_dram_parameter("up_out", [16, 64 * 128], F32, isOutput=True)
    P["flow_out"] = nc.declare_dram_parameter("flow_out", [2, 32 * W], F32, isOutput=True)
    P["mask_out"] = nc.declare_dram_parameter("mask_out", [1, 32 * W], F32, isOutput=True)

    with tile.TileContext(nc) as tc:
        _emit(nc, tc, P)
    nc.finalize()
    return nc


def _emit(nc, tc, P):
    from contextlib import ExitStack
    ctx = ExitStack()
    pool = ctx.enter_context(tc.tile_pool(name="main", bufs=1))
    stage = ctx.enter_context(tc.tile_pool(name="stage", bufs=3))
    small = ctx.enter_context(tc.tile_pool(name="small", bufs=2))
    dram = ctx.enter_context(tc.tile_pool(name="dram", bufs=1, space="DRAM"))
    ps_conv = ctx.enter_context(tc.tile_pool(name="ps_conv", bufs=2, space="PSUM"))
    ps_scores = ctx.enter_context(tc.tile_pool(name="ps_scores", bufs=2, space="PSUM"))
    ps_agg = ctx.enter_context(tc.tile_pool(name="ps_agg", bufs=2, space="PSUM"))

    scope = nc.named_scope

    # ---- weights, emitted in order of first use --------------------------
    WS = {}

    def loadw(*names):
        for name in names:
            spec = next(s for s in PARAM_SPECS if s[0] == name)
            t = pool.tile(spec[1], spec[2], name=f"sb_{name}")
            nc.sync.dma_start(t[:], P[name][:])
            WS[name] = t

    def wslice(name, tap, kt, nkt, m_lo, m_hi, Cout, K=128):
        base = (tap * nkt + kt) * Cout
        return WS[name][0:K, base + m_lo:base + m_hi]

    # ---- input casts -----------------------------------------------------
    def load_cast_flat(dst, dram_p, C, total):
        for lo, hi in chunks(0, total, 1024):
            st = stage.tile([C, hi - lo], F32, tag="f32stage")
            nc.sync.dma_start(st[0:C, 0:hi - lo], dram_p[0:C, lo:hi])
            nc.vector.tensor_copy(dst[0:C, lo:hi], st[0:C, 0:hi - lo])

    def load_cast_padt(dst, dram_p, C, key):
        lo, hi = ROWS[key]
        for r0, r1 in chunks(lo, hi, 16):
            st = stage.tile([C, (r1 - r0) * W], F32, tag="f32stage")
            nc.sync.dma_start(st[0:C, 0:(r1 - r0) * W],
                              dram_p[0:C, (r0 - lo) * W:(r1 - lo) * W])
            nc.vector.tensor_copy(
                dst.ap(r0, r1, c_lo=0, c_hi=C),
                st[0:C, 0:(r1 - r0) * W].rearrange("c (r w) -> c r w", w=W))

    loadw("w_c1", "b_c1a", "b_c1b")
    crows = _nrows("corr")
    corr_bf = pool.tile([81, crows * W], BF16, name="corr_bf", tag="T_corr_dsum_xs")
    load_cast_flat(corr_bf, P["corr"], 81, crows * W)
    f1rows0 = _nrows("f1out")
    f1stack = pool.tile([98, f1rows0 * W], BF16, name="f1stack", tag="T_f1s_upl")
    nc.sync.dma_start(f1stack[:, :], P["f1stk"][:, :])

    loadw("w_f1", "b_f1", "w_c2", "b_c2a", "b_c2b")

    inp_bf = pool.tile([128, HW], BF16, name="inp_bf", tag="T_inp_xs2")
    load_cast_flat(inp_bf, P["inp"], 128, HW)

    loadw("w_f2", "b_f2p", "w_cm", "b_cm", "w_v", "w_qk", "sel0", "sel1",
          "w_1", "b_1", "w_2", "b_2", "w_3", "b_3", "w_4", "b_4",
          "w_dc", "b_dc", "w_tr_a", "w_tr_b", "b_tr", "w_fm", "b_fm")

    net_bf = PadT(nc, pool, "net_bf", 128, "net", 1)
    upf_bf = PadT(nc, pool, "upf_bf", 128, "net", 1)
    net_bf.zero_margins(); upf_bf.zero_margins()
    load_cast_padt(net_bf, P["net"], 128, "net")
    load_cast_padt(upf_bf, P["upf"], 128, "net")

    ones_col = pool.tile([128, 1], BF16, name="ones_col")
    nc.gpsimd.memset(ones_col[:, :], 1.0)
    ones_row = pool.tile([1, 128], BF16, name="ones_row")
    nc.gpsimd.memset(ones_row[:, :], 1.0)

    # ---- generic 3x3 conv ------------------------------------------------
    def conv3x3(wname, srcs, parts, out_lo, out_hi, dil, Cout, act="lrelu"):
        nkt = len(srcs)
        for r0, r1 in chunks(out_lo, out_hi, 8):
            for m_lo, m_hi, dstt, poff, bias_ap in parts:
                mn = m_hi - m_lo
                ps = ps_conv.tile([poff + mn, (r1 - r0) * W], F32, tag="ps_conv")
                first = True
                for ti in range(9):
                    dy, dx = TAPS3[ti]
                    for kt, (src, K) in enumerate(srcs):
                        nc.tensor.matmul(
                            ps[poff:poff + mn, :],
                            lhsT=wslice(wname, ti, kt, nkt, m_lo, m_hi, Cout, K),
                            rhs=src.ap(r0, r1, dy * dil, dx * dil, 0, K),
                            start=first, stop=(ti == 8 and kt == nkt - 1),
                            tile_position=(0, poff) if poff else None)
                        first = False
                o = dstt.ap(r0, r1, 0, 0, poff, poff + mn)
                if act == "lrelu":
                    nc.scalar.activation(o, ps[poff:poff + mn, :], AF.Prelu,
                                         bias=bias_ap, alpha=0.1)
                else:
                    nc.scalar.activation(o, ps[poff:poff + mn, :], AF.Identity,
                                         bias=bias_ap)

    # ---- motion encoder (phase A: rows feeding own V^T; phase B: halo) ---
    _sc = scope("motencA"); _sc.__enter__()
    c1out_a = PadT(nc, pool, "c1out_a", 128, "c1out", 1, tag="T_c1a_upt")
    c1out_b = PadT(nc, pool, "c1out_b", 128, "c1out", 1, tag="T_c1b_upsa")
    c1out_a.zero_margins(); c1out_b.zero_margins()

    def convc1(lo, hi):
        for r0, r1 in chunks(lo, hi, 8):
            off = (r0 - ROWS["corr"][0]) * W
            nn_ = (r1 - r0) * W
            for dstt, m_lo, bias in ((c1out_a, 0, "b_c1a"), (c1out_b, 128, "b_c1b")):
                ps = ps_conv.tile([128, nn_], F32, tag="ps_conv")
                nc.tensor.matmul(ps[:, :], lhsT=WS["w_c1"][:, m_lo:m_lo + 128],
                                 rhs=corr_bf[:, off:off + nn_], start=True, stop=True)
                nc.scalar.activation(dstt.ap(r0, r1), ps[:, :], AF.Prelu,
                                     bias=WS[bias][:, 0:1], alpha=0.1)

    f1_lo = ROWS["f1out"][0]
    f1s3d = f1stack[:, :].rearrange("c (r w) -> c r w", w=W)

    f1out = PadT(nc, pool, "f1out", 128, "f1out", 1, tag="T_f1o_c3o")
    f1out.zero_margins()

    def convf1(lo, hi):
        for r0, r1 in chunks(lo, hi, 8):
            ps = ps_conv.tile([128, (r1 - r0) * W], F32, tag="ps_conv")
            o0 = (r0 - ROWS["f1out"][0]) * W
            nc.tensor.matmul(ps[:, :], lhsT=WS["w_f1"][:, :],
                             rhs=f1stack[:, o0:o0 + (r1 - r0) * W],
                             start=True, stop=True)
            nc.scalar.activation(f1out.ap(r0, r1), ps[:, :], AF.Prelu,
                                 bias=WS["b_f1"][:, 0:1], alpha=0.1)

    cf0 = PadT(nc, pool, "cf0", 128, "cor", 1, tag="T_cf0_c1o")
    cf1 = PadT(nc, pool, "cf1", 128, "cor", 1, tag="T_cf1_c2o")
    cf0.zero_margins(); cf1.zero_margins()

    def convc2(lo, hi):
        conv3x3("w_c2", [(c1out_a, 128), (c1out_b, 128)],
                [(0, 128, cf0, 0, WS["b_c2a"][:, 0:1]),
                 (128, 192, cf1, 0, WS["b_c2b"][0:64, 0:1])],
                lo, hi, 1, 192)

    def convf2(lo, hi):
        for r0, r1 in chunks(lo, hi, 8):
            ps = ps_conv.tile([128, (r1 - r0) * W], F32, tag="ps_conv")
            for ti in range(9):
                dy, dx = TAPS3[ti]
                nc.tensor.matmul(ps[64:128, :],
                                 lhsT=wslice("w_f2", ti, 0, 1, 0, 64, 64),
                                 rhs=f1out.ap(r0, r1, dy, dx),
                                 start=(ti == 0), stop=(ti == 8),
                                 tile_position=(0, 64))
            nc.scalar.activation(cf1.ap(r0, r1, 0, 0, 64, 128), ps[64:128, :],
                                 AF.Prelu, bias=WS["b_f2p"][64:128, 0:1], alpha=0.1)

    mf = PadT(nc, pool, "mf", 128, "mf", 1)

    def convcm(lo, hi):
        conv3x3("w_cm", [(cf0, 128), (cf1, 128)],
                [(0, 126, mf, 0, WS["b_cm"][:, 0:1])], lo, hi, 1, 126)

    def mf_concat(lo, hi):
        nc.sync.dma_start(mf.ap(lo, hi, c_lo=126, c_hi=128),
                          f1s3d[48:50, lo - f1_lo:hi - f1_lo, 0:W])

    # phase A: everything needed for own-rows V^T
    convc1(0, 34)
    convf1(0, 34)
    convc2(0, 33)
    convf2(0, 33)
    mf.zero_margins()
    convcm(0, 32)
    mf_concat(0, 32)
    _sc.__exit__(None, None, None)

    # ---- V^T + collective #1 (reverse-on-send) ---------------------------
    _sc = scope("vt_coll"); _sc.__enter__()
    vt_all = pool.tile([128, 32 * 128], BF16, name="vt_all")
    vt_in = dram.tile([2048, 128], BF16, name="vt_in")
    vt_out = dram.tile([4096, 128], BF16, name="vt_out")
    mf_flat = pool.tile([128, 2048], BF16, name="mf_flat", tag="T_mff_vtmp_xo")
    nc.vector.tensor_copy(mf_flat[:, :].rearrange("c (r w) -> c r w", w=W),
                          mf.ap(0, 32))
    for jt in range(16):
        ps = ps_conv.tile([128, 128], F32, tag="ps_conv")
        nc.tensor.matmul(ps[:, :], lhsT=mf_flat[:, 128 * jt:128 * (jt + 1)],
                         rhs=WS["w_v"][:, :], start=True, stop=True)
        nc.vector.tensor_copy(vt_all[:, 128 * jt:128 * (jt + 1)], ps[:, :])
        for rr in range(2):
            r = 2 * jt + rr
            nc.sync.dma_start(vt_in[(31 - r) * 64:(31 - r) * 64 + 64, :],
                              vt_all[64 * rr:64 * (rr + 1), 128 * jt:128 * (jt + 1)])
    nc.gpsimd.collective_compute("AllGather", mybir.AluOpType.bypass,
                                 replica_groups=GROUPS,
                                 ins=[vt_in.opt()], outs=[vt_out.opt()])
    _sc.__exit__(None, None, None)

    # ---- q, k (fills the collective wait) --------------------------------
    _sc = scope("qk"); _sc.__enter__()
    k_bf = pool.tile([128, HW], BF16, name="k_bf", tag="T_k_upq")
    q_bf = pool.tile([128, 2048], BF16, name="q_bf", tag="T_q_hblk1")
    for lo, hi in chunks(0, HW, 512):
        ps = ps_conv.tile([128, 512], F32, tag="ps_conv")
        nc.tensor.matmul(ps[:, :], lhsT=WS["w_qk"][:, 128:256],
                         rhs=inp_bf[:, lo:hi], start=True, stop=True)
        nc.vector.tensor_copy(k_bf[:, lo:hi], ps[:, :])
    for lo, hi in chunks(0, 2048, 512):
        ps = ps_conv.tile([128, 512], F32, tag="ps_conv")
        nc.tensor.matmul(ps[:, :], lhsT=WS["w_qk"][:, 0:128],
                         rhs=inp_bf[:, lo:hi], start=True, stop=True)
        nc.vector.tensor_copy(q_bf[:, lo:hi], ps[:, :])
    _sc.__exit__(None, None, None)

    # ---- motion encoder phase B (halo rows; also fills collective wait) --
    _sc = scope("motencB"); _sc.__enter__()
    convc1(34, 50)
    convf1(34, 50)
    convc2(33, 49)
    convf2(33, 49)
    convcm(32, 48)
    mf_concat(32, 48)
    _sc.__exit__(None, None, None)

    # ---- collective #1 receive (mask-select, partner rows ascend mine) ---
    _sc = scope("vt_recv"); _sc.__enter__()
    selp1 = pool.tile([128, 16 * 128], BF16, name="selp1", tag="T_selp1")
    nc.sync.dma_start(
        vt_all[:, 2048:4096].rearrange("p (j d) -> p j d", d=128),
        vt_out[0:2048, :].rearrange("(j p) d -> p j d", p=128))
    nc.sync.dma_start(
        selp1[:, :].rearrange("p (j d) -> p j d", d=128),
        vt_out[2048:4096, :].rearrange("(j p) d -> p j d", p=128))
    nc.vector.tensor_scalar_mul(vt_all[:, 2048:4096], vt_all[:, 2048:4096],
                                WS["sel0"][:, 0:1])
    nc.vector.tensor_scalar_mul(selp1[:, :], selp1[:, :], WS["sel1"][:, 0:1])
    nc.vector.tensor_add(vt_all[:, 2048:4096], vt_all[:, 2048:4096], selp1[:, :])
    _sc.__exit__(None, None, None)

    # ---- attention (i-chunks in order 2,3,0,1; pipelined normalize) ------
    _sc = scope("attn"); _sc.__enter__()
    mfg = PadT(nc, pool, "mfg", 128, "mfg", 1, tag="T_cf1_c2o")
    mfg.zero_margins()

    aggs = {}
    dsbs = {}

    def jloop(ic):
        i0 = ic * 512
        agg_ps = ps_agg.tile([128, 512], F32, tag="ps_agg")
        aggs[ic] = agg_ps
        dsum = pool.tile([128, 1024], BF16, name=f"dsum{ic}", tag="T_corr_dsum_xs")
        for jg in range(16):
            sc_ps = ps_scores.tile([128, 1024], F32, tag="ps_sc")
            for u in range(2):
                j = 2 * jg + u
                nc.tensor.matmul(sc_ps[:, 512 * u:512 * (u + 1)],
                                 lhsT=k_bf[:, 128 * j:128 * (j + 1)],
                                 rhs=q_bf[:, i0:i0 + 512], start=True, stop=True)
            pb = small.tile([128, 1024], BF16, tag="probs")
            nc.scalar.activation(pb[:, :], sc_ps[:, :], AF.Exp)
            if jg == 0:
                nc.vector.tensor_copy(dsum[:, :], pb[:, :])
            else:
                nc.vector.tensor_add(dsum[:, :], dsum[:, :], pb[:, :])
            for u in range(2):
                j = 2 * jg + u
                nc.tensor.matmul(agg_ps[:, :],
                                 lhsT=vt_all[:, 128 * j:128 * (j + 1)],
                                 rhs=pb[:, 512 * u:512 * (u + 1)],
                                 start=(j == 0), stop=(j == 31),
                                 skip_group_check=True)
        dsb = small.tile([128, 512], BF16, tag="dsb")
        nc.vector.tensor_add(dsb[:, :], dsum[:, 0:512], dsum[:, 512:1024])
        dsbs[ic] = dsb

    def norm_rest(ic):
        agg_ps = aggs.pop(ic)
        dsb = dsbs.pop(ic)
        sums_ps = ps_conv.tile([1, 512], F32, tag="ps_conv")
        nc.tensor.matmul(sums_ps[0:1, :], lhsT=ones_col[:, :], rhs=dsb[:, :],
                         start=True, stop=True)
        recip = small.tile([1, 512], F32, tag="recip")
        nc.vector.reciprocal(recip[0:1, :], sums_ps[0:1, :])
        recip_bf = small.tile([1, 512], BF16, tag="recipbf")
        nc.vector.tensor_copy(recip_bf[0:1, :], recip[0:1, :])
        bc_ps = ps_conv.tile([128, 512], F32, tag="ps_conv")
        nc.tensor.matmul(bc_ps[:, :], lhsT=ones_row[0:1, :], rhs=recip_bf[0:1, :],
                         start=True, stop=True)
        bc_sb = small.tile([128, 512], F32, tag="nrm_f32")
        nc.scalar.copy(bc_sb[:, :], bc_ps[:, :])
        agg_sb = small.tile([128, 512], F32, tag="nrm_f32")
        nc.vector.tensor_mul(agg_sb[:, :], agg_ps[:, :], bc_sb[:, :])
        r0 = ic * 8
        nc.vector.tensor_add(mfg.ap(r0, r0 + 8),
                             agg_sb[:, :].rearrange("c (r w) -> c r w", w=W),
                             mf.ap(r0, r0 + 8))

    mfg_in = dram.tile([128, 16 * W], BF16, name="mfg_in")
    mfg_out = dram.tile([256, 16 * W], BF16, name="mfg_out")

    jloop(2)
    jloop(3)
    norm_rest(2)
    jloop(0)
    norm_rest(3)
    # ---- collective #2 launch (edge rows 16..31 ready) -------------------
    for r in range(16, 32):
        nc.sync.dma_start(mfg_in[:, (31 - r) * W:(31 - r) * W + W], mfg.ap(r, r + 1))
    nc.gpsimd.collective_compute("AllGather", mybir.AluOpType.bypass,
                                 replica_groups=GROUPS,
                                 ins=[mfg_in.opt()], outs=[mfg_out.opt()])
    jloop(1)
    norm_rest(0)
    norm_rest(1)
    _sc.__exit__(None, None, None)

    # ---- collective #2 receive -------------------------------------------
    _sc = scope("halo"); _sc.__enter__()
    hblk1 = pool.tile([128, 16 * W], BF16, name="hblk1", tag="T_selp1")
    nc.sync.dma_start(mfg.ap(32, 48), mfg_out[0:128, :].rearrange("c (r w) -> c r w", w=W))
    nc.sync.dma_start(hblk1[:, :], mfg_out[128:256, :])
    nc.vector.tensor_scalar_mul(mfg.ap(32, 48), mfg.ap(32, 48), WS["sel0"][:, 0:1])
    nc.vector.tensor_scalar_mul(hblk1[:, :], hblk1[:, :], WS["sel1"][:, 0:1])
    nc.vector.tensor_add(mfg.ap(32, 48), mfg.ap(32, 48),
                         hblk1[:, :].rearrange("c (r w) -> c r w", w=W))
    _sc.__exit__(None, None, None)

    # ---- conv stack ------------------------------------------------------
    _sc = scope("convs"); _sc.__enter__()
    c1o = PadT(nc, pool, "c1o", 128, "c1o", 4)
    c1o.zero_margins()
    conv3x3("w_1", [(net_bf, 128), (upf_bf, 128), (mf, 128), (mfg, 128)],
            [(0, 128, c1o, 0, WS["b_1"][:, 0:1])], *ROWS["c1o"], 1, 128)
    c2o = PadT(nc, pool, "c2o", 96, "c2o", 8)
    c2o.zero_margins()
    conv3x3("w_2", [(c1o, 128)], [(0, 96, c2o, 0, WS["b_2"][:, 0:1])],
            *ROWS["c2o"], 4, 96)
    c3o = PadT(nc, pool, "c3o", 64, "c3o", 1)
    c3o.zero_margins()
    conv3x3("w_3", [(c2o, 96)], [(0, 64, c3o, 0, WS["b_3"][:, 0:1])],
            *ROWS["c3o"], 8, 64)
    xo = PadT(nc, pool, "xo", 32, "x", 1, tag="T_mff_vtmp_xo")
    xo.zero_margins()
    conv3x3("w_4", [(c3o, 64)], [(0, 32, xo, 0, WS["b_4"][:, 0:1])],
            *ROWS["x"], 1, 32)
    _sc.__exit__(None, None, None)

    # ---- tail: single x-stack, plane deconv, up, trade, flow/mask --------
    _sc = scope("tail"); _sc.__enter__()
    rho_lo, rho_hi = RHO
    nrho = rho_hi - rho_lo
    # xs_all[ (dx+1)*32+ci, r, c ] = x[ci, rho_lo-1+r, c+dx]; rows [-1, 34)
    xs_all = pool.tile([96, (nrho + 2) * W], BF16, name="xs_all",
                       tag="T_corr_dsum_xs")
    for dxi, dx in enumerate((-1, 0, 1)):
        nc.sync.dma_start(
            xs_all[32 * dxi:32 * (dxi + 1), :].rearrange("c (r w) -> c r w", w=W),
            xo.ap(rho_lo - 1, rho_hi + 1, 0, dx))

    def xs_rhs(r0, r1, dy):
        o = (r0 + dy + 1 - rho_lo) * W
        return xs_all[:, o:o + (r1 - r0) * W]

    # deconv -> parity-major planes up_pl [ (a*2+b)*16+co, rho*W ]
    up_pl = pool.tile([64, nrho * W], BF16, name="up_pl", tag="T_f1s_upl")
    for r0, r1 in chunks(rho_lo, rho_hi, 8):
        o0 = (r0 - rho_lo) * W
        nn_ = (r1 - r0) * W
        ps = ps_conv.tile([64, nn_], F32, tag="ps_conv")
        for dyi, dy in enumerate((-1, 0, 1)):
            nc.tensor.matmul(ps[0:64, :nn_],
                             lhsT=WS["w_dc"][:, 64 * dyi:64 * (dyi + 1)],
                             rhs=xs_rhs(r0, r1, dy),
                             start=(dyi == 0), stop=(dyi == 2))
        nc.scalar.activation(up_pl[0:64, o0:o0 + nn_], ps[0:64, :nn_], AF.Prelu,
                             bias=WS["b_dc"][:, 0:1], alpha=0.1)

    # up_t (interleaved + 1-px margins) via plane-extract DMA + DVE interleave
    up_rows = UP[1] - UP[0]
    up_t = pool.tile([16, up_rows + 2, 130], BF16, name="up_t", tag="T_c1a_upt")
    nc.vector.memset(up_t[:, 0:1, :], 0.0)
    nc.vector.memset(up_t[:, up_rows + 1:up_rows + 2, :], 0.0)
    nc.vector.memset(up_t[:, 1:up_rows + 1, 0:1], 0.0)
    nc.vector.memset(up_t[:, 1:up_rows + 1, 129:130], 0.0)
    nh = (nrho + 1) // 2
    for half in range(2):
        q0 = rho_lo + nh * half
        q1 = min(rho_hi, q0 + nh)
        nq = (q1 - q0) * W
        up_q = pool.tile([16, 4 * nh * W], BF16, name=f"up_q{half}", tag="T_k_upq")
        for par in range(4):
            nc.sync.dma_start(
                up_q[0:16, par * nq:(par + 1) * nq],
                up_pl[16 * par:16 * (par + 1),
                      (q0 - rho_lo) * W:(q0 - rho_lo) * W + nq])
        for a in range(2):
            for b in range(2):
                par = a * 2 + b
                lr = 1 + 2 * q0 + a - UP[0]
                nr = sum(1 for rho in range(q0, q1)
                         if 0 <= 2 * rho + a - UP[0] < up_rows)
                nc.vector.tensor_copy(
                    up_t[0:16, lr:lr + 2 * nr:2, 1 + b:1 + b + 128:2],
                    up_q[0:16, par * nq:par * nq + nr * W].rearrange(
                        "c (r w) -> c r w", w=W))

    tr_lo, tr_hi = UP_OUT
    for half in range(2):
        h0 = tr_lo + 32 * half
        h1 = h0 + 32
        upsa = pool.tile([128, 32 * 128], BF16, name=f"upsa{half}", tag="T_c1b_upsa")
        for ty in range(3):
            for tx in range(3):
                t = ty * 3 + tx
                if t == 8:
                    continue
                nc.sync.dma_start(
                    upsa[16 * t:16 * (t + 1), :].rearrange("c (r w) -> c r w", w=128),
                    up_t[:, h0 - UP[0] + ty:h0 - UP[0] + ty + 32, tx:tx + 128])
        for r0, r1 in chunks(h0, h1, 4):
            o0 = (r0 - h0) * 128
            nn_ = (r1 - r0) * 128
            ps = ps_conv.tile([64, nn_], F32, tag="ps_conv")
            nc.tensor.matmul(ps[0:64, :nn_], lhsT=WS["w_tr_a"][:, :],
                             rhs=upsa[:, o0:o0 + nn_], start=True, stop=False)
            nc.tensor.matmul(ps[0:64, :nn_], lhsT=WS["w_tr_b"][:, :],
                             rhs=up_t[0:16, r0 - UP[0] + 2:r1 - UP[0] + 2, 2:130],
                             start=False, stop=True)
            st = stage.tile([64, nn_], F32, tag="trstage")
            nc.scalar.activation(st[0:64, :nn_], ps[0:64, :nn_], AF.Identity,
                                 bias=WS["b_tr"][:, 0:1])
            nc.sync.dma_start(
                P["tradeoff_out"][:, (r0 - tr_lo) * 128:(r0 - tr_lo) * 128 + nn_],
                st[0:64, :nn_])

    for r0, r1 in chunks(*UP_OUT, 4):
        st = stage.tile([16, (r1 - r0) * 128], F32, tag="upstage")
        nc.vector.tensor_copy(
            st[0:16, :].rearrange("c (r w) -> c r w", w=128),
            up_t[:, 1 + r0 - UP[0]:1 + r1 - UP[0], 1:129])
        nc.sync.dma_start(
            P["up_out"][:, (r0 - UP_OUT[0]) * 128:(r1 - UP_OUT[0]) * 128],
            st[0:16, :])

    for r0, r1 in chunks(0, 32, 8):
        nn_ = (r1 - r0) * W
        ps = ps_conv.tile([3, nn_], F32, tag="ps_conv")
        for dyi, dy in enumerate((-1, 0, 1)):
            nc.tensor.matmul(ps[0:3, :nn_],
                             lhsT=WS["w_fm"][:, 3 * dyi:3 * (dyi + 1)],
                             rhs=xs_rhs(r0, r1, dy),
                             start=(dyi == 0), stop=(dyi == 2))
        st = stage.tile([3, nn_], F32, tag="trstage")
        nc.scalar.activation(st[0:3, :nn_], ps[0:3, :nn_], AF.Identity,
                             bias=WS["b_fm"][:, 0:1])
        nc.sync.dma_start(P["flow_out"][:, r0 * W:r1 * W], st[0:2, :nn_])
        nc.sync.dma_start(P["mask_out"][:, r0 * W:r1 * W], st[2:3, :nn_])
    _sc.__exit__(None, None, None)

    ctx.close()


# ---------------------------------------------------------------------------
# Host-side sharding / gather


_NC_CACHE = None
_W_CACHE = {}


def _get_nc():
    global _NC_CACHE
    if _NC_CACHE is None:
        _NC_CACHE = build_nc()
    return _NC_CACHE


def build_f1stack(flow_b, flip):
    """Tap-shifted flow copies for the K-stacked convf1: [98, f1rows*W]."""
    v = np.asarray(flow_b, np.float32)
    if flip:
        v = v[:, ::-1, :]
    lo, hi = ROWS["f1out"]
    out = np.zeros((98, hi - lo, W), np.float32)
    for ty in range(7):
        for tx in range(7):
            t = ty * 7 + tx
            r0 = max(lo, -(ty - 3))
            r1 = min(hi, 64 - (ty - 3))
            c0 = max(0, -(tx - 3))
            c1 = min(W, W - (tx - 3))
            if r0 >= r1 or c0 >= c1:
                continue
            out[2 * t:2 * t + 2, r0 - lo:r1 - lo, c0:c1] = \
                v[:, r0 + ty - 3:r1 + ty - 3, c0 + tx - 3:c1 + tx - 3]
    return np.ascontiguousarray(out.reshape(98, -1)).astype(ml_dtypes.bfloat16)


def prepare_in_maps(inputs):
    w0 = prep_weights(inputs, flip=False)
    w1 = prep_weights(inputs, flip=True)
    sel = {
        0: (np.zeros((128, 1), np.float32), np.ones((128, 1), np.float32)),
        1: (np.ones((128, 1), np.float32), np.zeros((128, 1), np.float32)),
    }
    in_maps = []
    for c in range(N_CORES):
        b, h = c // 2, c % 2
        flip = h == 1

        def rows(a, key):
            lo, hi = ROWS[key]
            v = a[:, ::-1, :] if flip else a
            return np.ascontiguousarray(v[:, lo:hi, :], dtype=np.float32).reshape(a.shape[0], -1)

        x1 = inputs["x1"][b]
        v = x1[NET:]
        v = v[:, ::-1, :] if flip else v
        m = {"inp": np.ascontiguousarray(v, dtype=np.float32).reshape(128, HW)}
        m["net"] = rows(x1[:NET], "net")
        m["upf"] = rows(inputs["upfeat"][b], "net")
        m["corr"] = rows(inputs["corr"][b], "corr")
        m["f1stk"] = build_f1stack(inputs["flow"][b], flip)
        wd = w1 if flip else w0
        for k, v2 in wd.items():
            m[k] = v2
        m["sel0"], m["sel1"] = sel[h]
        in_maps.append(m)
    return in_maps


def assemble_outputs(results):
    B = 4
    tradeoff = np.zeros((B, 64, 128, 128), np.float32)
    up = np.zeros((B, 16, 128, 128), np.float32)
    flow_out = np.zeros((B, 2, 64, 64), np.float32)
    mask = np.zeros((B, 1, 64, 64), np.float32)
    for c in range(N_CORES):
        b, h = c // 2, c % 2
        r = results[c]
        tr = r["tradeoff_out"].reshape(64, 64, 128)
        u = r["up_out"].reshape(16, 64, 128)
        fo = r["flow_out"].reshape(2, 32, 64)
        mo = r["mask_out"].reshape(1, 32, 64)
        if h == 1:
            tr, u, fo, mo = tr[:, ::-1], u[:, ::-1], fo[:, ::-1], mo[:, ::-1]
            tradeoff[b, :, 64:128] = tr
            up[b, :, 64:128] = u
            flow_out[b, :, 32:64] = fo
            mask[b, :, 32:64] = mo
        else:
            tradeoff[b, :, 0:64] = tr
            up[b, :, 0:64] = u
            flow_out[b, :, 0:32] = fo
            mask[b, :, 0:32] = mo
    return (tradeoff, up, flow_out, mask)


def run(inputs, trace=False, **kw):
    nc = _get_nc()
    in_maps = prepare_in_maps(inputs)
    res = run_bass_kernel_spmd(nc, in_maps, core_ids=list(range(N_CORES)),
                               trace=trace, **kw)
    return assemble_outputs(res.results), res


def kernel(**inputs):
    outs, _ = run(inputs)
    return outs


# revision 24
# speedup vs baseline: 1.0945x; 1.0057x over previous
"""Trainium2 8-core Bass kernel for nn_AttAggFME.

Sharding: core c = (batch b=c//2, half h=c%2). Every core runs IDENTICAL
code with "top half" geometry; h=1 cores receive vertically flipped data
(host flips rows) and dy-tap-flipped conv weights, and their outputs are
flipped back on the host.

Per core: 2048 attention queries (own rows), motion encoder on own rows +
16 halo rows (halo comes free from host-sliced inputs), conv1..4 with halo
recompute. Two intra-pair AllGathers: V^T (full 4096 values for attn@V) and
16 halo rows of motion_fea_global before conv1. Cross-core row-order and
rank-offset asymmetries are handled with reverse-on-send plus host-supplied
0/1 selector masks.

All matmuls bf16 with f32 PSUM accumulation; softmax logits stay f32 in
PSUM, exp on ScalarE; denominators via ones-matmul (scores are computed
transposed [keys, queries] so attn@V needs no transposes).
"""

import numpy as np
import ml_dtypes

import concourse.bass as bass
import concourse.tile as tile
from concourse import mybir, bacc
from concourse.bass_utils import run_bass_kernel_spmd

F32 = mybir.dt.float32
BF16 = mybir.dt.bfloat16
AF = mybir.ActivationFunctionType

N_CORES = 8
H = W = 64
HW = H * W
D = 128
NET = 128
GROUPS = [[0, 1], [2, 3], [4, 5], [6, 7]]

# Row ranges (own-frame; every core is "top half": own rows [0:32))
ROWS = {
    "own":   (0, 32),
    "mf":    (0, 48),   # motion_fea (conv1 needs 16 halo rows)
    "mfg":   (0, 48),   # motion_fea_global (own 0:32 + partner halo 32:48)
    "c1o":   (0, 47),   # conv1 out
    "c2o":   (0, 43),   # conv2 out
    "c3o":   (0, 35),   # conv3 out
    "x":     (0, 34),   # conv4 out
    "cor":   (0, 49),   # convc2 out
    "c1out": (0, 50),   # convc1 out
    "f1out": (0, 50),   # convf1 out
    "flo":   (0, 49),   # convf2 out
    "corr":  (0, 50),   # corr input rows shipped
    "flow":  (0, 53),   # flow input rows shipped
    "net":   (0, 48),   # net / upfeat input rows shipped
}
RHO = (0, 33)       # deconv x-row parity range
UP = (0, 66)        # up rows stored
UP_OUT = (0, 64)    # up rows output / trade out rows

TAPS3 = [(ty - 1, tx - 1) for ty in range(3) for tx in range(3)]


def _nrows(key):
    lo, hi = ROWS[key]
    return hi - lo


PARAM_SPECS = [
    # activations (f32, own-frame)
    ("inp",  [128, HW], F32),
    ("net",  [128, _nrows("net") * W], F32),
    ("upf",  [128, _nrows("net") * W], F32),
    ("corr", [81, _nrows("corr") * W], F32),
    ("f1stk", [98, _nrows("f1out") * W], BF16),
    # weights (bf16) / biases (f32) / selectors
    ("w_qk", [128, 256], BF16),
    ("w_c1", [81, 256], BF16), ("b_c1a", [128, 1], F32), ("b_c1b", [128, 1], F32),
    ("w_c2", [128, 9 * 2 * 192], BF16), ("b_c2a", [128, 1], F32), ("b_c2b", [64, 1], F32),
    ("w_f1", [98, 128], BF16), ("b_f1", [128, 1], F32),
    ("w_f2", [128, 9 * 64], BF16), ("b_f2p", [128, 1], F32),
    ("w_cm", [128, 9 * 2 * 126], BF16), ("b_cm", [126, 1], F32),
    ("w_v", [128, 128], BF16),
    ("w_1", [128, 9 * 4 * 128], BF16), ("b_1", [128, 1], F32),
    ("w_2", [128, 9 * 96], BF16), ("b_2", [96, 1], F32),
    ("w_3", [96, 9 * 64], BF16), ("b_3", [64, 1], F32),
    ("w_4", [64, 9 * 32], BF16), ("b_4", [32, 1], F32),
    ("w_dc", [96, 3 * 64], BF16), ("b_dc", [64, 1], F32),
    ("w_tr_a", [128, 64], BF16), ("w_tr_b", [16, 64], BF16), ("b_tr", [64, 1], F32),
    ("w_fm", [96, 3 * 3], BF16), ("b_fm", [3, 1], F32),
    ("sel0", [128, 1], F32), ("sel1", [128, 1], F32),
]


# ---------------------------------------------------------------------------
# Host-side weight prep


def prep_weights(inp, flip):
    """All conv weights packed for the kernel; flip=True mirrors dy taps."""
    def bf(a):
        return np.ascontiguousarray(a, dtype=np.float32).astype(ml_dtypes.bfloat16)

    def col(a, n=None, off=0):
        a = np.asarray(a, np.float32).reshape(-1)
        n = n or a.shape[0]
        out = np.zeros((n, 1), np.float32)
        out[off:off + a.shape[0], 0] = a
        return out

    def fl(wt):  # [Cout, Cin, kh, kw] -> dy mirrored
        return wt[:, :, ::-1, :] if flip else wt

    w = {}
    qk = inp["att_to_qk_w"][:, :, 0, 0].astype(np.float64).T.copy()  # [128, 256]
    qk[:, :D] *= D ** -0.5
    w["w_qk"] = bf(qk)

    w["w_c1"] = bf(inp["convc1_w"][:, :, 0, 0].T)
    w["b_c1a"] = col(inp["convc1_b"][:128])
    w["b_c1b"] = col(inp["convc1_b"][128:])

    def conv3x3(wt):
        wt = fl(wt)
        Cout, Cin = wt.shape[:2]
        nkt = (Cin + 127) // 128
        K = 128 if nkt > 1 else Cin
        blocks = []
        for (dy, dx) in TAPS3:
            for kt in range(nkt):
                cs, ce = kt * 128, min(Cin, (kt + 1) * 128)
                blk = np.zeros((K, Cout), np.float64)
                blk[: ce - cs] = wt[:, cs:ce, dy + 1, dx + 1].T
                blocks.append(blk)
        return bf(np.concatenate(blocks, axis=1))

    w["w_c2"] = conv3x3(inp["convc2_w"])
    w["b_c2a"] = col(inp["convc2_b"][:128])
    w["b_c2b"] = col(inp["convc2_b"][128:])

    f1 = fl(inp["convf1_w"])
    lf1 = np.zeros((98, 128), np.float64)
    for ty in range(7):
        for tx in range(7):
            for ci in range(2):
                lf1[2 * (ty * 7 + tx) + ci] = f1[:, ci, ty, tx]
    w["w_f1"] = bf(lf1)
    w["b_f1"] = col(inp["convf1_b"])

    w["w_f2"] = conv3x3(inp["convf2_w"])
    w["b_f2p"] = col(inp["convf2_b"], n=128, off=64)

    w["w_cm"] = conv3x3(inp["conv_motion_w"])
    w["b_cm"] = col(inp["conv_motion_b"])

    gv = float(np.asarray(inp["gamma"]).reshape(-1)[0]) * inp["agg_to_v_w"][:, :, 0, 0]
    w["w_v"] = bf(gv.T)

    w["w_1"] = conv3x3(inp["conv1_w"]); w["b_1"] = col(inp["conv1_b"])
    w["w_2"] = conv3x3(inp["conv2_w"]); w["b_2"] = col(inp["conv2_b"])
    w["w_3"] = conv3x3(inp["conv3_w"]); w["b_3"] = col(inp["conv3_b"])
    w["w_4"] = conv3x3(inp["conv4_w"]); w["b_4"] = col(inp["conv4_b"])

    # deconv: ConvTranspose2d(k4,s2,p1) w [32ci,16co,4,4]; flip: kh -> 3-kh
    dw = inp["upfeat_w"].astype(np.float64)
    if flip:
        dw = dw[:, :, ::-1, :]
    ldc = np.zeros((3, 96, 64), np.float64)
    for dyi, dy in enumerate((-1, 0, 1)):
        for dxi, dx in enumerate((-1, 0, 1)):
            for a in range(2):
                kh = a + 1 - 2 * dy
                if not (0 <= kh < 4) or (a == 0 and dy not in (0, -1)) or (a == 1 and dy not in (0, 1)):
                    continue
                for b in range(2):
                    kw = b + 1 - 2 * dx
                    if not (0 <= kw < 4) or (b == 0 and dx not in (0, -1)) or (b == 1 and dx not in (0, 1)):
                        continue
                    ldc[dyi, dxi * 32:dxi * 32 + 32, (a * 2 + b) * 16:(a * 2 + b) * 16 + 16] = dw[:, :, kh, kw]
    w["w_dc"] = bf(ldc.transpose(1, 0, 2).reshape(96, 3 * 64))
    w["b_dc"] = col(np.tile(np.asarray(inp["upfeat_b"], np.float64), 4))

    tw = fl(inp["trade_w"]).astype(np.float64)
    la = np.zeros((128, 64), np.float64)
    lb = np.zeros((16, 64), np.float64)
    for ty in range(3):
        for tx in range(3):
            t = ty * 3 + tx
            blk = tw[:, :, ty, tx].T
            if t < 8:
                la[t * 16:(t + 1) * 16] = blk
            else:
                lb[:] = blk
    w["w_tr_a"] = bf(la)
    w["w_tr_b"] = bf(lb)
    w["b_tr"] = col(inp["trade_b"])

    fw = fl(inp["flow_w"]).astype(np.float64)
    mw = fl(inp["mask_w"]).astype(np.float64)
    lfm = np.zeros((3, 96, 3), np.float64)
    for dyi, dy in enumerate((-1, 0, 1)):
        for dxi, dx in enumerate((-1, 0, 1)):
            lfm[dyi, dxi * 32:dxi * 32 + 32, 0:2] = fw[:, :, dy + 1, dx + 1].T
            lfm[dyi, dxi * 32:dxi * 32 + 32, 2] = mw[0, :, dy + 1, dx + 1]
    w["w_fm"] = bf(lfm.transpose(1, 0, 2).reshape(96, 3 * 3))
    w["b_fm"] = col(np.concatenate([np.asarray(inp["flow_b"]), np.asarray(inp["mask_b"])]))
    return w


# ---------------------------------------------------------------------------
# Padded spatial SBUF tensors


class PadT:
    """SBUF tile [C, rtot, stride] with `pad` zeroed margin rows/cols; g0 =
    own-frame row of the first real row."""

    def __init__(self, nc, pool, name, C, key_or_range, pad, dtype=BF16, tag=None):
        g0, g1 = ROWS[key_or_range] if isinstance(key_or_range, str) else key_or_range
        self.nc, self.C, self.g0, self.rows, self.pad = nc, C, g0, g1 - g0, pad
        self.stride = W + 2 * pad
        self.rtot = self.rows + 2 * pad
        self.t = pool.tile([C, self.rtot, self.stride], dtype, name=name,
                           tag=tag or name)

    def zero_margins(self):
        nc, p = self.nc, self.pad
        if p == 0:
            return
        nc.gpsimd.memset(self.t[:, 0:p, :], 0.0)
        nc.gpsimd.memset(self.t[:, self.rtot - p:self.rtot, :], 0.0)
        nc.gpsimd.memset(self.t[:, p:p + self.rows, 0:p], 0.0)
        nc.gpsimd.memset(self.t[:, p:p + self.rows, self.stride - p:self.stride], 0.0)

    def ap(self, r_lo, r_hi, dy=0, dx=0, c_lo=0, c_hi=None):
        c_hi = self.C if c_hi is None else c_hi
        a = r_lo - self.g0 + self.pad + dy
        b = r_hi - self.g0 + self.pad + dy
        assert 0 <= a and b <= self.rtot, (r_lo, r_hi, dy, self.g0, self.rows)
        assert 0 <= self.pad + dx and dx <= self.pad
        return self.t[c_lo:c_hi, a:b, self.pad + dx:self.pad + dx + W]


def chunks(lo, hi, step):
    r = lo
    while r < hi:
        yield r, min(hi, r + step)
        r += step


# ---------------------------------------------------------------------------
# Graph build


def build_nc():
    nc = bacc.Bacc()
    P = {}
    for name, shape, dt in PARAM_SPECS:
        P[name] = nc.declare_dram_parameter(name, shape, dt, isOutput=False)
    P["tradeoff_out"] = nc.declare_dram_parameter("tradeoff_out", [64, 64 * 128], F32, isOutput=True)
    P["up_out"] = nc.declare_dram_parameter("up_out", [16, 64 * 128], F32, isOutput=True)
    P["flow_out"] = nc.declare_dram_parameter("flow_out", [2, 32 * W], F32, isOutput=True)
    P["mask_out"] = nc.declare_dram_parameter("mask_out", [1, 32 * W], F32, isOutput=True)

    with tile.TileContext(nc) as tc:
        _emit(nc, tc, P)
    nc.finalize()
    return nc


def _emit(nc, tc, P):
    from contextlib import ExitStack
    ctx = ExitStack()
    pool = ctx.enter_context(tc.tile_pool(name="main", bufs=1))
    stage = ctx.enter_context(tc.tile_pool(name="stage", bufs=3))
    small = ctx.enter_context(tc.tile_pool(name="small", bufs=2))
    dram = ctx.enter_context(tc.tile_pool(name="dram", bufs=1, space="DRAM"))
    ps_conv = ctx.enter_context(tc.tile_pool(name="ps_conv", bufs=2, space="PSUM"))
    ps_scores = ctx.enter_context(tc.tile_pool(name="ps_scores", bufs=2, space="PSUM"))
    ps_agg = ctx.enter_context(tc.tile_pool(name="ps_agg", bufs=2, space="PSUM"))

    scope = nc.named_scope

    # ---- weights, emitted in order of first use --------------------------
    WS = {}

    def loadw(*names):
        for name in names:
            spec = next(s for s in PARAM_SPECS if s[0] == name)
            t = pool.tile(spec[1], spec[2], name=f"sb_{name}")
            nc.sync.dma_start(t[:], P[name][:])
            WS[name] = t

    def wslice(name, tap, kt, nkt, m_lo, m_hi, Cout, K=128):
        base = (tap * nkt + kt) * Cout
        return WS[name][0:K, base + m_lo:base + m_hi]

    # ---- input casts -----------------------------------------------------
    def load_cast_flat(dst, dram_p, C, total):
        for lo, hi in chunks(0, total, 1024):
            st = stage.tile([C, hi - lo], F32, tag="f32stage")
            nc.sync.dma_start(st[0:C, 0:hi - lo], dram_p[0:C, lo:hi])
            nc.vector.tensor_copy(dst[0:C, lo:hi], st[0:C, 0:hi - lo])

    def load_cast_padt(dst, dram_p, C, key):
        lo, hi = ROWS[key]
        for r0, r1 in chunks(lo, hi, 16):
            st = stage.tile([C, (r1 - r0) * W], F32, tag="f32stage")
            nc.sync.dma_start(st[0:C, 0:(r1 - r0) * W],
                              dram_p[0:C, (r0 - lo) * W:(r1 - lo) * W])
            nc.vector.tensor_copy(
                dst.ap(r0, r1, c_lo=0, c_hi=C),
                st[0:C, 0:(r1 - r0) * W].rearrange("c (r w) -> c r w", w=W))

    loadw("w_c1", "b_c1a", "b_c1b")
    crows = _nrows("corr")
    corr_bf = pool.tile([81, crows * W], BF16, name="corr_bf", tag="T_corr_dsum_xs")
    load_cast_flat(corr_bf, P["corr"], 81, crows * W)
    f1rows0 = _nrows("f1out")
    f1stack = pool.tile([98, f1rows0 * W], BF16, name="f1stack", tag="T_f1s_upl")
    nc.sync.dma_start(f1stack[:, :], P["f1stk"][:, :])

    loadw("w_f1", "b_f1", "w_c2", "b_c2a", "b_c2b")

    inp_bf = pool.tile([128, HW], BF16, name="inp_bf", tag="T_inp_xs2")
    load_cast_flat(inp_bf, P["inp"], 128, HW)

    loadw("w_f2", "b_f2p", "w_cm", "b_cm", "w_v", "w_qk", "sel0", "sel1",
          "w_1", "b_1", "w_2", "b_2", "w_3", "b_3", "w_4", "b_4",
          "w_dc", "b_dc", "w_tr_a", "w_tr_b", "b_tr", "w_fm", "b_fm")

    net_bf = PadT(nc, pool, "net_bf", 128, "net", 1)
    upf_bf = PadT(nc, pool, "upf_bf", 128, "net", 1)
    net_bf.zero_margins(); upf_bf.zero_margins()
    load_cast_padt(net_bf, P["net"], 128, "net")
    load_cast_padt(upf_bf, P["upf"], 128, "net")

    ones_col = pool.tile([128, 1], BF16, name="ones_col")
    nc.gpsimd.memset(ones_col[:, :], 1.0)
    ones_row = pool.tile([1, 128], BF16, name="ones_row")
    nc.gpsimd.memset(ones_row[:, :], 1.0)

    # ---- generic 3x3 conv ------------------------------------------------
    def conv3x3(wname, srcs, parts, out_lo, out_hi, dil, Cout, act="lrelu",
                kt_major=False):
        nkt = len(srcs)
        order = [(ti, kt) for kt in range(nkt) for ti in range(9)] if kt_major \
            else [(ti, kt) for ti in range(9) for kt in range(nkt)]
        for r0, r1 in chunks(out_lo, out_hi, 8):
            for m_lo, m_hi, dstt, poff, bias_ap in parts:
                mn = m_hi - m_lo
                ps = ps_conv.tile([poff + mn, (r1 - r0) * W], F32, tag="ps_conv")
                for oi, (ti, kt) in enumerate(order):
                    dy, dx = TAPS3[ti]
                    src, K = srcs[kt]
                    nc.tensor.matmul(
                        ps[poff:poff + mn, :],
                        lhsT=wslice(wname, ti, kt, nkt, m_lo, m_hi, Cout, K),
                        rhs=src.ap(r0, r1, dy * dil, dx * dil, 0, K),
                        start=(oi == 0), stop=(oi == len(order) - 1),
                        tile_position=(0, poff) if poff else None)
                o = dstt.ap(r0, r1, 0, 0, poff, poff + mn)
                if act == "lrelu":
                    nc.scalar.activation(o, ps[poff:poff + mn, :], AF.Prelu,
                                         bias=bias_ap, alpha=0.1)
                else:
                    nc.scalar.activation(o, ps[poff:poff + mn, :], AF.Identity,
                                         bias=bias_ap)

    # ---- motion encoder (phase A: rows feeding own V^T; phase B: halo) ---
    _sc = scope("motencA"); _sc.__enter__()
    c1out_a = PadT(nc, pool, "c1out_a", 128, "c1out", 1, tag="T_c1a_upt")
    c1out_b = PadT(nc, pool, "c1out_b", 128, "c1out", 1, tag="T_c1b_upsa")
    c1out_a.zero_margins(); c1out_b.zero_margins()

    def convc1(lo, hi):
        for r0, r1 in chunks(lo, hi, 8):
            off = (r0 - ROWS["corr"][0]) * W
            nn_ = (r1 - r0) * W
            for dstt, m_lo, bias in ((c1out_a, 0, "b_c1a"), (c1out_b, 128, "b_c1b")):
                ps = ps_conv.tile([128, nn_], F32, tag="ps_conv")
                nc.tensor.matmul(ps[:, :], lhsT=WS["w_c1"][:, m_lo:m_lo + 128],
                                 rhs=corr_bf[:, off:off + nn_], start=True, stop=True)
                nc.scalar.activation(dstt.ap(r0, r1), ps[:, :], AF.Prelu,
                                     bias=WS[bias][:, 0:1], alpha=0.1)

    f1_lo = ROWS["f1out"][0]
    f1s3d = f1stack[:, :].rearrange("c (r w) -> c r w", w=W)

    f1out = PadT(nc, pool, "f1out", 128, "f1out", 1, tag="T_f1o_c3o")
    f1out.zero_margins()

    def convf1(lo, hi):
        for r0, r1 in chunks(lo, hi, 8):
            ps = ps_conv.tile([128, (r1 - r0) * W], F32, tag="ps_conv")
            o0 = (r0 - ROWS["f1out"][0]) * W
            nc.tensor.matmul(ps[:, :], lhsT=WS["w_f1"][:, :],
                             rhs=f1stack[:, o0:o0 + (r1 - r0) * W],
                             start=True, stop=True)
            nc.scalar.activation(f1out.ap(r0, r1), ps[:, :], AF.Prelu,
                                 bias=WS["b_f1"][:, 0:1], alpha=0.1)

    cf0 = PadT(nc, pool, "cf0", 128, "cor", 1, tag="T_cf0_c1o")
    cf1 = PadT(nc, pool, "cf1", 128, "cor", 1, tag="T_cf1_c2o")
    cf0.zero_margins(); cf1.zero_margins()

    def convc2(lo, hi):
        conv3x3("w_c2", [(c1out_a, 128), (c1out_b, 128)],
                [(0, 128, cf0, 0, WS["b_c2a"][:, 0:1]),
                 (128, 192, cf1, 0, WS["b_c2b"][0:64, 0:1])],
                lo, hi, 1, 192)

    def convf2(lo, hi):
        for r0, r1 in chunks(lo, hi, 8):
            ps = ps_conv.tile([128, (r1 - r0) * W], F32, tag="ps_conv")
            for ti in range(9):
                dy, dx = TAPS3[ti]
                nc.tensor.matmul(ps[64:128, :],
                                 lhsT=wslice("w_f2", ti, 0, 1, 0, 64, 64),
                                 rhs=f1out.ap(r0, r1, dy, dx),
                                 start=(ti == 0), stop=(ti == 8),
                                 tile_position=(0, 64))
            nc.scalar.activation(cf1.ap(r0, r1, 0, 0, 64, 128), ps[64:128, :],
                                 AF.Prelu, bias=WS["b_f2p"][64:128, 0:1], alpha=0.1)

    mf = PadT(nc, pool, "mf", 128, "mf", 1)

    def convcm(lo, hi):
        conv3x3("w_cm", [(cf0, 128), (cf1, 128)],
                [(0, 126, mf, 0, WS["b_cm"][:, 0:1])], lo, hi, 1, 126)

    def mf_concat(lo, hi):
        nc.sync.dma_start(mf.ap(lo, hi, c_lo=126, c_hi=128),
                          f1s3d[48:50, lo - f1_lo:hi - f1_lo, 0:W])

    # phase A: everything needed for own-rows V^T
    convc1(0, 34)
    convf1(0, 34)
    convc2(0, 33)
    convf2(0, 33)
    mf.zero_margins()
    convcm(0, 32)
    mf_concat(0, 32)
    _sc.__exit__(None, None, None)

    # ---- V^T + collective #1 (reverse-on-send) ---------------------------
    _sc = scope("vt_coll"); _sc.__enter__()
    vt_all = pool.tile([128, 32 * 128], BF16, name="vt_all")
    vt_in = dram.tile([2048, 128], BF16, name="vt_in")
    vt_out = dram.tile([4096, 128], BF16, name="vt_out")
    mf_flat = pool.tile([128, 2048], BF16, name="mf_flat", tag="T_mff_vtmp_xo")
    nc.vector.tensor_copy(mf_flat[:, :].rearrange("c (r w) -> c r w", w=W),
                          mf.ap(0, 32))
    for jt in range(16):
        ps = ps_conv.tile([128, 128], F32, tag="ps_conv")
        nc.tensor.matmul(ps[:, :], lhsT=mf_flat[:, 128 * jt:128 * (jt + 1)],
                         rhs=WS["w_v"][:, :], start=True, stop=True)
        nc.vector.tensor_copy(vt_all[:, 128 * jt:128 * (jt + 1)], ps[:, :])
        for rr in range(2):
            r = 2 * jt + rr
            nc.sync.dma_start(vt_in[(31 - r) * 64:(31 - r) * 64 + 64, :],
                              vt_all[64 * rr:64 * (rr + 1), 128 * jt:128 * (jt + 1)])
    nc.gpsimd.collective_compute("AllGather", mybir.AluOpType.bypass,
                                 replica_groups=GROUPS,
                                 ins=[vt_in.opt()], outs=[vt_out.opt()])
    _sc.__exit__(None, None, None)

    # ---- q, k (fills the collective wait) --------------------------------
    _sc = scope("qk"); _sc.__enter__()
    k_bf = pool.tile([128, HW], BF16, name="k_bf", tag="T_k_upq")
    q_bf = pool.tile([128, 2048], BF16, name="q_bf", tag="T_q_hblk1")
    for lo, hi in chunks(0, HW, 512):
        ps = ps_conv.tile([128, 512], F32, tag="ps_conv")
        nc.tensor.matmul(ps[:, :], lhsT=WS["w_qk"][:, 128:256],
                         rhs=inp_bf[:, lo:hi], start=True, stop=True)
        nc.vector.tensor_copy(k_bf[:, lo:hi], ps[:, :])
    for lo, hi in chunks(0, 2048, 512):
        ps = ps_conv.tile([128, 512], F32, tag="ps_conv")
        nc.tensor.matmul(ps[:, :], lhsT=WS["w_qk"][:, 0:128],
                         rhs=inp_bf[:, lo:hi], start=True, stop=True)
        nc.vector.tensor_copy(q_bf[:, lo:hi], ps[:, :])
    _sc.__exit__(None, None, None)

    # ---- motion encoder phase B (halo rows; also fills collective wait) --
    _sc = scope("motencB"); _sc.__enter__()
    convc1(34, 50)
    convf1(34, 50)
    convc2(33, 49)
    convf2(33, 49)
    convcm(32, 48)
    mf_concat(32, 48)
    _sc.__exit__(None, None, None)

    # ---- collective #1 receive (mask-select, partner rows ascend mine) ---
    _sc = scope("vt_recv"); _sc.__enter__()
    selp1 = pool.tile([128, 16 * 128], BF16, name="selp1", tag="T_selp1")
    nc.sync.dma_start(
        vt_all[:, 2048:4096].rearrange("p (j d) -> p j d", d=128),
        vt_out[0:2048, :].rearrange("(j p) d -> p j d", p=128))
    nc.sync.dma_start(
        selp1[:, :].rearrange("p (j d) -> p j d", d=128),
        vt_out[2048:4096, :].rearrange("(j p) d -> p j d", p=128))
    nc.vector.tensor_scalar_mul(vt_all[:, 2048:4096], vt_all[:, 2048:4096],
                                WS["sel0"][:, 0:1])
    nc.vector.tensor_scalar_mul(selp1[:, :], selp1[:, :], WS["sel1"][:, 0:1])
    nc.vector.tensor_add(vt_all[:, 2048:4096], vt_all[:, 2048:4096], selp1[:, :])
    _sc.__exit__(None, None, None)

    # ---- attention (i-chunks in order 2,3,0,1; pipelined normalize) ------
    _sc = scope("attn"); _sc.__enter__()
    mfg = PadT(nc, pool, "mfg", 128, "mfg", 1, tag="T_cf1_c2o")
    mfg.zero_margins()

    aggs = {}
    dsbs = {}

    def jloop(ic):
        i0 = ic * 512
        agg_ps = ps_agg.tile([128, 512], F32, tag="ps_agg")
        aggs[ic] = agg_ps
        dsum = pool.tile([128, 1024], BF16, name=f"dsum{ic}", tag="T_corr_dsum_xs")
        for jg in range(16):
            sc_ps = ps_scores.tile([128, 1024], F32, tag="ps_sc")
            for u in range(2):
                j = 2 * jg + u
                nc.tensor.matmul(sc_ps[:, 512 * u:512 * (u + 1)],
                                 lhsT=k_bf[:, 128 * j:128 * (j + 1)],
                                 rhs=q_bf[:, i0:i0 + 512], start=True, stop=True)
            pb = small.tile([128, 1024], BF16, tag="probs")
            nc.scalar.activation(pb[:, :], sc_ps[:, :], AF.Exp)
            if jg == 0:
                nc.vector.tensor_copy(dsum[:, :], pb[:, :])
            else:
                nc.vector.tensor_add(dsum[:, :], dsum[:, :], pb[:, :])
            for u in range(2):
                j = 2 * jg + u
                nc.tensor.matmul(agg_ps[:, :],
                                 lhsT=vt_all[:, 128 * j:128 * (j + 1)],
                                 rhs=pb[:, 512 * u:512 * (u + 1)],
                                 start=(j == 0), stop=(j == 31),
                                 skip_group_check=True)
        dsb = small.tile([128, 512], BF16, tag="dsb")
        nc.vector.tensor_add(dsb[:, :], dsum[:, 0:512], dsum[:, 512:1024])
        dsbs[ic] = dsb

    def norm_rest(ic):
        agg_ps = aggs.pop(ic)
        dsb = dsbs.pop(ic)
        sums_ps = ps_conv.tile([1, 512], F32, tag="ps_conv")
        nc.tensor.matmul(sums_ps[0:1, :], lhsT=ones_col[:, :], rhs=dsb[:, :],
                         start=True, stop=True)
        recip = small.tile([1, 512], F32, tag="recip")
        nc.vector.reciprocal(recip[0:1, :], sums_ps[0:1, :])
        recip_bf = small.tile([1, 512], BF16, tag="recipbf")
        nc.vector.tensor_copy(recip_bf[0:1, :], recip[0:1, :])
        bc_ps = ps_conv.tile([128, 512], F32, tag="ps_conv")
        nc.tensor.matmul(bc_ps[:, :], lhsT=ones_row[0:1, :], rhs=recip_bf[0:1, :],
                         start=True, stop=True)
        bc_sb = small.tile([128, 512], F32, tag="nrm_f32")
        nc.scalar.copy(bc_sb[:, :], bc_ps[:, :])
        agg_sb = small.tile([128, 512], F32, tag="nrm_f32")
        nc.vector.tensor_mul(agg_sb[:, :], agg_ps[:, :], bc_sb[:, :])
        r0 = ic * 8
        nc.vector.tensor_add(mfg.ap(r0, r0 + 8),
                             agg_sb[:, :].rearrange("c (r w) -> c r w", w=W),
                             mf.ap(r0, r0 + 8))

    mfg_in = dram.tile([128, 16 * W], BF16, name="mfg_in")
    mfg_out = dram.tile([256, 16 * W], BF16, name="mfg_out")

    jloop(0)
    jloop(1)
    norm_rest(0)
    jloop(2)
    norm_rest(1)
    jloop(3)
    norm_rest(2)
    norm_rest(3)
    # ---- collective #2 launch (edge rows 16..31 ready) -------------------
    for r in range(16, 32):
        nc.sync.dma_start(mfg_in[:, (31 - r) * W:(31 - r) * W + W], mfg.ap(r, r + 1))
    nc.gpsimd.collective_compute("AllGather", mybir.AluOpType.bypass,
                                 replica_groups=GROUPS,
                                 ins=[mfg_in.opt()], outs=[mfg_out.opt()])
    _sc.__exit__(None, None, None)

    # ---- collective #2 receive -------------------------------------------
    _sc = scope("halo"); _sc.__enter__()
    hblk1 = pool.tile([128, 16 * W], BF16, name="hblk1", tag="T_selp1")
    nc.sync.dma_start(mfg.ap(32, 48), mfg_out[0:128, :].rearrange("c (r w) -> c r w", w=W))
    nc.sync.dma_start(hblk1[:, :], mfg_out[128:256, :])
    nc.vector.tensor_scalar_mul(mfg.ap(32, 48), mfg.ap(32, 48), WS["sel0"][:, 0:1])
    nc.vector.tensor_scalar_mul(hblk1[:, :], hblk1[:, :], WS["sel1"][:, 0:1])
    nc.vector.tensor_add(mfg.ap(32, 48), mfg.ap(32, 48),
                         hblk1[:, :].rearrange("c (r w) -> c r w", w=W))
    _sc.__exit__(None, None, None)

    # ---- conv stack ------------------------------------------------------
    _sc = scope("convs"); _sc.__enter__()
    c1o = PadT(nc, pool, "c1o", 128, "c1o", 4)
    c1o.zero_margins()
    conv3x3("w_1", [(net_bf, 128), (upf_bf, 128), (mf, 128), (mfg, 128)],
            [(0, 128, c1o, 0, WS["b_1"][:, 0:1])], *ROWS["c1o"], 1, 128,
            kt_major=True)
    c2o = PadT(nc, pool, "c2o", 96, "c2o", 8)
    c2o.zero_margins()
    conv3x3("w_2", [(c1o, 128)], [(0, 96, c2o, 0, WS["b_2"][:, 0:1])],
            *ROWS["c2o"], 4, 96)
    c3o = PadT(nc, pool, "c3o", 64, "c3o", 1)
    c3o.zero_margins()
    conv3x3("w_3", [(c2o, 96)], [(0, 64, c3o, 0, WS["b_3"][:, 0:1])],
            *ROWS["c3o"], 8, 64)
    xo = PadT(nc, pool, "xo", 32, "x", 1, tag="T_mff_vtmp_xo")
    xo.zero_margins()
    conv3x3("w_4", [(c3o, 64)], [(0, 32, xo, 0, WS["b_4"][:, 0:1])],
            *ROWS["x"], 1, 32)
    _sc.__exit__(None, None, None)

    # ---- tail: single x-stack, plane deconv, up, trade, flow/mask --------
    _sc = scope("tail"); _sc.__enter__()
    rho_lo, rho_hi = RHO
    nrho = rho_hi - rho_lo
    # xs_all[ (dx+1)*32+ci, r, c ] = x[ci, rho_lo-1+r, c+dx]; rows [-1, 34)
    xs_all = pool.tile([96, (nrho + 2) * W], BF16, name="xs_all",
                       tag="T_corr_dsum_xs")
    for dxi, dx in enumerate((-1, 0, 1)):
        nc.sync.dma_start(
            xs_all[32 * dxi:32 * (dxi + 1), :].rearrange("c (r w) -> c r w", w=W),
            xo.ap(rho_lo - 1, rho_hi + 1, 0, dx))

    def xs_rhs(r0, r1, dy):
        o = (r0 + dy + 1 - rho_lo) * W
        return xs_all[:, o:o + (r1 - r0) * W]

    # deconv -> parity-major planes up_pl [ (a*2+b)*16+co, rho*W ]
    up_pl = pool.tile([64, nrho * W], BF16, name="up_pl", tag="T_f1s_upl")
    for r0, r1 in chunks(rho_lo, rho_hi, 8):
        o0 = (r0 - rho_lo) * W
        nn_ = (r1 - r0) * W
        ps = ps_conv.tile([64, nn_], F32, tag="ps_conv")
        for dyi, dy in enumerate((-1, 0, 1)):
            nc.tensor.matmul(ps[0:64, :nn_],
                             lhsT=WS["w_dc"][:, 64 * dyi:64 * (dyi + 1)],
                             rhs=xs_rhs(r0, r1, dy),
                             start=(dyi == 0), stop=(dyi == 2))
        nc.scalar.activation(up_pl[0:64, o0:o0 + nn_], ps[0:64, :nn_], AF.Prelu,
                             bias=WS["b_dc"][:, 0:1], alpha=0.1)

    # up_t (interleaved + 1-px margins) via plane-extract DMA + DVE interleave
    up_rows = UP[1] - UP[0]
    up_t = pool.tile([16, up_rows + 2, 130], BF16, name="up_t", tag="T_c1a_upt")
    nc.vector.memset(up_t[:, 0:1, :], 0.0)
    nc.vector.memset(up_t[:, up_rows + 1:up_rows + 2, :], 0.0)
    nc.vector.memset(up_t[:, 1:up_rows + 1, 0:1], 0.0)
    nc.vector.memset(up_t[:, 1:up_rows + 1, 129:130], 0.0)
    nh = (nrho + 1) // 2
    for half in range(2):
        q0 = rho_lo + nh * half
        q1 = min(rho_hi, q0 + nh)
        nq = (q1 - q0) * W
        up_q = pool.tile([16, 4 * nh * W], BF16, name=f"up_q{half}", tag="T_k_upq")
        for par in range(4):
            nc.scalar.dma_start(
                up_q[0:16, par * nq:(par + 1) * nq],
                up_pl[16 * par:16 * (par + 1),
                      (q0 - rho_lo) * W:(q0 - rho_lo) * W + nq])
        for a in range(2):
            for b in range(2):
                par = a * 2 + b
                lr = 1 + 2 * q0 + a - UP[0]
                nr = sum(1 for rho in range(q0, q1)
                         if 0 <= 2 * rho + a - UP[0] < up_rows)
                nc.vector.tensor_copy(
                    up_t[0:16, lr:lr + 2 * nr:2, 1 + b:1 + b + 128:2],
                    up_q[0:16, par * nq:par * nq + nr * W].rearrange(
                        "c (r w) -> c r w", w=W))

    tr_lo, tr_hi = UP_OUT
    for half in range(2):
        h0 = tr_lo + 32 * half
        h1 = h0 + 32
        upsa = pool.tile([128, 32 * 128], BF16, name=f"upsa{half}",
                         tag="T_c1b_upsa" if half == 0 else "T_cf1_c2o")
        for ty in range(3):
            for tx in range(3):
                t = ty * 3 + tx
                if t == 8:
                    continue
                nc.scalar.dma_start(
                    upsa[16 * t:16 * (t + 1), :].rearrange("c (r w) -> c r w", w=128),
                    up_t[:, h0 - UP[0] + ty:h0 - UP[0] + ty + 32, tx:tx + 128])
        for r0, r1 in chunks(h0, h1, 4):
            o0 = (r0 - h0) * 128
            nn_ = (r1 - r0) * 128
            ps = ps_conv.tile([64, nn_], F32, tag="ps_conv")
            nc.tensor.matmul(ps[0:64, :nn_], lhsT=WS["w_tr_a"][:, :],
                             rhs=upsa[:, o0:o0 + nn_], start=True, stop=False)
            nc.tensor.matmul(ps[0:64, :nn_], lhsT=WS["w_tr_b"][:, :],
                             rhs=up_t[0:16, r0 - UP[0] + 2:r1 - UP[0] + 2, 2:130],
                             start=False, stop=True)
            st = stage.tile([64, nn_], F32, tag="trstage")
            nc.scalar.activation(st[0:64, :nn_], ps[0:64, :nn_], AF.Identity,
                                 bias=WS["b_tr"][:, 0:1])
            nc.sync.dma_start(
                P["tradeoff_out"][:, (r0 - tr_lo) * 128:(r0 - tr_lo) * 128 + nn_],
                st[0:64, :nn_])

    for r0, r1 in chunks(*UP_OUT, 4):
        st = stage.tile([16, (r1 - r0) * 128], F32, tag="upstage")
        nc.vector.tensor_copy(
            st[0:16, :].rearrange("c (r w) -> c r w", w=128),
            up_t[:, 1 + r0 - UP[0]:1 + r1 - UP[0], 1:129])
        nc.gpsimd.dma_start(
            P["up_out"][:, (r0 - UP_OUT[0]) * 128:(r1 - UP_OUT[0]) * 128],
            st[0:16, :])

    for r0, r1 in chunks(0, 32, 8):
        nn_ = (r1 - r0) * W
        ps = ps_conv.tile([3, nn_], F32, tag="ps_conv")
        for dyi, dy in enumerate((-1, 0, 1)):
            nc.tensor.matmul(ps[0:3, :nn_],
                             lhsT=WS["w_fm"][:, 3 * dyi:3 * (dyi + 1)],
                             rhs=xs_rhs(r0, r1, dy),
                             start=(dyi == 0), stop=(dyi == 2))
        st = stage.tile([3, nn_], F32, tag="trstage")
        nc.scalar.activation(st[0:3, :nn_], ps[0:3, :nn_], AF.Identity,
                             bias=WS["b_fm"][:, 0:1])
        nc.gpsimd.dma_start(P["flow_out"][:, r0 * W:r1 * W], st[0:2, :nn_])
        nc.gpsimd.dma_start(P["mask_out"][:, r0 * W:r1 * W], st[2:3, :nn_])
    _sc.__exit__(None, None, None)

    ctx.close()


# ---------------------------------------------------------------------------
# Host-side sharding / gather


_NC_CACHE = None
_W_CACHE = {}


def _get_nc():
    global _NC_CACHE
    if _NC_CACHE is None:
        _NC_CACHE = build_nc()
    return _NC_CACHE


def build_f1stack(flow_b, flip):
    """Tap-shifted flow copies for the K-stacked convf1: [98, f1rows*W]."""
    v = np.asarray(flow_b, np.float32)
    if flip:
        v = v[:, ::-1, :]
    lo, hi = ROWS["f1out"]
    out = np.zeros((98, hi - lo, W), np.float32)
    for ty in range(7):
        for tx in range(7):
            t = ty * 7 + tx
            r0 = max(lo, -(ty - 3))
            r1 = min(hi, 64 - (ty - 3))
            c0 = max(0, -(tx - 3))
            c1 = min(W, W - (tx - 3))
            if r0 >= r1 or c0 >= c1:
                continue
            out[2 * t:2 * t + 2, r0 - lo:r1 - lo, c0:c1] = \
                v[:, r0 + ty - 3:r1 + ty - 3, c0 + tx - 3:c1 + tx - 3]
    return np.ascontiguousarray(out.reshape(98, -1)).astype(ml_dtypes.bfloat16)


def prepare_in_maps(inputs):
    w0 = prep_weights(inputs, flip=False)
    w1 = prep_weights(inputs, flip=True)
    sel = {
        0: (np.zeros((128, 1), np.float32), np.ones((128, 1), np.float32)),
        1: (np.ones((128, 1), np.float32), np.zeros((128, 1), np.float32)),
    }
    in_maps = []
    for c in range(N_CORES):
        b, h = c // 2, c % 2
        flip = h == 1

        def rows(a, key):
            lo, hi = ROWS[key]
            v = a[:, ::-1, :] if flip else a
            return np.ascontiguousarray(v[:, lo:hi, :], dtype=np.float32).reshape(a.shape[0], -1)

        x1 = inputs["x1"][b]
        v = x1[NET:]
        v = v[:, ::-1, :] if flip else v
        m = {"inp": np.ascontiguousarray(v, dtype=np.float32).reshape(128, HW)}
        m["net"] = rows(x1[:NET], "net")
        m["upf"] = rows(inputs["upfeat"][b], "net")
        m["corr"] = rows(inputs["corr"][b], "corr")
        m["f1stk"] = build_f1stack(inputs["flow"][b], flip)
        wd = w1 if flip else w0
        for k, v2 in wd.items():
            m[k] = v2
        m["sel0"], m["sel1"] = sel[h]
        in_maps.append(m)
    return in_maps


def assemble_outputs(results):
    B = 4
    tradeoff = np.zeros((B, 64, 128, 128), np.float32)
    up = np.zeros((B, 16, 128, 128), np.float32)
    flow_out = np.zeros((B, 2, 64, 64), np.float32)
    mask = np.zeros((B, 1, 64, 64), np.float32)
    for c in range(N_CORES):
        b, h = c // 2, c % 2
        r = results[c]
        tr = r["tradeoff_out"].reshape(64, 64, 128)
        u = r["up_out"].reshape(16, 64, 128)
        fo = r["flow_out"].reshape(2, 32, 64)
        mo = r["mask_out"].reshape(1, 32, 64)
        if h == 1:
            tr, u, fo, mo = tr[:, ::-1], u[:, ::-1], fo[:, ::-1], mo[:, ::-1]
            tradeoff[b, :, 64:128] = tr
            up[b, :, 64:128] = u
            flow_out[b, :, 32:64] = fo
            mask[b, :, 32:64] = mo
        else:
            tradeoff[b, :, 0:64] = tr
            up[b, :, 0:64] = u
            flow_out[b, :, 0:32] = fo
            mask[b, :, 0:32] = mo
    return (tradeoff, up, flow_out, mask)


def run(inputs, trace=False, **kw):
    nc = _get_nc()
    in_maps = prepare_in_maps(inputs)
    res = run_bass_kernel_spmd(nc, in_maps, core_ids=list(range(N_CORES)),
                               trace=trace, **kw)
    return assemble_outputs(res.results), res


def kernel(**inputs):
    outs, _ = run(inputs)
    return outs


# revision 27
# speedup vs baseline: 1.1198x; 1.0232x over previous
"""Trainium2 8-core Bass kernel for nn_AttAggFME.

Sharding: core c = (batch b=c//2, half h=c%2). Every core runs IDENTICAL
code with "top half" geometry; h=1 cores receive vertically flipped data
(host flips rows) and dy-tap-flipped conv weights, and their outputs are
flipped back on the host.

Per core: 2048 attention queries (own rows), motion encoder on own rows +
16 halo rows (halo comes free from host-sliced inputs), conv1..4 with halo
recompute. Two intra-pair AllGathers: V^T (full 4096 values for attn@V) and
16 halo rows of motion_fea_global before conv1. Cross-core row-order and
rank-offset asymmetries are handled with reverse-on-send plus host-supplied
0/1 selector masks.

All matmuls bf16 with f32 PSUM accumulation; softmax logits stay f32 in
PSUM, exp on ScalarE; denominators via ones-matmul (scores are computed
transposed [keys, queries] so attn@V needs no transposes).
"""

import numpy as np
import ml_dtypes

import concourse.bass as bass
import concourse.tile as tile
from concourse import mybir, bacc
from concourse.bass_utils import run_bass_kernel_spmd

F32 = mybir.dt.float32
BF16 = mybir.dt.bfloat16
AF = mybir.ActivationFunctionType

N_CORES = 8
H = W = 64
HW = H * W
D = 128
NET = 128
GROUPS = [[0, 1], [2, 3], [4, 5], [6, 7]]

# Row ranges (own-frame; every core is "top half": own rows [0:32))
ROWS = {
    "own":   (0, 32),
    "mf":    (0, 48),   # motion_fea (conv1 needs 16 halo rows)
    "mfg":   (0, 48),   # motion_fea_global (own 0:32 + partner halo 32:48)
    "c1o":   (0, 47),   # conv1 out
    "c2o":   (0, 43),   # conv2 out
    "c3o":   (0, 35),   # conv3 out
    "x":     (0, 34),   # conv4 out
    "cor":   (0, 49),   # convc2 out
    "c1out": (0, 50),   # convc1 out
    "f1out": (0, 50),   # convf1 out
    "flo":   (0, 49),   # convf2 out
    "corr":  (0, 50),   # corr input rows shipped
    "flow":  (0, 53),   # flow input rows shipped
    "net":   (0, 48),   # net / upfeat input rows shipped
}
RHO = (0, 33)       # deconv x-row parity range
UP = (0, 66)        # up rows stored
UP_OUT = (0, 64)    # up rows output / trade out rows

TAPS3 = [(ty - 1, tx - 1) for ty in range(3) for tx in range(3)]


def _nrows(key):
    lo, hi = ROWS[key]
    return hi - lo


PARAM_SPECS = [
    # activations (f32, own-frame)
    ("inp",  [128, HW], F32),
    ("net",  [128, _nrows("net") * W], F32),
    ("upf",  [128, _nrows("net") * W], F32),
    ("corr", [81, _nrows("corr") * W], F32),
    ("f1stk", [98, _nrows("f1out") * W], BF16),
    # weights (bf16) / biases (f32) / selectors
    ("w_qk", [128, 256], BF16),
    ("w_c1", [81, 256], BF16), ("b_c1a", [128, 1], F32), ("b_c1b", [128, 1], F32),
    ("w_c2", [128, 9 * 2 * 192], BF16), ("b_c2a", [128, 1], F32), ("b_c2b", [64, 1], F32),
    ("w_f1", [98, 128], BF16), ("b_f1", [128, 1], F32),
    ("w_f2", [128, 9 * 64], BF16), ("b_f2p", [128, 1], F32),
    ("w_cm", [128, 9 * 2 * 126], BF16), ("b_cm", [126, 1], F32),
    ("w_v", [128, 128], BF16),
    ("w_1", [128, 9 * 4 * 128], BF16), ("b_1", [128, 1], F32),
    ("w_2", [128, 9 * 96], BF16), ("b_2", [96, 1], F32),
    ("w_3", [96, 9 * 64], BF16), ("b_3", [64, 1], F32),
    ("w_4", [64, 9 * 32], BF16), ("b_4", [32, 1], F32),
    ("w_dc", [96, 3 * 64], BF16), ("b_dc", [64, 1], F32),
    ("w_tr_a", [128, 64], BF16), ("w_tr_b", [16, 64], BF16), ("b_tr", [64, 1], F32),
    ("w_fm", [96, 3 * 3], BF16), ("b_fm", [3, 1], F32),
    ("sel0", [128, 1], F32), ("sel1", [128, 1], F32),
]


# ---------------------------------------------------------------------------
# Host-side weight prep


def prep_weights(inp, flip):
    """All conv weights packed for the kernel; flip=True mirrors dy taps."""
    def bf(a):
        return np.ascontiguousarray(a, dtype=np.float32).astype(ml_dtypes.bfloat16)

    def col(a, n=None, off=0):
        a = np.asarray(a, np.float32).reshape(-1)
        n = n or a.shape[0]
        out = np.zeros((n, 1), np.float32)
        out[off:off + a.shape[0], 0] = a
        return out

    def fl(wt):  # [Cout, Cin, kh, kw] -> dy mirrored
        return wt[:, :, ::-1, :] if flip else wt

    w = {}
    qk = inp["att_to_qk_w"][:, :, 0, 0].astype(np.float64).T.copy()  # [128, 256]
    qk[:, :D] *= D ** -0.5
    w["w_qk"] = bf(qk)

    w["w_c1"] = bf(inp["convc1_w"][:, :, 0, 0].T)
    w["b_c1a"] = col(inp["convc1_b"][:128])
    w["b_c1b"] = col(inp["convc1_b"][128:])

    def conv3x3(wt):
        wt = fl(wt)
        Cout, Cin = wt.shape[:2]
        nkt = (Cin + 127) // 128
        K = 128 if nkt > 1 else Cin
        blocks = []
        for (dy, dx) in TAPS3:
            for kt in range(nkt):
                cs, ce = kt * 128, min(Cin, (kt + 1) * 128)
                blk = np.zeros((K, Cout), np.float64)
                blk[: ce - cs] = wt[:, cs:ce, dy + 1, dx + 1].T
                blocks.append(blk)
        return bf(np.concatenate(blocks, axis=1))

    w["w_c2"] = conv3x3(inp["convc2_w"])
    w["b_c2a"] = col(inp["convc2_b"][:128])
    w["b_c2b"] = col(inp["convc2_b"][128:])

    f1 = fl(inp["convf1_w"])
    lf1 = np.zeros((98, 128), np.float64)
    for ty in range(7):
        for tx in range(7):
            for ci in range(2):
                lf1[2 * (ty * 7 + tx) + ci] = f1[:, ci, ty, tx]
    w["w_f1"] = bf(lf1)
    w["b_f1"] = col(inp["convf1_b"])

    w["w_f2"] = conv3x3(inp["convf2_w"])
    w["b_f2p"] = col(inp["convf2_b"], n=128, off=64)

    w["w_cm"] = conv3x3(inp["conv_motion_w"])
    w["b_cm"] = col(inp["conv_motion_b"])

    gv = float(np.asarray(inp["gamma"]).reshape(-1)[0]) * inp["agg_to_v_w"][:, :, 0, 0]
    w["w_v"] = bf(gv.T)

    w["w_1"] = conv3x3(inp["conv1_w"]); w["b_1"] = col(inp["conv1_b"])
    w["w_2"] = conv3x3(inp["conv2_w"]); w["b_2"] = col(inp["conv2_b"])
    w["w_3"] = conv3x3(inp["conv3_w"]); w["b_3"] = col(inp["conv3_b"])
    w["w_4"] = conv3x3(inp["conv4_w"]); w["b_4"] = col(inp["conv4_b"])

    # deconv: ConvTranspose2d(k4,s2,p1) w [32ci,16co,4,4]; flip: kh -> 3-kh
    dw = inp["upfeat_w"].astype(np.float64)
    if flip:
        dw = dw[:, :, ::-1, :]
    ldc = np.zeros((3, 96, 64), np.float64)
    for dyi, dy in enumerate((-1, 0, 1)):
        for dxi, dx in enumerate((-1, 0, 1)):
            for a in range(2):
                kh = a + 1 - 2 * dy
                if not (0 <= kh < 4) or (a == 0 and dy not in (0, -1)) or (a == 1 and dy not in (0, 1)):
                    continue
                for b in range(2):
                    kw = b + 1 - 2 * dx
                    if not (0 <= kw < 4) or (b == 0 and dx not in (0, -1)) or (b == 1 and dx not in (0, 1)):
                        continue
                    ldc[dyi, dxi * 32:dxi * 32 + 32, (a * 2 + b) * 16:(a * 2 + b) * 16 + 16] = dw[:, :, kh, kw]
    w["w_dc"] = bf(ldc.transpose(1, 0, 2).reshape(96, 3 * 64))
    w["b_dc"] = col(np.tile(np.asarray(inp["upfeat_b"], np.float64), 4))

    tw = fl(inp["trade_w"]).astype(np.float64)
    la = np.zeros((128, 64), np.float64)
    lb = np.zeros((16, 64), np.float64)
    for ty in range(3):
        for tx in range(3):
            t = ty * 3 + tx
            blk = tw[:, :, ty, tx].T
            if t < 8:
                la[t * 16:(t + 1) * 16] = blk
            else:
                lb[:] = blk
    w["w_tr_a"] = bf(la)
    w["w_tr_b"] = bf(lb)
    w["b_tr"] = col(inp["trade_b"])

    fw = fl(inp["flow_w"]).astype(np.float64)
    mw = fl(inp["mask_w"]).astype(np.float64)
    lfm = np.zeros((3, 96, 3), np.float64)
    for dyi, dy in enumerate((-1, 0, 1)):
        for dxi, dx in enumerate((-1, 0, 1)):
            lfm[dyi, dxi * 32:dxi * 32 + 32, 0:2] = fw[:, :, dy + 1, dx + 1].T
            lfm[dyi, dxi * 32:dxi * 32 + 32, 2] = mw[0, :, dy + 1, dx + 1]
    w["w_fm"] = bf(lfm.transpose(1, 0, 2).reshape(96, 3 * 3))
    w["b_fm"] = col(np.concatenate([np.asarray(inp["flow_b"]), np.asarray(inp["mask_b"])]))
    return w


# ---------------------------------------------------------------------------
# Padded spatial SBUF tensors


class PadT:
    """SBUF tile [C, rtot, stride] with `pad` zeroed margin rows/cols; g0 =
    own-frame row of the first real row."""

    def __init__(self, nc, pool, name, C, key_or_range, pad, dtype=BF16, tag=None):
        g0, g1 = ROWS[key_or_range] if isinstance(key_or_range, str) else key_or_range
        self.nc, self.C, self.g0, self.rows, self.pad = nc, C, g0, g1 - g0, pad
        self.stride = W + 2 * pad
        self.rtot = self.rows + 2 * pad
        self.t = pool.tile([C, self.rtot, self.stride], dtype, name=name,
                           tag=tag or name)

    def zero_margins(self):
        nc, p = self.nc, self.pad
        if p == 0:
            return
        nc.gpsimd.memset(self.t[:, 0:p, :], 0.0)
        nc.gpsimd.memset(self.t[:, self.rtot - p:self.rtot, :], 0.0)
        nc.gpsimd.memset(self.t[:, p:p + self.rows, 0:p], 0.0)
        nc.gpsimd.memset(self.t[:, p:p + self.rows, self.stride - p:self.stride], 0.0)

    def ap(self, r_lo, r_hi, dy=0, dx=0, c_lo=0, c_hi=None):
        c_hi = self.C if c_hi is None else c_hi
        a = r_lo - self.g0 + self.pad + dy
        b = r_hi - self.g0 + self.pad + dy
        assert 0 <= a and b <= self.rtot, (r_lo, r_hi, dy, self.g0, self.rows)
        assert 0 <= self.pad + dx and dx <= self.pad
        return self.t[c_lo:c_hi, a:b, self.pad + dx:self.pad + dx + W]


def chunks(lo, hi, step):
    r = lo
    while r < hi:
        yield r, min(hi, r + step)
        r += step


# ---------------------------------------------------------------------------
# Graph build


def build_nc():
    nc = bacc.Bacc()
    P = {}
    for name, shape, dt in PARAM_SPECS:
        P[name] = nc.declare_dram_parameter(name, shape, dt, isOutput=False)
    P["tradeoff_out"] = nc.declare_dram_parameter("tradeoff_out", [64, 64 * 128], F32, isOutput=True)
    P["up_out"] = nc.declare_dram_parameter("up_out", [16, 64 * 128], F32, isOutput=True)
    P["flow_out"] = nc.declare_dram_parameter("flow_out", [2, 32 * W], F32, isOutput=True)
    P["mask_out"] = nc.declare_dram_parameter("mask_out", [1, 32 * W], F32, isOutput=True)

    with tile.TileContext(nc) as tc:
        _emit(nc, tc, P)
    nc.finalize()
    return nc


def _emit(nc, tc, P):
    from contextlib import ExitStack
    ctx = ExitStack()
    pool = ctx.enter_context(tc.tile_pool(name="main", bufs=1))
    stage = ctx.enter_context(tc.tile_pool(name="stage", bufs=3))
    small = ctx.enter_context(tc.tile_pool(name="small", bufs=2))
    small1 = ctx.enter_context(tc.tile_pool(name="small1", bufs=1))
    dram = ctx.enter_context(tc.tile_pool(name="dram", bufs=1, space="DRAM"))
    ps_conv = ctx.enter_context(tc.tile_pool(name="ps_conv", bufs=2, space="PSUM"))
    ps_scores = ctx.enter_context(tc.tile_pool(name="ps_scores", bufs=2, space="PSUM"))
    ps_agg = ctx.enter_context(tc.tile_pool(name="ps_agg", bufs=2, space="PSUM"))

    scope = nc.named_scope

    # ---- weights, emitted in order of first use --------------------------
    WS = {}

    def loadw(*names):
        for name in names:
            spec = next(s for s in PARAM_SPECS if s[0] == name)
            t = pool.tile(spec[1], spec[2], name=f"sb_{name}")
            nc.sync.dma_start(t[:], P[name][:])
            WS[name] = t

    def wslice(name, tap, kt, nkt, m_lo, m_hi, Cout, K=128):
        base = (tap * nkt + kt) * Cout
        return WS[name][0:K, base + m_lo:base + m_hi]

    # ---- input casts -----------------------------------------------------
    def load_cast_flat(dst, dram_p, C, total):
        for lo, hi in chunks(0, total, 1024):
            st = stage.tile([C, hi - lo], F32, tag="f32stage")
            nc.sync.dma_start(st[0:C, 0:hi - lo], dram_p[0:C, lo:hi])
            nc.vector.tensor_copy(dst[0:C, lo:hi], st[0:C, 0:hi - lo])

    def load_cast_padt(dst, dram_p, C, key):
        lo, hi = ROWS[key]
        for r0, r1 in chunks(lo, hi, 16):
            st = stage.tile([C, (r1 - r0) * W], F32, tag="f32stage")
            nc.sync.dma_start(st[0:C, 0:(r1 - r0) * W],
                              dram_p[0:C, (r0 - lo) * W:(r1 - lo) * W])
            nc.vector.tensor_copy(
                dst.ap(r0, r1, c_lo=0, c_hi=C),
                st[0:C, 0:(r1 - r0) * W].rearrange("c (r w) -> c r w", w=W))

    loadw("w_c1", "b_c1a", "b_c1b")
    crows = _nrows("corr")
    corr_bf = pool.tile([81, crows * W], BF16, name="corr_bf", tag="T_corr_dsum_xs")
    load_cast_flat(corr_bf, P["corr"], 81, crows * W)
    f1rows0 = _nrows("f1out")
    f1stack = pool.tile([98, f1rows0 * W], BF16, name="f1stack", tag="T_f1s_upl")
    nc.sync.dma_start(f1stack[:, :], P["f1stk"][:, :])

    loadw("w_f1", "b_f1", "w_c2", "b_c2a", "b_c2b")

    inp_bf = pool.tile([128, HW], BF16, name="inp_bf", tag="T_inp_xs2")
    load_cast_flat(inp_bf, P["inp"], 128, HW)

    loadw("w_f2", "b_f2p", "w_cm", "b_cm", "w_v", "w_qk", "sel0", "sel1",
          "w_1", "b_1", "w_2", "b_2", "w_3", "b_3", "w_4", "b_4",
          "w_dc", "b_dc", "w_tr_a", "w_tr_b", "b_tr", "w_fm", "b_fm")

    net_bf = PadT(nc, pool, "net_bf", 128, "net", 1)
    upf_bf = PadT(nc, pool, "upf_bf", 128, "net", 1)
    net_bf.zero_margins(); upf_bf.zero_margins()
    load_cast_padt(net_bf, P["net"], 128, "net")
    load_cast_padt(upf_bf, P["upf"], 128, "net")

    ones_col = pool.tile([128, 1], BF16, name="ones_col")
    nc.gpsimd.memset(ones_col[:, :], 1.0)
    ones_row = pool.tile([1, 128], BF16, name="ones_row")
    nc.gpsimd.memset(ones_row[:, :], 1.0)

    # ---- generic 3x3 conv ------------------------------------------------
    def conv3x3(wname, srcs, parts, out_lo, out_hi, dil, Cout, act="lrelu",
                kt_major=False):
        nkt = len(srcs)
        order = [(ti, kt) for kt in range(nkt) for ti in range(9)] if kt_major \
            else [(ti, kt) for ti in range(9) for kt in range(nkt)]
        for r0, r1 in chunks(out_lo, out_hi, 8):
            for m_lo, m_hi, dstt, poff, bias_ap in parts:
                mn = m_hi - m_lo
                ps = ps_conv.tile([poff + mn, (r1 - r0) * W], F32, tag="ps_conv")
                for oi, (ti, kt) in enumerate(order):
                    dy, dx = TAPS3[ti]
                    src, K = srcs[kt]
                    nc.tensor.matmul(
                        ps[poff:poff + mn, :],
                        lhsT=wslice(wname, ti, kt, nkt, m_lo, m_hi, Cout, K),
                        rhs=src.ap(r0, r1, dy * dil, dx * dil, 0, K),
                        start=(oi == 0), stop=(oi == len(order) - 1),
                        tile_position=(0, poff) if poff else None)
                o = dstt.ap(r0, r1, 0, 0, poff, poff + mn)
                if act == "lrelu":
                    nc.scalar.activation(o, ps[poff:poff + mn, :], AF.Prelu,
                                         bias=bias_ap, alpha=0.1)
                else:
                    nc.scalar.activation(o, ps[poff:poff + mn, :], AF.Identity,
                                         bias=bias_ap)

    # ---- motion encoder (phase A: rows feeding own V^T; phase B: halo) ---
    _sc = scope("motencA"); _sc.__enter__()
    c1out_a = PadT(nc, pool, "c1out_a", 128, "c1out", 1, tag="T_c1a_upt")
    c1out_b = PadT(nc, pool, "c1out_b", 128, "c1out", 1, tag="T_c1b_upsa")
    c1out_a.zero_margins(); c1out_b.zero_margins()

    def convc1(lo, hi):
        for r0, r1 in chunks(lo, hi, 8):
            off = (r0 - ROWS["corr"][0]) * W
            nn_ = (r1 - r0) * W
            for dstt, m_lo, bias in ((c1out_a, 0, "b_c1a"), (c1out_b, 128, "b_c1b")):
                ps = ps_conv.tile([128, nn_], F32, tag="ps_conv")
                nc.tensor.matmul(ps[:, :], lhsT=WS["w_c1"][:, m_lo:m_lo + 128],
                                 rhs=corr_bf[:, off:off + nn_], start=True, stop=True)
                nc.scalar.activation(dstt.ap(r0, r1), ps[:, :], AF.Prelu,
                                     bias=WS[bias][:, 0:1], alpha=0.1)

    f1_lo = ROWS["f1out"][0]
    f1s3d = f1stack[:, :].rearrange("c (r w) -> c r w", w=W)

    f1out = PadT(nc, pool, "f1out", 128, "f1out", 1, tag="T_f1o_c3o")
    f1out.zero_margins()

    def convf1(lo, hi):
        for r0, r1 in chunks(lo, hi, 8):
            ps = ps_conv.tile([128, (r1 - r0) * W], F32, tag="ps_conv")
            o0 = (r0 - ROWS["f1out"][0]) * W
            nc.tensor.matmul(ps[:, :], lhsT=WS["w_f1"][:, :],
                             rhs=f1stack[:, o0:o0 + (r1 - r0) * W],
                             start=True, stop=True)
            nc.scalar.activation(f1out.ap(r0, r1), ps[:, :], AF.Prelu,
                                 bias=WS["b_f1"][:, 0:1], alpha=0.1)

    cf0 = PadT(nc, pool, "cf0", 128, "cor", 1, tag="T_cf0_c1o")
    cf1 = PadT(nc, pool, "cf1", 128, "cor", 1, tag="T_cf1_c2o")
    cf0.zero_margins(); cf1.zero_margins()

    def convc2(lo, hi):
        conv3x3("w_c2", [(c1out_a, 128), (c1out_b, 128)],
                [(0, 128, cf0, 0, WS["b_c2a"][:, 0:1]),
                 (128, 192, cf1, 0, WS["b_c2b"][0:64, 0:1])],
                lo, hi, 1, 192)

    def convf2(lo, hi):
        for r0, r1 in chunks(lo, hi, 8):
            ps = ps_conv.tile([128, (r1 - r0) * W], F32, tag="ps_conv")
            for ti in range(9):
                dy, dx = TAPS3[ti]
                nc.tensor.matmul(ps[64:128, :],
                                 lhsT=wslice("w_f2", ti, 0, 1, 0, 64, 64),
                                 rhs=f1out.ap(r0, r1, dy, dx),
                                 start=(ti == 0), stop=(ti == 8),
                                 tile_position=(0, 64))
            nc.scalar.activation(cf1.ap(r0, r1, 0, 0, 64, 128), ps[64:128, :],
                                 AF.Prelu, bias=WS["b_f2p"][64:128, 0:1], alpha=0.1)

    mf = PadT(nc, pool, "mf", 128, "mf", 1)

    def convcm(lo, hi):
        conv3x3("w_cm", [(cf0, 128), (cf1, 128)],
                [(0, 126, mf, 0, WS["b_cm"][:, 0:1])], lo, hi, 1, 126)

    def mf_concat(lo, hi):
        nc.sync.dma_start(mf.ap(lo, hi, c_lo=126, c_hi=128),
                          f1s3d[48:50, lo - f1_lo:hi - f1_lo, 0:W])

    # phase A: everything needed for own-rows V^T
    convc1(0, 34)
    convf1(0, 34)
    convc2(0, 33)
    convf2(0, 33)
    mf.zero_margins()
    convcm(0, 32)
    mf_concat(0, 32)
    _sc.__exit__(None, None, None)

    # ---- V^T + collective #1 (reverse-on-send) ---------------------------
    _sc = scope("vt_coll"); _sc.__enter__()
    vt_all = pool.tile([128, 32 * 128], BF16, name="vt_all")
    vt_in = dram.tile([2048, 128], BF16, name="vt_in")
    vt_out = dram.tile([4096, 128], BF16, name="vt_out")
    mf_flat = pool.tile([128, 2048], BF16, name="mf_flat", tag="T_mff_vtmp_xo")
    nc.vector.tensor_copy(mf_flat[:, :].rearrange("c (r w) -> c r w", w=W),
                          mf.ap(0, 32))
    for jt in range(16):
        ps = ps_conv.tile([128, 128], F32, tag="ps_conv")
        nc.tensor.matmul(ps[:, :], lhsT=mf_flat[:, 128 * jt:128 * (jt + 1)],
                         rhs=WS["w_v"][:, :], start=True, stop=True)
        nc.vector.tensor_copy(vt_all[:, 128 * jt:128 * (jt + 1)], ps[:, :])
        for rr in range(2):
            r = 2 * jt + rr
            nc.sync.dma_start(vt_in[(31 - r) * 64:(31 - r) * 64 + 64, :],
                              vt_all[64 * rr:64 * (rr + 1), 128 * jt:128 * (jt + 1)])
    nc.gpsimd.collective_compute("AllGather", mybir.AluOpType.bypass,
                                 replica_groups=GROUPS,
                                 ins=[vt_in.opt()], outs=[vt_out.opt()])
    _sc.__exit__(None, None, None)

    # ---- q, k (fills the collective wait) --------------------------------
    _sc = scope("qk"); _sc.__enter__()
    k_bf = pool.tile([128, HW], BF16, name="k_bf", tag="T_k_upq")
    q_bf = pool.tile([128, 2048], BF16, name="q_bf", tag="T_q_hblk1")
    for lo, hi in chunks(0, HW, 512):
        ps = ps_conv.tile([128, 512], F32, tag="ps_conv")
        nc.tensor.matmul(ps[:, :], lhsT=WS["w_qk"][:, 128:256],
                         rhs=inp_bf[:, lo:hi], start=True, stop=True)
        nc.vector.tensor_copy(k_bf[:, lo:hi], ps[:, :])
    for lo, hi in chunks(0, 2048, 512):
        ps = ps_conv.tile([128, 512], F32, tag="ps_conv")
        nc.tensor.matmul(ps[:, :], lhsT=WS["w_qk"][:, 0:128],
                         rhs=inp_bf[:, lo:hi], start=True, stop=True)
        nc.vector.tensor_copy(q_bf[:, lo:hi], ps[:, :])
    _sc.__exit__(None, None, None)

    # ---- motion encoder phase B (halo rows; also fills collective wait) --
    _sc = scope("motencB"); _sc.__enter__()
    convc1(34, 50)
    convf1(34, 50)
    convc2(33, 49)
    convf2(33, 49)
    convcm(32, 48)
    mf_concat(32, 48)
    _sc.__exit__(None, None, None)

    # ---- collective #1 receive (mask-select, partner rows ascend mine) ---
    _sc = scope("vt_recv"); _sc.__enter__()
    selp1 = pool.tile([128, 16 * 128], BF16, name="selp1", tag="T_selp1")
    nc.sync.dma_start(
        vt_all[:, 2048:4096].rearrange("p (j d) -> p j d", d=128),
        vt_out[0:2048, :].rearrange("(j p) d -> p j d", p=128))
    nc.sync.dma_start(
        selp1[:, :].rearrange("p (j d) -> p j d", d=128),
        vt_out[2048:4096, :].rearrange("(j p) d -> p j d", p=128))
    nc.vector.tensor_scalar_mul(vt_all[:, 2048:4096], vt_all[:, 2048:4096],
                                WS["sel0"][:, 0:1])
    nc.vector.tensor_scalar_mul(selp1[:, :], selp1[:, :], WS["sel1"][:, 0:1])
    nc.vector.tensor_add(vt_all[:, 2048:4096], vt_all[:, 2048:4096], selp1[:, :])
    _sc.__exit__(None, None, None)

    # ---- attention (i-chunks in order 2,3,0,1; pipelined normalize) ------
    _sc = scope("attn"); _sc.__enter__()
    mfg = PadT(nc, pool, "mfg", 128, "mfg", 1, tag="T_cf1_c2o")
    mfg.zero_margins()

    aggs = {}
    dsbs = {}

    def jloop(ic):
        i0 = ic * 512
        agg_ps = ps_agg.tile([128, 512], F32, tag="ps_agg")
        aggs[ic] = agg_ps
        dsum = pool.tile([128, 1024], BF16, name=f"dsum{ic}", tag="T_corr_dsum_xs")
        for jg in range(16):
            sc_ps = ps_scores.tile([128, 1024], F32, tag="ps_sc")
            for u in range(2):
                j = 2 * jg + u
                nc.tensor.matmul(sc_ps[:, 512 * u:512 * (u + 1)],
                                 lhsT=k_bf[:, 128 * j:128 * (j + 1)],
                                 rhs=q_bf[:, i0:i0 + 512], start=True, stop=True)
            pb = small.tile([128, 1024], BF16, tag="probs")
            nc.scalar.activation(pb[:, :], sc_ps[:, :], AF.Exp)
            if jg == 0:
                nc.vector.tensor_copy(dsum[:, :], pb[:, :])
            else:
                nc.vector.tensor_add(dsum[:, :], dsum[:, :], pb[:, :])
            for u in range(2):
                j = 2 * jg + u
                nc.tensor.matmul(agg_ps[:, :],
                                 lhsT=vt_all[:, 128 * j:128 * (j + 1)],
                                 rhs=pb[:, 512 * u:512 * (u + 1)],
                                 start=(j == 0), stop=(j == 31),
                                 skip_group_check=True)
        dsb = small.tile([128, 512], BF16, tag="dsb")
        nc.vector.tensor_add(dsb[:, :], dsum[:, 0:512], dsum[:, 512:1024])
        dsbs[ic] = dsb

    def norm_rest(ic):
        agg_ps = aggs.pop(ic)
        dsb = dsbs.pop(ic)
        sums_ps = ps_conv.tile([1, 512], F32, tag="ps_conv")
        nc.tensor.matmul(sums_ps[0:1, :], lhsT=ones_col[:, :], rhs=dsb[:, :],
                         start=True, stop=True)
        recip = small1.tile([1, 512], F32, tag="recip")
        nc.vector.reciprocal(recip[0:1, :], sums_ps[0:1, :])
        recip_bf = small1.tile([1, 512], BF16, tag="recipbf")
        nc.vector.tensor_copy(recip_bf[0:1, :], recip[0:1, :])
        bc_ps = ps_conv.tile([128, 512], F32, tag="ps_conv")
        nc.tensor.matmul(bc_ps[:, :], lhsT=ones_row[0:1, :], rhs=recip_bf[0:1, :],
                         start=True, stop=True)
        bc_sb = small.tile([128, 512], F32, tag="nrm_f32")
        nc.scalar.copy(bc_sb[:, :], bc_ps[:, :])
        agg_sb = small.tile([128, 512], F32, tag="nrm_f32")
        nc.vector.tensor_mul(agg_sb[:, :], agg_ps[:, :], bc_sb[:, :])
        r0 = ic * 8
        nc.vector.tensor_add(mfg.ap(r0, r0 + 8),
                             agg_sb[:, :].rearrange("c (r w) -> c r w", w=W),
                             mf.ap(r0, r0 + 8))

    mfg_in = dram.tile([128, 16 * W], BF16, name="mfg_in")
    mfg_out = dram.tile([256, 16 * W], BF16, name="mfg_out")

    jloop(0)
    jloop(1)
    norm_rest(0)
    jloop(2)
    norm_rest(1)
    jloop(3)
    norm_rest(2)
    norm_rest(3)
    # ---- collective #2 launch (edge rows 16..31 ready) -------------------
    for r in range(16, 32):
        nc.sync.dma_start(mfg_in[:, (31 - r) * W:(31 - r) * W + W], mfg.ap(r, r + 1))
    nc.gpsimd.collective_compute("AllGather", mybir.AluOpType.bypass,
                                 replica_groups=GROUPS,
                                 ins=[mfg_in.opt()], outs=[mfg_out.opt()])
    _sc.__exit__(None, None, None)

    # ---- collective #2 receive -------------------------------------------
    _sc = scope("halo"); _sc.__enter__()
    hblk1 = pool.tile([128, 16 * W], BF16, name="hblk1", tag="T_selp1")
    nc.sync.dma_start(mfg.ap(32, 48), mfg_out[0:128, :].rearrange("c (r w) -> c r w", w=W))
    nc.sync.dma_start(hblk1[:, :], mfg_out[128:256, :])
    nc.vector.tensor_scalar_mul(mfg.ap(32, 48), mfg.ap(32, 48), WS["sel0"][:, 0:1])
    nc.vector.tensor_scalar_mul(hblk1[:, :], hblk1[:, :], WS["sel1"][:, 0:1])
    nc.vector.tensor_add(mfg.ap(32, 48), mfg.ap(32, 48),
                         hblk1[:, :].rearrange("c (r w) -> c r w", w=W))
    _sc.__exit__(None, None, None)

    # ---- conv stack ------------------------------------------------------
    _sc = scope("convs"); _sc.__enter__()
    c1o = PadT(nc, pool, "c1o", 128, "c1o", 4)
    c1o.zero_margins()
    conv3x3("w_1", [(net_bf, 128), (upf_bf, 128), (mf, 128), (mfg, 128)],
            [(0, 128, c1o, 0, WS["b_1"][:, 0:1])], *ROWS["c1o"], 1, 128,
            kt_major=True)
    c2o = PadT(nc, pool, "c2o", 96, "c2o", 8)
    c2o.zero_margins()
    conv3x3("w_2", [(c1o, 128)], [(0, 96, c2o, 0, WS["b_2"][:, 0:1])],
            *ROWS["c2o"], 4, 96)
    c3o = PadT(nc, pool, "c3o", 64, "c3o", 1)
    c3o.zero_margins()
    conv3x3("w_3", [(c2o, 96)], [(0, 64, c3o, 0, WS["b_3"][:, 0:1])],
            *ROWS["c3o"], 8, 64)
    xo = PadT(nc, pool, "xo", 32, "x", 1, tag="T_mff_vtmp_xo")
    xo.zero_margins()
    conv3x3("w_4", [(c3o, 64)], [(0, 32, xo, 0, WS["b_4"][:, 0:1])],
            *ROWS["x"], 1, 32)
    _sc.__exit__(None, None, None)

    # ---- tail: single x-stack, plane deconv, up, trade, flow/mask --------
    _sc = scope("tail"); _sc.__enter__()
    rho_lo, rho_hi = RHO
    nrho = rho_hi - rho_lo
    # xs_all[ (dx+1)*32+ci, r, c ] = x[ci, rho_lo-1+r, c+dx]; rows [-1, 34)
    xs_all = pool.tile([96, (nrho + 2) * W], BF16, name="xs_all",
                       tag="T_corr_dsum_xs")
    for dxi, dx in enumerate((-1, 0, 1)):
        for q0, q1 in chunks(rho_lo - 1, rho_hi + 1, 9):
            o0 = (q0 - rho_lo + 1) * W
            nc.sync.dma_start(
                xs_all[32 * dxi:32 * (dxi + 1),
                       o0:o0 + (q1 - q0) * W].rearrange("c (r w) -> c r w", w=W),
                xo.ap(q0, q1, 0, dx))

    def xs_rhs(r0, r1, dy):
        o = (r0 + dy + 1 - rho_lo) * W
        return xs_all[:, o:o + (r1 - r0) * W]

    # deconv -> parity-major planes up_pl [ (a*2+b)*16+co, rho*W ]
    up_pl = pool.tile([64, nrho * W], BF16, name="up_pl", tag="T_f1s_upl")
    for r0, r1 in chunks(rho_lo, rho_hi, 8):
        o0 = (r0 - rho_lo) * W
        nn_ = (r1 - r0) * W
        ps = ps_conv.tile([64, nn_], F32, tag="ps_conv")
        for dyi, dy in enumerate((-1, 0, 1)):
            nc.tensor.matmul(ps[0:64, :nn_],
                             lhsT=WS["w_dc"][:, 64 * dyi:64 * (dyi + 1)],
                             rhs=xs_rhs(r0, r1, dy),
                             start=(dyi == 0), stop=(dyi == 2))
        nc.scalar.activation(up_pl[0:64, o0:o0 + nn_], ps[0:64, :nn_], AF.Prelu,
                             bias=WS["b_dc"][:, 0:1], alpha=0.1)

    # up_t (interleaved + 1-px margins) via plane-extract DMA + DVE interleave
    up_rows = UP[1] - UP[0]
    up_t = pool.tile([16, up_rows + 2, 130], BF16, name="up_t", tag="T_c1a_upt")
    nc.vector.memset(up_t[:, 0:1, :], 0.0)
    nc.vector.memset(up_t[:, up_rows + 1:up_rows + 2, :], 0.0)
    nc.vector.memset(up_t[:, 1:up_rows + 1, 0:1], 0.0)
    nc.vector.memset(up_t[:, 1:up_rows + 1, 129:130], 0.0)
    nh = (nrho + 1) // 2
    for half in range(2):
        q0 = rho_lo + nh * half
        q1 = min(rho_hi, q0 + nh)
        nq = (q1 - q0) * W
        up_q = pool.tile([16, 4 * nh * W], BF16, name=f"up_q{half}", tag="T_k_upq")
        for par in range(4):
            nc.scalar.dma_start(
                up_q[0:16, par * nq:(par + 1) * nq],
                up_pl[16 * par:16 * (par + 1),
                      (q0 - rho_lo) * W:(q0 - rho_lo) * W + nq])
        for a in range(2):
            for b in range(2):
                par = a * 2 + b
                lr = 1 + 2 * q0 + a - UP[0]
                nr = sum(1 for rho in range(q0, q1)
                         if 0 <= 2 * rho + a - UP[0] < up_rows)
                dst = up_t[0:16, lr:lr + 2 * nr:2, 1 + b:1 + b + 128:2]
                src = up_q[0:16, par * nq:par * nq + nr * W].rearrange(
                    "c (r w) -> c r w", w=W)
                if par % 2 == 0:
                    nc.vector.tensor_copy(dst, src)
                else:
                    nc.scalar.activation(dst, src, AF.Copy)

    tr_lo, tr_hi = UP_OUT
    for half in range(2):
        h0 = tr_lo + 32 * half
        h1 = h0 + 32
        upsa = pool.tile([128, 32 * 128], BF16, name=f"upsa{half}",
                         tag="T_c1b_upsa" if half == 0 else "T_cf1_c2o")
        for ty in range(3):
            for tx in range(3):
                t = ty * 3 + tx
                if t == 8:
                    continue
                nc.scalar.dma_start(
                    upsa[16 * t:16 * (t + 1), :].rearrange("c (r w) -> c r w", w=128),
                    up_t[:, h0 - UP[0] + ty:h0 - UP[0] + ty + 32, tx:tx + 128])
        for r0, r1 in chunks(h0, h1, 4):
            o0 = (r0 - h0) * 128
            nn_ = (r1 - r0) * 128
            ps = ps_conv.tile([64, nn_], F32, tag="ps_conv")
            nc.tensor.matmul(ps[0:64, :nn_], lhsT=WS["w_tr_a"][:, :],
                             rhs=upsa[:, o0:o0 + nn_], start=True, stop=False)
            nc.tensor.matmul(ps[0:64, :nn_], lhsT=WS["w_tr_b"][:, :],
                             rhs=up_t[0:16, r0 - UP[0] + 2:r1 - UP[0] + 2, 2:130],
                             start=False, stop=True)
            st = stage.tile([64, nn_], F32, tag="trstage")
            nc.scalar.activation(st[0:64, :nn_], ps[0:64, :nn_], AF.Identity,
                                 bias=WS["b_tr"][:, 0:1])
            nc.sync.dma_start(
                P["tradeoff_out"][:, (r0 - tr_lo) * 128:(r0 - tr_lo) * 128 + nn_],
                st[0:64, :nn_])

    for r0, r1 in chunks(*UP_OUT, 4):
        st = stage.tile([16, (r1 - r0) * 128], F32, tag="upstage")
        nc.vector.tensor_copy(
            st[0:16, :].rearrange("c (r w) -> c r w", w=128),
            up_t[:, 1 + r0 - UP[0]:1 + r1 - UP[0], 1:129])
        nc.gpsimd.dma_start(
            P["up_out"][:, (r0 - UP_OUT[0]) * 128:(r1 - UP_OUT[0]) * 128],
            st[0:16, :])

    for r0, r1 in chunks(0, 32, 8):
        nn_ = (r1 - r0) * W
        ps = ps_conv.tile([3, nn_], F32, tag="ps_conv")
        for dyi, dy in enumerate((-1, 0, 1)):
            nc.tensor.matmul(ps[0:3, :nn_],
                             lhsT=WS["w_fm"][:, 3 * dyi:3 * (dyi + 1)],
                             rhs=xs_rhs(r0, r1, dy),
                             start=(dyi == 0), stop=(dyi == 2))
        st = stage.tile([3, nn_], F32, tag="trstage")
        nc.scalar.activation(st[0:3, :nn_], ps[0:3, :nn_], AF.Identity,
                             bias=WS["b_fm"][:, 0:1])
        nc.gpsimd.dma_start(P["flow_out"][:, r0 * W:r1 * W], st[0:2, :nn_])
        nc.gpsimd.dma_start(P["mask_out"][:, r0 * W:r1 * W], st[2:3, :nn_])
    _sc.__exit__(None, None, None)

    ctx.close()


# ---------------------------------------------------------------------------
# Host-side sharding / gather


_NC_CACHE = None
_W_CACHE = {}


def _get_nc():
    global _NC_CACHE
    if _NC_CACHE is None:
        _NC_CACHE = build_nc()
    return _NC_CACHE


def build_f1stack(flow_b, flip):
    """Tap-shifted flow copies for the K-stacked convf1: [98, f1rows*W]."""
    v = np.asarray(flow_b, np.float32)
    if flip:
        v = v[:, ::-1, :]
    lo, hi = ROWS["f1out"]
    out = np.zeros((98, hi - lo, W), np.float32)
    for ty in range(7):
        for tx in range(7):
            t = ty * 7 + tx
            r0 = max(lo, -(ty - 3))
            r1 = min(hi, 64 - (ty - 3))
            c0 = max(0, -(tx - 3))
            c1 = min(W, W - (tx - 3))
            if r0 >= r1 or c0 >= c1:
                continue
            out[2 * t:2 * t + 2, r0 - lo:r1 - lo, c0:c1] = \
                v[:, r0 + ty - 3:r1 + ty - 3, c0 + tx - 3:c1 + tx - 3]
    return np.ascontiguousarray(out.reshape(98, -1)).astype(ml_dtypes.bfloat16)


def prepare_in_maps(inputs):
    w0 = prep_weights(inputs, flip=False)
    w1 = prep_weights(inputs, flip=True)
    sel = {
        0: (np.zeros((128, 1), np.float32), np.ones((128, 1), np.float32)),
        1: (np.ones((128, 1), np.float32), np.zeros((128, 1), np.float32)),
    }
    in_maps = []
    for c in range(N_CORES):
        b, h = c // 2, c % 2
        flip = h == 1

        def rows(a, key):
            lo, hi = ROWS[key]
            v = a[:, ::-1, :] if flip else a
            return np.ascontiguousarray(v[:, lo:hi, :], dtype=np.float32).reshape(a.shape[0], -1)

        x1 = inputs["x1"][b]
        v = x1[NET:]
        v = v[:, ::-1, :] if flip else v
        m = {"inp": np.ascontiguousarray(v, dtype=np.float32).reshape(128, HW)}
        m["net"] = rows(x1[:NET], "net")
        m["upf"] = rows(inputs["upfeat"][b], "net")
        m["corr"] = rows(inputs["corr"][b], "corr")
        m["f1stk"] = build_f1stack(inputs["flow"][b], flip)
        wd = w1 if flip else w0
        for k, v2 in wd.items():
            m[k] = v2
        m["sel0"], m["sel1"] = sel[h]
        in_maps.append(m)
    return in_maps


def assemble_outputs(results):
    B = 4
    tradeoff = np.zeros((B, 64, 128, 128), np.float32)
    up = np.zeros((B, 16, 128, 128), np.float32)
    flow_out = np.zeros((B, 2, 64, 64), np.float32)
    mask = np.zeros((B, 1, 64, 64), np.float32)
    for c in range(N_CORES):
        b, h = c // 2, c % 2
        r = results[c]
        tr = r["tradeoff_out"].reshape(64, 64, 128)
        u = r["up_out"].reshape(16, 64, 128)
        fo = r["flow_out"].reshape(2, 32, 64)
        mo = r["mask_out"].reshape(1, 32, 64)
        if h == 1:
            tr, u, fo, mo = tr[:, ::-1], u[:, ::-1], fo[:, ::-1], mo[:, ::-1]
            tradeoff[b, :, 64:128] = tr
            up[b, :, 64:128] = u
            flow_out[b, :, 32:64] = fo
            mask[b, :, 32:64] = mo
        else:
            tradeoff[b, :, 0:64] = tr
            up[b, :, 0:64] = u
            flow_out[b, :, 0:32] = fo
            mask[b, :, 0:32] = mo
    return (tradeoff, up, flow_out, mask)


def run(inputs, trace=False, **kw):
    nc = _get_nc()
    in_maps = prepare_in_maps(inputs)
    res = run_bass_kernel_spmd(nc, in_maps, core_ids=list(range(N_CORES)),
                               trace=trace, **kw)
    return assemble_outputs(res.results), res


def kernel(**inputs):
    outs, _ = run(inputs)
    return outs


# revision 29
# speedup vs baseline: 1.3452x; 1.2013x over previous
"""Trainium2 8-core Bass kernel for nn_AttAggFME.

Sharding: core c = (batch b=c//2, half h=c%2). Every core runs IDENTICAL
code with "top half" geometry; h=1 cores receive vertically flipped data
(host flips rows) and dy-tap-flipped conv weights, and their outputs are
flipped back on the host.

Per core: 2048 attention queries (own rows), motion encoder on own rows +
16 halo rows (halo comes free from host-sliced inputs), conv1..4 with halo
recompute. Two intra-pair AllGathers: V^T (full 4096 values for attn@V) and
16 halo rows of motion_fea_global before conv1. Cross-core row-order and
rank-offset asymmetries are handled with reverse-on-send plus host-supplied
0/1 selector masks.

All matmuls bf16 with f32 PSUM accumulation; softmax logits stay f32 in
PSUM, exp on ScalarE; denominators via ones-matmul (scores are computed
transposed [keys, queries] so attn@V needs no transposes).
"""

import numpy as np
import ml_dtypes

import concourse.bass as bass
import concourse.tile as tile
from concourse import mybir, bacc
from concourse.bass_utils import run_bass_kernel_spmd

F32 = mybir.dt.float32
BF16 = mybir.dt.bfloat16
AF = mybir.ActivationFunctionType

N_CORES = 8
H = W = 64
HW = H * W
D = 128
NET = 128
GROUPS = [[0, 1], [2, 3], [4, 5], [6, 7]]

# Row ranges (own-frame; every core is "top half": own rows [0:32))
ROWS = {
    "own":   (0, 32),
    "mf":    (0, 48),   # motion_fea (conv1 needs 16 halo rows)
    "mfg":   (0, 48),   # motion_fea_global (own 0:32 + partner halo 32:48)
    "c1o":   (0, 47),   # conv1 out
    "c2o":   (0, 43),   # conv2 out
    "c3o":   (0, 35),   # conv3 out
    "x":     (0, 34),   # conv4 out
    "cor":   (0, 49),   # convc2 out
    "c1out": (0, 50),   # convc1 out
    "f1out": (0, 50),   # convf1 out
    "flo":   (0, 49),   # convf2 out
    "corr":  (0, 50),   # corr input rows shipped
    "flow":  (0, 53),   # flow input rows shipped
    "net":   (0, 48),   # net / upfeat input rows shipped
}
RHO = (0, 33)       # deconv x-row parity range
UP = (0, 66)        # up rows stored
UP_OUT = (0, 64)    # up rows output / trade out rows

TAPS3 = [(ty - 1, tx - 1) for ty in range(3) for tx in range(3)]


def _nrows(key):
    lo, hi = ROWS[key]
    return hi - lo


PARAM_SPECS = [
    # activations (f32, own-frame)
    ("inp",  [128, HW], F32),
    ("net",  [128, _nrows("net") * W], F32),
    ("upf",  [128, _nrows("net") * W], F32),
    ("corr", [81, _nrows("corr") * W], F32),
    ("f1stk", [98, _nrows("f1out") * W], BF16),
    # weights (bf16) / biases (f32) / selectors
    ("w_qk", [128, 256], BF16),
    ("w_c1", [81, 256], BF16), ("b_c1a", [128, 1], F32), ("b_c1b", [128, 1], F32),
    ("w_c2", [128, 9 * 2 * 192], BF16), ("b_c2a", [128, 1], F32), ("b_c2b", [64, 1], F32),
    ("w_f1", [98, 128], BF16), ("b_f1", [128, 1], F32),
    ("w_f2", [128, 9 * 64], BF16), ("b_f2p", [128, 1], F32),
    ("w_cm", [128, 9 * 2 * 126], BF16), ("b_cm", [126, 1], F32),
    ("w_v", [128, 128], BF16),
    ("w_1", [128, 9 * 4 * 128], BF16), ("b_1", [128, 1], F32),
    ("w_2", [128, 9 * 96], BF16), ("b_2", [96, 1], F32),
    ("w_3", [96, 9 * 64], BF16), ("b_3", [64, 1], F32),
    ("w_4", [64, 9 * 32], BF16), ("b_4", [32, 1], F32),
    ("w_dc", [96, 3 * 64], BF16), ("b_dc", [64, 1], F32),
    ("w_tr_a", [128, 64], BF16), ("w_tr_b", [16, 64], BF16), ("b_tr", [64, 1], F32),
    ("w_fm", [96, 3 * 3], BF16), ("b_fm", [3, 1], F32),
    ("sel0", [128, 1], F32), ("sel1", [128, 1], F32),
]


# ---------------------------------------------------------------------------
# Host-side weight prep


def prep_weights(inp, flip):
    """All conv weights packed for the kernel; flip=True mirrors dy taps."""
    def bf(a):
        return np.ascontiguousarray(a, dtype=np.float32).astype(ml_dtypes.bfloat16)

    def col(a, n=None, off=0):
        a = np.asarray(a, np.float32).reshape(-1)
        n = n or a.shape[0]
        out = np.zeros((n, 1), np.float32)
        out[off:off + a.shape[0], 0] = a
        return out

    def fl(wt):  # [Cout, Cin, kh, kw] -> dy mirrored
        return wt[:, :, ::-1, :] if flip else wt

    w = {}
    qk = inp["att_to_qk_w"][:, :, 0, 0].astype(np.float64).T.copy()  # [128, 256]
    qk[:, :D] *= D ** -0.5
    w["w_qk"] = bf(qk)

    w["w_c1"] = bf(inp["convc1_w"][:, :, 0, 0].T)
    w["b_c1a"] = col(inp["convc1_b"][:128])
    w["b_c1b"] = col(inp["convc1_b"][128:])

    def conv3x3(wt):
        wt = fl(wt)
        Cout, Cin = wt.shape[:2]
        nkt = (Cin + 127) // 128
        K = 128 if nkt > 1 else Cin
        blocks = []
        for (dy, dx) in TAPS3:
            for kt in range(nkt):
                cs, ce = kt * 128, min(Cin, (kt + 1) * 128)
                blk = np.zeros((K, Cout), np.float64)
                blk[: ce - cs] = wt[:, cs:ce, dy + 1, dx + 1].T
                blocks.append(blk)
        return bf(np.concatenate(blocks, axis=1))

    w["w_c2"] = conv3x3(inp["convc2_w"])
    w["b_c2a"] = col(inp["convc2_b"][:128])
    w["b_c2b"] = col(inp["convc2_b"][128:])

    f1 = fl(inp["convf1_w"])
    lf1 = np.zeros((98, 128), np.float64)
    for ty in range(7):
        for tx in range(7):
            for ci in range(2):
                lf1[2 * (ty * 7 + tx) + ci] = f1[:, ci, ty, tx]
    w["w_f1"] = bf(lf1)
    w["b_f1"] = col(inp["convf1_b"])

    w["w_f2"] = conv3x3(inp["convf2_w"])
    w["b_f2p"] = col(inp["convf2_b"], n=128, off=64)

    w["w_cm"] = conv3x3(inp["conv_motion_w"])
    w["b_cm"] = col(inp["conv_motion_b"])

    gv = float(np.asarray(inp["gamma"]).reshape(-1)[0]) * inp["agg_to_v_w"][:, :, 0, 0]
    w["w_v"] = bf(gv.T)

    w["w_1"] = conv3x3(inp["conv1_w"]); w["b_1"] = col(inp["conv1_b"])
    w["w_2"] = conv3x3(inp["conv2_w"]); w["b_2"] = col(inp["conv2_b"])
    w["w_3"] = conv3x3(inp["conv3_w"]); w["b_3"] = col(inp["conv3_b"])
    w["w_4"] = conv3x3(inp["conv4_w"]); w["b_4"] = col(inp["conv4_b"])

    # deconv: ConvTranspose2d(k4,s2,p1) w [32ci,16co,4,4]; flip: kh -> 3-kh
    dw = inp["upfeat_w"].astype(np.float64)
    if flip:
        dw = dw[:, :, ::-1, :]
    ldc = np.zeros((3, 96, 64), np.float64)
    for dyi, dy in enumerate((-1, 0, 1)):
        for dxi, dx in enumerate((-1, 0, 1)):
            for a in range(2):
                kh = a + 1 - 2 * dy
                if not (0 <= kh < 4) or (a == 0 and dy not in (0, -1)) or (a == 1 and dy not in (0, 1)):
                    continue
                for b in range(2):
                    kw = b + 1 - 2 * dx
                    if not (0 <= kw < 4) or (b == 0 and dx not in (0, -1)) or (b == 1 and dx not in (0, 1)):
                        continue
                    ldc[dyi, dxi * 32:dxi * 32 + 32, (a * 2 + b) * 16:(a * 2 + b) * 16 + 16] = dw[:, :, kh, kw]
    w["w_dc"] = bf(ldc.transpose(1, 0, 2).reshape(96, 3 * 64))
    w["b_dc"] = col(np.tile(np.asarray(inp["upfeat_b"], np.float64), 4))

    tw = fl(inp["trade_w"]).astype(np.float64)
    la = np.zeros((128, 64), np.float64)
    lb = np.zeros((16, 64), np.float64)
    for ty in range(3):
        for tx in range(3):
            t = ty * 3 + tx
            blk = tw[:, :, ty, tx].T
            if t < 8:
                la[t * 16:(t + 1) * 16] = blk
            else:
                lb[:] = blk
    w["w_tr_a"] = bf(la)
    w["w_tr_b"] = bf(lb)
    w["b_tr"] = col(inp["trade_b"])

    fw = fl(inp["flow_w"]).astype(np.float64)
    mw = fl(inp["mask_w"]).astype(np.float64)
    lfm = np.zeros((3, 96, 3), np.float64)
    for dyi, dy in enumerate((-1, 0, 1)):
        for dxi, dx in enumerate((-1, 0, 1)):
            lfm[dyi, dxi * 32:dxi * 32 + 32, 0:2] = fw[:, :, dy + 1, dx + 1].T
            lfm[dyi, dxi * 32:dxi * 32 + 32, 2] = mw[0, :, dy + 1, dx + 1]
    w["w_fm"] = bf(lfm.transpose(1, 0, 2).reshape(96, 3 * 3))
    w["b_fm"] = col(np.concatenate([np.asarray(inp["flow_b"]), np.asarray(inp["mask_b"])]))
    return w


# ---------------------------------------------------------------------------
# Padded spatial SBUF tensors


class PadT:
    """SBUF tile [C, rtot, stride] with `pad` zeroed margin rows/cols; g0 =
    own-frame row of the first real row."""

    def __init__(self, nc, pool, name, C, key_or_range, pad, dtype=BF16, tag=None):
        g0, g1 = ROWS[key_or_range] if isinstance(key_or_range, str) else key_or_range
        self.nc, self.C, self.g0, self.rows, self.pad = nc, C, g0, g1 - g0, pad
        self.stride = W + 2 * pad
        self.rtot = self.rows + 2 * pad
        self.t = pool.tile([C, self.rtot, self.stride], dtype, name=name,
                           tag=tag or name)

    def zero_margins(self):
        nc, p = self.nc, self.pad
        if p == 0:
            return
        nc.gpsimd.memset(self.t[:, 0:p, :], 0.0)
        nc.gpsimd.memset(self.t[:, self.rtot - p:self.rtot, :], 0.0)
        nc.gpsimd.memset(self.t[:, p:p + self.rows, 0:p], 0.0)
        nc.gpsimd.memset(self.t[:, p:p + self.rows, self.stride - p:self.stride], 0.0)

    def ap(self, r_lo, r_hi, dy=0, dx=0, c_lo=0, c_hi=None):
        c_hi = self.C if c_hi is None else c_hi
        a = r_lo - self.g0 + self.pad + dy
        b = r_hi - self.g0 + self.pad + dy
        assert 0 <= a and b <= self.rtot, (r_lo, r_hi, dy, self.g0, self.rows)
        assert 0 <= self.pad + dx and dx <= self.pad
        return self.t[c_lo:c_hi, a:b, self.pad + dx:self.pad + dx + W]


def chunks(lo, hi, step):
    r = lo
    while r < hi:
        yield r, min(hi, r + step)
        r += step


# ---------------------------------------------------------------------------
# Graph build


def build_nc():
    nc = bacc.Bacc()
    P = {}
    for name, shape, dt in PARAM_SPECS:
        P[name] = nc.declare_dram_parameter(name, shape, dt, isOutput=False)
    P["tradeoff_out"] = nc.declare_dram_parameter("tradeoff_out", [64, 64 * 128], F32, isOutput=True)
    P["up_out"] = nc.declare_dram_parameter("up_out", [16, 64 * 128], F32, isOutput=True)
    P["flow_out"] = nc.declare_dram_parameter("flow_out", [2, 32 * W], F32, isOutput=True)
    P["mask_out"] = nc.declare_dram_parameter("mask_out", [1, 32 * W], F32, isOutput=True)

    with tile.TileContext(nc) as tc:
        _emit(nc, tc, P)
    nc.finalize()
    return nc


def _emit(nc, tc, P):
    from contextlib import ExitStack
    ctx = ExitStack()
    pool = ctx.enter_context(tc.tile_pool(name="main", bufs=1))
    stage = ctx.enter_context(tc.tile_pool(name="stage", bufs=3))
    small = ctx.enter_context(tc.tile_pool(name="small", bufs=2))
    small1 = ctx.enter_context(tc.tile_pool(name="small1", bufs=1))
    dram = ctx.enter_context(tc.tile_pool(name="dram", bufs=1, space="DRAM"))
    ps_conv = ctx.enter_context(tc.tile_pool(name="ps_conv", bufs=2, space="PSUM"))
    ps_scores = ctx.enter_context(tc.tile_pool(name="ps_scores", bufs=2, space="PSUM"))
    ps_agg = ctx.enter_context(tc.tile_pool(name="ps_agg", bufs=2, space="PSUM"))

    scope = nc.named_scope

    # ---- weights, emitted in order of first use --------------------------
    WS = {}

    def loadw(*names):
        for name in names:
            spec = next(s for s in PARAM_SPECS if s[0] == name)
            t = pool.tile(spec[1], spec[2], name=f"sb_{name}")
            nc.sync.dma_start(t[:], P[name][:])
            WS[name] = t

    def wslice(name, tap, kt, nkt, m_lo, m_hi, Cout, K=128):
        base = (tap * nkt + kt) * Cout
        return WS[name][0:K, base + m_lo:base + m_hi]

    # ---- input casts -----------------------------------------------------
    def load_cast_flat(dst, dram_p, C, total):
        for lo, hi in chunks(0, total, 1024):
            st = stage.tile([C, hi - lo], F32, tag="f32stage")
            nc.sync.dma_start(st[0:C, 0:hi - lo], dram_p[0:C, lo:hi])
            nc.vector.tensor_copy(dst[0:C, lo:hi], st[0:C, 0:hi - lo])

    def load_cast_padt(dst, dram_p, C, key):
        lo, hi = ROWS[key]
        for r0, r1 in chunks(lo, hi, 16):
            st = stage.tile([C, (r1 - r0) * W], F32, tag="f32stage")
            nc.sync.dma_start(st[0:C, 0:(r1 - r0) * W],
                              dram_p[0:C, (r0 - lo) * W:(r1 - lo) * W])
            nc.vector.tensor_copy(
                dst.ap(r0, r1, c_lo=0, c_hi=C),
                st[0:C, 0:(r1 - r0) * W].rearrange("c (r w) -> c r w", w=W))

    loadw("w_c1", "b_c1a", "b_c1b")
    crows = _nrows("corr")
    corr_bf = pool.tile([81, crows * W], BF16, name="corr_bf", tag="T_corr_dsum_xs")
    load_cast_flat(corr_bf, P["corr"], 81, crows * W)
    f1rows0 = _nrows("f1out")
    f1stack = pool.tile([98, f1rows0 * W], BF16, name="f1stack", tag="T_f1s_upl")
    nc.sync.dma_start(f1stack[:, :], P["f1stk"][:, :])

    loadw("w_f1", "b_f1", "w_c2", "b_c2a", "b_c2b")

    inp_bf = pool.tile([128, HW], BF16, name="inp_bf", tag="T_inp_xs2")
    load_cast_flat(inp_bf, P["inp"], 128, HW)

    loadw("w_f2", "b_f2p", "w_cm", "b_cm", "w_v", "w_qk", "sel0", "sel1",
          "w_1", "b_1", "w_2", "b_2", "w_3", "b_3", "w_4", "b_4",
          "w_dc", "b_dc", "w_tr_a", "w_tr_b", "b_tr", "w_fm", "b_fm")

    net_bf = PadT(nc, pool, "net_bf", 128, "net", 1)
    upf_bf = PadT(nc, pool, "upf_bf", 128, "net", 1)
    net_bf.zero_margins(); upf_bf.zero_margins()
    load_cast_padt(net_bf, P["net"], 128, "net")
    load_cast_padt(upf_bf, P["upf"], 128, "net")

    ones_col = pool.tile([128, 1], BF16, name="ones_col")
    nc.gpsimd.memset(ones_col[:, :], 1.0)
    ones_row = pool.tile([1, 128], BF16, name="ones_row")
    nc.gpsimd.memset(ones_row[:, :], 1.0)

    # ---- generic 3x3 conv ------------------------------------------------
    def conv3x3(wname, srcs, parts, out_lo, out_hi, dil, Cout, act="lrelu",
                kt_major=False):
        nkt = len(srcs)
        order = [(ti, kt) for kt in range(nkt) for ti in range(9)] if kt_major \
            else [(ti, kt) for ti in range(9) for kt in range(nkt)]
        for r0, r1 in chunks(out_lo, out_hi, 8):
            for m_lo, m_hi, dstt, poff, bias_ap in parts:
                mn = m_hi - m_lo
                ps = ps_conv.tile([poff + mn, (r1 - r0) * W], F32, tag="ps_conv")
                for oi, (ti, kt) in enumerate(order):
                    dy, dx = TAPS3[ti]
                    src, K = srcs[kt]
                    nc.tensor.matmul(
                        ps[poff:poff + mn, :],
                        lhsT=wslice(wname, ti, kt, nkt, m_lo, m_hi, Cout, K),
                        rhs=src.ap(r0, r1, dy * dil, dx * dil, 0, K),
                        start=(oi == 0), stop=(oi == len(order) - 1),
                        tile_position=(0, poff) if poff else None)
                o = dstt.ap(r0, r1, 0, 0, poff, poff + mn)
                if act == "lrelu":
                    nc.scalar.activation(o, ps[poff:poff + mn, :], AF.Prelu,
                                         bias=bias_ap, alpha=0.1)
                else:
                    nc.scalar.activation(o, ps[poff:poff + mn, :], AF.Identity,
                                         bias=bias_ap)

    # ---- motion encoder (phase A: rows feeding own V^T; phase B: halo) ---
    _sc = scope("motencA"); _sc.__enter__()
    c1out_a = PadT(nc, pool, "c1out_a", 128, "c1out", 1, tag="T_c1a_upt")
    c1out_b = PadT(nc, pool, "c1out_b", 128, "c1out", 1, tag="T_c1b_upsa")
    c1out_a.zero_margins(); c1out_b.zero_margins()

    def convc1(lo, hi):
        for r0, r1 in chunks(lo, hi, 8):
            off = (r0 - ROWS["corr"][0]) * W
            nn_ = (r1 - r0) * W
            for dstt, m_lo, bias in ((c1out_a, 0, "b_c1a"), (c1out_b, 128, "b_c1b")):
                ps = ps_conv.tile([128, nn_], F32, tag="ps_conv")
                nc.tensor.matmul(ps[:, :], lhsT=WS["w_c1"][:, m_lo:m_lo + 128],
                                 rhs=corr_bf[:, off:off + nn_], start=True, stop=True)
                nc.scalar.activation(dstt.ap(r0, r1), ps[:, :], AF.Prelu,
                                     bias=WS[bias][:, 0:1], alpha=0.1)

    f1_lo = ROWS["f1out"][0]
    f1s3d = f1stack[:, :].rearrange("c (r w) -> c r w", w=W)

    f1out = PadT(nc, pool, "f1out", 128, "f1out", 1, tag="T_f1o_c3o")
    f1out.zero_margins()

    def convf1(lo, hi):
        for r0, r1 in chunks(lo, hi, 8):
            ps = ps_conv.tile([128, (r1 - r0) * W], F32, tag="ps_conv")
            o0 = (r0 - ROWS["f1out"][0]) * W
            nc.tensor.matmul(ps[:, :], lhsT=WS["w_f1"][:, :],
                             rhs=f1stack[:, o0:o0 + (r1 - r0) * W],
                             start=True, stop=True)
            nc.scalar.activation(f1out.ap(r0, r1), ps[:, :], AF.Prelu,
                                 bias=WS["b_f1"][:, 0:1], alpha=0.1)

    cf0 = PadT(nc, pool, "cf0", 128, "cor", 1, tag="T_cf0_c1o")
    cf1 = PadT(nc, pool, "cf1", 128, "cor", 1, tag="T_cf1_c2o")
    cf0.zero_margins(); cf1.zero_margins()

    def convc2(lo, hi):
        conv3x3("w_c2", [(c1out_a, 128), (c1out_b, 128)],
                [(0, 128, cf0, 0, WS["b_c2a"][:, 0:1]),
                 (128, 192, cf1, 0, WS["b_c2b"][0:64, 0:1])],
                lo, hi, 1, 192)

    def convf2(lo, hi):
        for r0, r1 in chunks(lo, hi, 8):
            ps = ps_conv.tile([128, (r1 - r0) * W], F32, tag="ps_conv")
            for ti in range(9):
                dy, dx = TAPS3[ti]
                nc.tensor.matmul(ps[64:128, :],
                                 lhsT=wslice("w_f2", ti, 0, 1, 0, 64, 64),
                                 rhs=f1out.ap(r0, r1, dy, dx),
                                 start=(ti == 0), stop=(ti == 8),
                                 tile_position=(0, 64))
            nc.scalar.activation(cf1.ap(r0, r1, 0, 0, 64, 128), ps[64:128, :],
                                 AF.Prelu, bias=WS["b_f2p"][64:128, 0:1], alpha=0.1)

    mf = PadT(nc, pool, "mf", 128, "mf", 1)

    def convcm(lo, hi):
        conv3x3("w_cm", [(cf0, 128), (cf1, 128)],
                [(0, 126, mf, 0, WS["b_cm"][:, 0:1])], lo, hi, 1, 126)

    def mf_concat(lo, hi):
        nc.sync.dma_start(mf.ap(lo, hi, c_lo=126, c_hi=128),
                          f1s3d[48:50, lo - f1_lo:hi - f1_lo, 0:W])

    # phase A: everything needed for own-rows V^T
    convc1(0, 34)
    convf1(0, 34)
    convc2(0, 33)
    convf2(0, 33)
    mf.zero_margins()
    convcm(0, 32)
    mf_concat(0, 32)
    _sc.__exit__(None, None, None)

    # ---- V^T + collective #1 (reverse-on-send) ---------------------------
    _sc = scope("vt_coll"); _sc.__enter__()
    vt_all = pool.tile([128, 32 * 128], BF16, name="vt_all")
    vt_in = dram.tile([2048, 128], BF16, name="vt_in")
    vt_out = dram.tile([4096, 128], BF16, name="vt_out")
    mf_flat = pool.tile([128, 2048], BF16, name="mf_flat", tag="T_mff_vtmp_xo")
    nc.vector.tensor_copy(mf_flat[:, :].rearrange("c (r w) -> c r w", w=W),
                          mf.ap(0, 32))
    for jt in range(16):
        ps = ps_conv.tile([128, 128], F32, tag="ps_conv")
        nc.tensor.matmul(ps[:, :], lhsT=mf_flat[:, 128 * jt:128 * (jt + 1)],
                         rhs=WS["w_v"][:, :], start=True, stop=True)
        nc.vector.tensor_copy(vt_all[:, 128 * jt:128 * (jt + 1)], ps[:, :])
        for rr in range(2):
            r = 2 * jt + rr
            nc.sync.dma_start(vt_in[(31 - r) * 64:(31 - r) * 64 + 64, :],
                              vt_all[64 * rr:64 * (rr + 1), 128 * jt:128 * (jt + 1)])
    nc.gpsimd.collective_compute("AllGather", mybir.AluOpType.bypass,
                                 replica_groups=GROUPS,
                                 ins=[vt_in.opt()], outs=[vt_out.opt()])
    _sc.__exit__(None, None, None)

    # ---- q, k (fills the collective wait) --------------------------------
    _sc = scope("qk"); _sc.__enter__()
    k_bf = pool.tile([128, HW], BF16, name="k_bf", tag="T_k_upq")
    q_bf = pool.tile([128, 2048], BF16, name="q_bf", tag="T_q_hblk1")
    for lo, hi in chunks(0, HW, 512):
        ps = ps_conv.tile([128, 512], F32, tag="ps_conv")
        nc.tensor.matmul(ps[:, :], lhsT=WS["w_qk"][:, 128:256],
                         rhs=inp_bf[:, lo:hi], start=True, stop=True)
        nc.vector.tensor_copy(k_bf[:, lo:hi], ps[:, :])
    for lo, hi in chunks(0, 2048, 512):
        ps = ps_conv.tile([128, 512], F32, tag="ps_conv")
        nc.tensor.matmul(ps[:, :], lhsT=WS["w_qk"][:, 0:128],
                         rhs=inp_bf[:, lo:hi], start=True, stop=True)
        nc.vector.tensor_copy(q_bf[:, lo:hi], ps[:, :])
    _sc.__exit__(None, None, None)

    # ---- motion encoder phase B (halo rows; also fills collective wait) --
    _sc = scope("motencB"); _sc.__enter__()
    convc1(34, 50)
    convf1(34, 50)
    convc2(33, 49)
    convf2(33, 49)
    convcm(32, 48)
    mf_concat(32, 48)
    _sc.__exit__(None, None, None)

    # ---- collective #1 receive (mask-select, partner rows ascend mine) ---
    _sc = scope("vt_recv"); _sc.__enter__()
    selp1 = pool.tile([128, 16 * 128], BF16, name="selp1", tag="T_selp1")
    nc.sync.dma_start(
        vt_all[:, 2048:4096].rearrange("p (j d) -> p j d", d=128),
        vt_out[0:2048, :].rearrange("(j p) d -> p j d", p=128))
    nc.sync.dma_start(
        selp1[:, :].rearrange("p (j d) -> p j d", d=128),
        vt_out[2048:4096, :].rearrange("(j p) d -> p j d", p=128))
    nc.vector.tensor_scalar_mul(vt_all[:, 2048:4096], vt_all[:, 2048:4096],
                                WS["sel0"][:, 0:1])
    nc.vector.tensor_scalar_mul(selp1[:, :], selp1[:, :], WS["sel1"][:, 0:1])
    nc.vector.tensor_add(vt_all[:, 2048:4096], vt_all[:, 2048:4096], selp1[:, :])
    _sc.__exit__(None, None, None)

    # ---- attention (i-chunks in order 2,3,0,1; pipelined normalize) ------
    _sc = scope("attn"); _sc.__enter__()
    mfg = PadT(nc, pool, "mfg", 128, "mfg", 1, tag="T_cf1_c2o")
    mfg.zero_margins()

    aggs = {}
    dsbs = {}

    def jloop(ic):
        i0 = ic * 512
        agg_ps = ps_agg.tile([128, 512], F32, tag="ps_agg")
        aggs[ic] = agg_ps
        dsum = pool.tile([128, 1024], BF16, name=f"dsum{ic}", tag="T_corr_dsum_xs")
        for jg in range(16):
            sc_ps = ps_scores.tile([128, 1024], F32, tag="ps_sc")
            for u in range(2):
                j = 2 * jg + u
                nc.tensor.matmul(sc_ps[:, 512 * u:512 * (u + 1)],
                                 lhsT=k_bf[:, 128 * j:128 * (j + 1)],
                                 rhs=q_bf[:, i0:i0 + 512], start=True, stop=True)
            pb = small.tile([128, 1024], BF16, tag="probs", bufs=3)
            nc.scalar.activation(pb[:, :], sc_ps[:, :], AF.Exp)
            if jg == 0:
                nc.vector.tensor_copy(dsum[:, :], pb[:, :])
            else:
                nc.vector.tensor_add(dsum[:, :], dsum[:, :], pb[:, :])
            for u in range(2):
                j = 2 * jg + u
                nc.tensor.matmul(agg_ps[:, :],
                                 lhsT=vt_all[:, 128 * j:128 * (j + 1)],
                                 rhs=pb[:, 512 * u:512 * (u + 1)],
                                 start=(j == 0), stop=(j == 31),
                                 skip_group_check=True)
        dsb = small.tile([128, 512], BF16, tag="dsb")
        nc.vector.tensor_add(dsb[:, :], dsum[:, 0:512], dsum[:, 512:1024])
        dsbs[ic] = dsb

    def norm_rest(ic):
        agg_ps = aggs.pop(ic)
        dsb = dsbs.pop(ic)
        sums_ps = ps_conv.tile([1, 512], F32, tag="ps_conv")
        nc.tensor.matmul(sums_ps[0:1, :], lhsT=ones_col[:, :], rhs=dsb[:, :],
                         start=True, stop=True)
        recip = small1.tile([1, 512], F32, tag="recip")
        nc.vector.reciprocal(recip[0:1, :], sums_ps[0:1, :])
        recip_bf = small1.tile([1, 512], BF16, tag="recipbf")
        nc.vector.tensor_copy(recip_bf[0:1, :], recip[0:1, :])
        bc_ps = ps_conv.tile([128, 512], F32, tag="ps_conv")
        nc.tensor.matmul(bc_ps[:, :], lhsT=ones_row[0:1, :], rhs=recip_bf[0:1, :],
                         start=True, stop=True)
        bc_sb = small.tile([128, 512], F32, tag="nrm_f32")
        nc.scalar.copy(bc_sb[:, :], bc_ps[:, :])
        agg_sb = small.tile([128, 512], F32, tag="nrm_f32")
        nc.vector.tensor_mul(agg_sb[:, :], agg_ps[:, :], bc_sb[:, :])
        r0 = ic * 8
        nc.vector.tensor_add(mfg.ap(r0, r0 + 8),
                             agg_sb[:, :].rearrange("c (r w) -> c r w", w=W),
                             mf.ap(r0, r0 + 8))

    mfg_in = dram.tile([128, 16 * W], BF16, name="mfg_in")
    mfg_out = dram.tile([256, 16 * W], BF16, name="mfg_out")

    jloop(0)
    jloop(1)
    norm_rest(0)
    jloop(2)
    norm_rest(1)
    jloop(3)
    norm_rest(2)
    norm_rest(3)
    # ---- collective #2 launch (edge rows 16..31 ready) -------------------
    for r in range(16, 32):
        nc.sync.dma_start(mfg_in[:, (31 - r) * W:(31 - r) * W + W], mfg.ap(r, r + 1))
    nc.gpsimd.collective_compute("AllGather", mybir.AluOpType.bypass,
                                 replica_groups=GROUPS,
                                 ins=[mfg_in.opt()], outs=[mfg_out.opt()])
    _sc.__exit__(None, None, None)

    # ---- collective #2 receive -------------------------------------------
    _sc = scope("halo"); _sc.__enter__()
    hblk1 = pool.tile([128, 16 * W], BF16, name="hblk1", tag="T_selp1")
    nc.sync.dma_start(mfg.ap(32, 48), mfg_out[0:128, :].rearrange("c (r w) -> c r w", w=W))
    nc.sync.dma_start(hblk1[:, :], mfg_out[128:256, :])
    nc.vector.tensor_scalar_mul(mfg.ap(32, 48), mfg.ap(32, 48), WS["sel0"][:, 0:1])
    nc.vector.tensor_scalar_mul(hblk1[:, :], hblk1[:, :], WS["sel1"][:, 0:1])
    nc.vector.tensor_add(mfg.ap(32, 48), mfg.ap(32, 48),
                         hblk1[:, :].rearrange("c (r w) -> c r w", w=W))
    _sc.__exit__(None, None, None)

    # ---- conv stack ------------------------------------------------------
    _sc = scope("convs"); _sc.__enter__()
    c1o = PadT(nc, pool, "c1o", 128, "c1o", 4)
    c1o.zero_margins()
    conv3x3("w_1", [(net_bf, 128), (upf_bf, 128), (mf, 128), (mfg, 128)],
            [(0, 128, c1o, 0, WS["b_1"][:, 0:1])], *ROWS["c1o"], 1, 128,
            kt_major=True)
    c2o = PadT(nc, pool, "c2o", 96, "c2o", 8)
    c2o.zero_margins()
    conv3x3("w_2", [(c1o, 128)], [(0, 96, c2o, 0, WS["b_2"][:, 0:1])],
            *ROWS["c2o"], 4, 96)
    c3o = PadT(nc, pool, "c3o", 64, "c3o", 1)
    c3o.zero_margins()
    conv3x3("w_3", [(c2o, 96)], [(0, 64, c3o, 0, WS["b_3"][:, 0:1])],
            *ROWS["c3o"], 8, 64)
    xo = PadT(nc, pool, "xo", 32, "x", 1, tag="T_mff_vtmp_xo")
    xo.zero_margins()
    conv3x3("w_4", [(c3o, 64)], [(0, 32, xo, 0, WS["b_4"][:, 0:1])],
            *ROWS["x"], 1, 32)
    _sc.__exit__(None, None, None)

    # ---- tail: single x-stack, plane deconv, up, trade, flow/mask --------
    _sc = scope("tail"); _sc.__enter__()
    rho_lo, rho_hi = RHO
    nrho = rho_hi - rho_lo
    # xs_all[ (dx+1)*32+ci, r, c ] = x[ci, rho_lo-1+r, c+dx]; rows [-1, 34)
    xs_all = pool.tile([96, (nrho + 2) * W], BF16, name="xs_all",
                       tag="T_corr_dsum_xs")
    for dxi, dx in enumerate((-1, 0, 1)):
        for q0, q1 in chunks(rho_lo - 1, rho_hi + 1, 9):
            o0 = (q0 - rho_lo + 1) * W
            nc.sync.dma_start(
                xs_all[32 * dxi:32 * (dxi + 1),
                       o0:o0 + (q1 - q0) * W].rearrange("c (r w) -> c r w", w=W),
                xo.ap(q0, q1, 0, dx))

    def xs_rhs(r0, r1, dy):
        o = (r0 + dy + 1 - rho_lo) * W
        return xs_all[:, o:o + (r1 - r0) * W]

    # deconv -> parity-major planes up_pl [ (a*2+b)*16+co, rho*W ]
    up_pl = pool.tile([64, nrho * W], BF16, name="up_pl", tag="T_f1s_upl")
    for r0, r1 in chunks(rho_lo, rho_hi, 8):
        o0 = (r0 - rho_lo) * W
        nn_ = (r1 - r0) * W
        ps = ps_conv.tile([64, nn_], F32, tag="ps_conv")
        for dyi, dy in enumerate((-1, 0, 1)):
            nc.tensor.matmul(ps[0:64, :nn_],
                             lhsT=WS["w_dc"][:, 64 * dyi:64 * (dyi + 1)],
                             rhs=xs_rhs(r0, r1, dy),
                             start=(dyi == 0), stop=(dyi == 2))
        nc.scalar.activation(up_pl[0:64, o0:o0 + nn_], ps[0:64, :nn_], AF.Prelu,
                             bias=WS["b_dc"][:, 0:1], alpha=0.1)

    # up_t (interleaved + 1-px margins) via plane-extract DMA + DVE interleave
    up_rows = UP[1] - UP[0]
    up_t = pool.tile([16, up_rows + 2, 130], BF16, name="up_t", tag="T_c1a_upt")
    nc.vector.memset(up_t[:, 0:1, :], 0.0)
    nc.vector.memset(up_t[:, up_rows + 1:up_rows + 2, :], 0.0)
    nc.vector.memset(up_t[:, 1:up_rows + 1, 0:1], 0.0)
    nc.vector.memset(up_t[:, 1:up_rows + 1, 129:130], 0.0)
    nh = (nrho + 3) // 4
    for half in range(4):
        q0 = rho_lo + nh * half
        q1 = min(rho_hi, q0 + nh)
        nq = (q1 - q0) * W
        up_q = pool.tile([16, 4 * nh * W], BF16, name=f"up_q{half}",
                         tag="T_k_upq" if half % 2 == 0 else "T_selp1")
        for par in range(4):
            nc.scalar.dma_start(
                up_q[0:16, par * nq:(par + 1) * nq],
                up_pl[16 * par:16 * (par + 1),
                      (q0 - rho_lo) * W:(q0 - rho_lo) * W + nq])
        for a in range(2):
            for b in range(2):
                par = a * 2 + b
                lr = 1 + 2 * q0 + a - UP[0]
                nr = sum(1 for rho in range(q0, q1)
                         if 0 <= 2 * rho + a - UP[0] < up_rows)
                dst = up_t[0:16, lr:lr + 2 * nr:2, 1 + b:1 + b + 128:2]
                src = up_q[0:16, par * nq:par * nq + nr * W].rearrange(
                    "c (r w) -> c r w", w=W)
                if par % 2 == 0:
                    nc.vector.tensor_copy(dst, src)
                else:
                    nc.scalar.activation(dst, src, AF.Copy)

    for r0, r1 in chunks(0, 32, 8):
        nn_ = (r1 - r0) * W
        ps = ps_conv.tile([3, nn_], F32, tag="ps_conv")
        for dyi, dy in enumerate((-1, 0, 1)):
            nc.tensor.matmul(ps[0:3, :nn_],
                             lhsT=WS["w_fm"][:, 3 * dyi:3 * (dyi + 1)],
                             rhs=xs_rhs(r0, r1, dy),
                             start=(dyi == 0), stop=(dyi == 2))
        st = stage.tile([3, nn_], F32, tag="trstage")
        nc.scalar.activation(st[0:3, :nn_], ps[0:3, :nn_], AF.Identity,
                             bias=WS["b_fm"][:, 0:1])
        nc.gpsimd.dma_start(P["flow_out"][:, r0 * W:r1 * W], st[0:2, :nn_])
        nc.gpsimd.dma_start(P["mask_out"][:, r0 * W:r1 * W], st[2:3, :nn_])

    tr_lo, tr_hi = UP_OUT
    for half in range(2):
        h0 = tr_lo + 32 * half
        h1 = h0 + 32
        upsa = pool.tile([128, 32 * 128], BF16, name=f"upsa{half}",
                         tag="T_c1b_upsa" if half == 0 else "T_cf1_c2o")
        for ty in range(3):
            for tx in range(3):
                t = ty * 3 + tx
                if t == 8:
                    continue
                nc.sync.dma_start(
                    upsa[16 * t:16 * (t + 1), :].rearrange("c (r w) -> c r w", w=128),
                    up_t[:, h0 - UP[0] + ty:h0 - UP[0] + ty + 32, tx:tx + 128])
        for r0, r1 in chunks(h0, h1, 4):
            o0 = (r0 - h0) * 128
            nn_ = (r1 - r0) * 128
            ps = ps_conv.tile([64, nn_], F32, tag="ps_conv")
            nc.tensor.matmul(ps[0:64, :nn_], lhsT=WS["w_tr_a"][:, :],
                             rhs=upsa[:, o0:o0 + nn_], start=True, stop=False)
            nc.tensor.matmul(ps[0:64, :nn_], lhsT=WS["w_tr_b"][:, :],
                             rhs=up_t[0:16, r0 - UP[0] + 2:r1 - UP[0] + 2, 2:130],
                             start=False, stop=True)
            st = stage.tile([64, nn_], F32, tag="trstage")
            nc.scalar.activation(st[0:64, :nn_], ps[0:64, :nn_], AF.Identity,
                                 bias=WS["b_tr"][:, 0:1])
            nc.sync.dma_start(
                P["tradeoff_out"][:, (r0 - tr_lo) * 128:(r0 - tr_lo) * 128 + nn_],
                st[0:64, :nn_])

    for r0, r1 in chunks(*UP_OUT, 4):
        st = stage.tile([16, (r1 - r0) * 128], F32, tag="upstage")
        nc.vector.tensor_copy(
            st[0:16, :].rearrange("c (r w) -> c r w", w=128),
            up_t[:, 1 + r0 - UP[0]:1 + r1 - UP[0], 1:129])
        nc.gpsimd.dma_start(
            P["up_out"][:, (r0 - UP_OUT[0]) * 128:(r1 - UP_OUT[0]) * 128],
            st[0:16, :])

    _sc.__exit__(None, None, None)

    ctx.close()


# ---------------------------------------------------------------------------
# Host-side sharding / gather


_NC_CACHE = None
_W_CACHE = {}


def _get_nc():
    global _NC_CACHE
    if _NC_CACHE is None:
        _NC_CACHE = build_nc()
    return _NC_CACHE


def build_f1stack(flow_b, flip):
    """Tap-shifted flow copies for the K-stacked convf1: [98, f1rows*W]."""
    v = np.asarray(flow_b, np.float32)
    if flip:
        v = v[:, ::-1, :]
    lo, hi = ROWS["f1out"]
    out = np.zeros((98, hi - lo, W), np.float32)
    for ty in range(7):
        for tx in range(7):
            t = ty * 7 + tx
            r0 = max(lo, -(ty - 3))
            r1 = min(hi, 64 - (ty - 3))
            c0 = max(0, -(tx - 3))
            c1 = min(W, W - (tx - 3))
            if r0 >= r1 or c0 >= c1:
                continue
            out[2 * t:2 * t + 2, r0 - lo:r1 - lo, c0:c1] = \
                v[:, r0 + ty - 3:r1 + ty - 3, c0 + tx - 3:c1 + tx - 3]
    return np.ascontiguousarray(out.reshape(98, -1)).astype(ml_dtypes.bfloat16)


def prepare_in_maps(inputs):
    w0 = prep_weights(inputs, flip=False)
    w1 = prep_weights(inputs, flip=True)
    sel = {
        0: (np.zeros((128, 1), np.float32), np.ones((128, 1), np.float32)),
        1: (np.ones((128, 1), np.float32), np.zeros((128, 1), np.float32)),
    }
    in_maps = []
    for c in range(N_CORES):
        b, h = c // 2, c % 2
        flip = h == 1

        def rows(a, key):
            lo, hi = ROWS[key]
            v = a[:, ::-1, :] if flip else a
            return np.ascontiguousarray(v[:, lo:hi, :], dtype=np.float32).reshape(a.shape[0], -1)

        x1 = inputs["x1"][b]
        v = x1[NET:]
        v = v[:, ::-1, :] if flip else v
        m = {"inp": np.ascontiguousarray(v, dtype=np.float32).reshape(128, HW)}
        m["net"] = rows(x1[:NET], "net")
        m["upf"] = rows(inputs["upfeat"][b], "net")
        m["corr"] = rows(inputs["corr"][b], "corr")
        m["f1stk"] = build_f1stack(inputs["flow"][b], flip)
        wd = w1 if flip else w0
        for k, v2 in wd.items():
            m[k] = v2
        m["sel0"], m["sel1"] = sel[h]
        in_maps.append(m)
    return in_maps


def assemble_outputs(results):
    B = 4
    tradeoff = np.zeros((B, 64, 128, 128), np.float32)
    up = np.zeros((B, 16, 128, 128), np.float32)
    flow_out = np.zeros((B, 2, 64, 64), np.float32)
    mask = np.zeros((B, 1, 64, 64), np.float32)
    for c in range(N_CORES):
        b, h = c // 2, c % 2
        r = results[c]
        tr = r["tradeoff_out"].reshape(64, 64, 128)
        u = r["up_out"].reshape(16, 64, 128)
        fo = r["flow_out"].reshape(2, 32, 64)
        mo = r["mask_out"].reshape(1, 32, 64)
        if h == 1:
            tr, u, fo, mo = tr[:, ::-1], u[:, ::-1], fo[:, ::-1], mo[:, ::-1]
            tradeoff[b, :, 64:128] = tr
            up[b, :, 64:128] = u
            flow_out[b, :, 32:64] = fo
            mask[b, :, 32:64] = mo
        else:
            tradeoff[b, :, 0:64] = tr
            up[b, :, 0:64] = u
            flow_out[b, :, 0:32] = fo
            mask[b, :, 0:32] = mo
    return (tradeoff, up, flow_out, mask)


def run(inputs, trace=False, **kw):
    nc = _get_nc()
    in_maps = prepare_in_maps(inputs)
    res = run_bass_kernel_spmd(nc, in_maps, core_ids=list(range(N_CORES)),
                               trace=trace, **kw)
    return assemble_outputs(res.results), res


def kernel(**inputs):
    outs, _ = run(inputs)
    return outs
